# revision 3
# baseline (speedup 1.0000x reference)
"""Trainium2 Bass kernel for nn_Attention (T=2048, D=2048, H=16, Dh=128).

Tensor-parallel over heads, 2 heads per core on 8 cores. v2 schedule:
  - DMA issue order = need order: x strip 0 (quartered) -> wq -> wk ->
    cos/sin strip 0 -> wv -> x s1 -> ... -> wo -> x s3; input x streamed
    strip-major so the RMSNorm scale s[j] unblocks per strip.
  - RMSNorm: per-strip squares (ACT/DVE) + ones-matmul; s = exp(-0.5
    ln(mean+eps)); broadcast via Pool partition_broadcast (no PE);
    per-strip DRAM round trip for the [128, TT] t-tile layout (v scaling).
  - q/k^T projections from resident x^T; RoPE on DVE with s-folded tables.
  - v projected directly in [t, dh] layout (lhsT = x^T tile), evacuated
    via ACT copy with per-partition scale = s (no DMA transpose).
  - causal attention in S^T[tk,tq] layout, per-diagonal-tile trimming;
    strip 0 in bf16; strips 1-3 run PV + softmax-sum matmuls in fp8e4
    DoubleRow (two key tiles per matmul) — exp emitted straight to packed
    fp8 pairs; scores stay bf16 everywhere.
  - softmax normalization deferred: rec = exp(-ln(sum)), Pool broadcast,
    DVE multiply into outT; per-head output projection accumulated in
    PSUM; partial outputs written bf16 (summed f32 on host with residual).
"""

import math
import os
import sys
import time

for _p in ("/opt/trn_rl_repo", "/root/.axon_site/_ro/trn_rl_repo"):
    if os.path.isdir(_p) and _p not in sys.path:
        sys.path.insert(0, _p)

import numpy as np
import ml_dtypes

import concourse.bass as bass
import concourse.tile as tile
from concourse.bass import InstructionNameOrderedSet
from concourse import bacc, mybir
from concourse.bass_utils import run_bass_kernel_spmd

BF16 = mybir.dt.bfloat16
F32 = mybir.dt.float32
FP8 = mybir.dt.float8e4
AF = mybir.ActivationFunctionType

T = 2048
D = 2048
N_H = 16
D_H = 128
N_CORES = 8
H_LOC = N_H // N_CORES          # heads per core = 2
NL = H_LOC * D_H                # local head width = 256
KD = D // 128                   # contraction tiles = 16
TT = T // 128                   # t tiles = 16
NS = T // 512                   # 512-wide strips = 4
EPS = 1e-5
INV_SQRT_DH = 1.0 / math.sqrt(D_H)
FP8_EXP_BIAS = -4.0             # keeps exp() under fp8e4m3 max (saw 8.6 sigma); cancels in norm

FP8_ATT = os.environ.get('FP8_ATT', '1') == '1'                  # fp8 DoubleRow PV+sum for strips >= 1
FP8_SSQ = os.environ.get('FP8_SSQ', '1') == '1'                  # fp8 DoubleRow for sum(x^2)

_CACHED = {}
PHASES = []  # (label, first_instruction_id) — emission-order markers for sim analysis


def _build_program(repeats=1):
    if repeats in _CACHED:
        return _CACHED[repeats]

    nc = bacc.Bacc("TRN2", target_bir_lowering=False, debug=False, num_devices=N_CORES)

    xT_d = nc.dram_tensor("xT", [D, T], BF16, kind="ExternalInput")
    wq_d = nc.dram_tensor("wqT", [D, NL], BF16, kind="ExternalInput")
    wk_d = nc.dram_tensor("wkT", [D, NL], BF16, kind="ExternalInput")
    wv_d = nc.dram_tensor("wvT", [D, NL], BF16, kind="ExternalInput")
    wo_d = nc.dram_tensor("woT", [NL, T], BF16, kind="ExternalInput")
    cos_d = nc.dram_tensor("cosT", [D_H, T], BF16, kind="ExternalInput")
    sin_d = nc.dram_tensor("sinT", [D_H, T], BF16, kind="ExternalInput")
    msk_d = nc.dram_tensor("masks", [128, 128], BF16, kind="ExternalInput")
    on128_d = nc.dram_tensor("ones128", [128, 1], BF16, kind="ExternalInput")
    onedr_d = nc.dram_tensor("ones_dr", [128, 2, 16], FP8, kind="ExternalInput")
    out_d = nc.dram_tensor("out", [T, D], BF16, kind="ExternalOutput")
    DBG = os.environ.get("DBG_OUTT", "0") == "1"
    if DBG:
        outT_dbg = nc.dram_tensor("outT_dbg", [128, H_LOC, T], BF16, kind="ExternalOutput")
        su_dbg = nc.dram_tensor("su_dbg", [H_LOC, T], F32, kind="ExternalOutput")
        sk_dbg = nc.dram_tensor("sk_dbg", [128, TT], F32, kind="ExternalOutput")
        v_dbg = nc.dram_tensor("v_dbg", [128, TT, NL], BF16, kind="ExternalOutput")
        rec_dbg = nc.dram_tensor("rec_dbg", [H_LOC, T], F32, kind="ExternalOutput")
    # DRAM scratch for the s row->tile-layout round trip
    s_scr = nc.dram_tensor("s_scr", [TT, 128], F32, kind="Internal")

    ap = lambda h: h.ap()
    xT, out_ap, s_scr_ap = ap(xT_d), ap(out_d), ap(s_scr)

    from contextlib import ExitStack

    with tile.TileContext(nc) as tc, ExitStack() as ctx:
        P = ctx.enter_context  # noqa

        singles = P(tc.tile_pool(name="singles", bufs=1))
        sq = P(tc.tile_pool(name="sq", bufs=2))            # square scratch
        rope = P(tc.tile_pool(name="rope", bufs=4))        # [128,512] bf16
        qtmp = P(tc.tile_pool(name="qtmp", bufs=2 if os.environ.get("DBG_OUTT","0")=="0" else 1))        # raw qk evac copies
        epool = P(tc.tile_pool(name="epool", bufs=4 if os.environ.get("DBG_OUTT","0")=="0" else 3))      # fp8 exp pairs
        ebf = P(tc.tile_pool(name="ebf", bufs=2 if os.environ.get("DBG_OUTT","0")=="0" else 1))          # bf16 exp tiles (strip 0)
        small = P(tc.tile_pool(name="small", bufs=2))      # [1,512] f32
        bcast = P(tc.tile_pool(name="bcast", bufs=2))      # [128,512] bcast rows
        stage = P(tc.tile_pool(name="stage", bufs=3))      # [128,T] out staging
        pmm = P(tc.tile_pool(name="pmm", bufs=2, space="PSUM"))
        pvm = P(tc.tile_pool(name="pvm", bufs=1, space="PSUM"))
        psc = P(tc.tile_pool(name="psc", bufs=2, space="PSUM"))
        ppv = P(tc.tile_pool(name="ppv", bufs=2, space="PSUM"))
        psu = P(tc.tile_pool(name="psu", bufs=1, space="PSUM"))

        def mark(label):
            PHASES.append((label, nc.next_id()))



        def emit_body(rep):
            # ---------------- DMA issue (need-ordered) -------------------------
            mark("dma_issue")
            xt = singles.tile([128, KD, T], BF16, tag="xt")
            xTv = xT.rearrange("(n p) t -> p n t", p=128)

            def load_x_chunk(j, k0, k1):
                js = slice(j * 512, (j + 1) * 512)
                nc.sync.dma_start(out=xt[:, k0:k1, js], in_=xTv[:, k0:k1, js])

            def load_w(dram, tag):
                t_ = singles.tile([128, KD, NL], BF16, tag=tag)
                nc.sync.dma_start(out=t_, in_=ap(dram).rearrange("(a p) m -> p a m", p=128))
                return t_

            cosr = singles.tile([128, T], BF16, tag="cosr")
            sinr = singles.tile([128, T], BF16, tag="sinr")

            def load_cs_strip(j):
                js = slice(j * 512, (j + 1) * 512)
                nc.sync.dma_start(out=cosr[:, js], in_=ap(cos_d)[:, js])
                nc.sync.dma_start(out=sinr[:, js], in_=ap(sin_d)[:, js])

            # part A: everything needed before/while s0 resolves.  Later loads
            # are issued after ssq0's round-trip DMAs so the round trip does
            # not queue behind them on the serialized DMA engines.
            load_x_chunk(0, 0, 2)
            on128 = singles.tile([128, 1], BF16, tag="on128")
            nc.sync.dma_start(out=on128, in_=ap(on128_d))
            onedr_f = singles.tile([128, 2, 16], FP8, tag="onedr")
            nc.sync.dma_start(out=onedr_f, in_=ap(onedr_d))
            # dual-fp8 ldweights needs the pair-dim step 16B-aligned
            onedr = onedr_f[:, :, 0:1]
            wk = load_w(wk_d, "wk")
            load_x_chunk(0, 2, 9)
            load_x_chunk(0, 9, 16)
            wq = load_w(wq_d, "wq")
            wv = load_w(wv_d, "wv")
            load_cs_strip(0)
            msk = singles.tile([128, 128], BF16, tag="msk")
            nc.sync.dma_start(out=msk, in_=ap(msk_d))
            load_x_chunk(1, 0, 8)
            load_x_chunk(1, 8, 16)
            load_cs_strip(1)
            wo = singles.tile([128, H_LOC, T], BF16, tag="wo")

            def load_part_b():
                load_x_chunk(2, 0, 8)
                load_x_chunk(2, 8, 16)
                load_cs_strip(2)
                nc.sync.dma_start(
                    out=wo, in_=ap(wo_d).rearrange("(h p) t -> p h t", p=128))
                load_x_chunk(3, 0, 8)
                load_x_chunk(3, 8, 16)
                load_cs_strip(3)

            # ---------------- persistent SBUF state ----------------------------
            epsb = singles.tile([1, 1], F32, tag="epsb")
            nc.vector.memset(epsb, EPS)
            f8bias = singles.tile([128, 1], F32, tag="f8bias")
            nc.vector.memset(f8bias, FP8_EXP_BIAS)
            s_row = singles.tile([1, T], F32, tag="srow")
            lnm = singles.tile([1, T], F32, tag="lnm")
            cos_s = singles.tile([128, T], BF16, tag="cos_s")
            sin_s = singles.tile([128, T], BF16, tag="sin_s")
            sk_t = singles.tile([128, TT], F32, tag="sk")
            skx = singles.tile([128, TT], F32, tag="skx")
            q_sb = singles.tile([128, H_LOC, T], BF16, tag="q_sb")
            k_sb = singles.tile([128, H_LOC, T], BF16, tag="k_sb")
            v_sb = singles.tile([128, TT, NL], BF16, tag="v_sb")
            if FP8_ATT:
                v_dr = singles.tile([128, TT // 2, 2, NL], FP8, tag="v_dr")
            outT = singles.tile([128, H_LOC, T], BF16, tag="outT")

            # ---------------- per-strip RMSNorm sums + s pipeline ---------------
            def emit_ssq_s_strip(j):
                mark(f"ssq_s{j}")
                js = slice(j * 512, (j + 1) * 512)
                ssq = psu.tile([1, 512], F32, tag="su", name=f"ssq{j}_{rep}")
                # squares striped across ACT/DVE/Pool so no engine serializes
                sq_rot = [1, 2, 1, 2, 1, 1, 2, 1, 1, 2, 1, 1, 2, 1, 2, 1]

                def emit_square(dst, kd):
                    eng = sq_rot[kd]
                    if eng == 0:
                        nc.scalar.activation(dst, xt[:, kd, js], AF.Square)
                    else:
                        (None, nc.vector, nc.gpsimd)[eng].tensor_mul(
                            dst, xt[:, kd, js], xt[:, kd, js]
                        )

                if FP8_SSQ:
                    for p_ in range(KD // 2):
                        xsq = sq.tile([128, 2, 512], FP8, tag="xsq")
                        for m in range(2):
                            emit_square(xsq[:, m, :], 2 * p_ + m)
                        nc.tensor.matmul(
                            ssq, lhsT=onedr, rhs=xsq,
                            start=(p_ == 0), stop=(p_ == KD // 2 - 1),
                            perf_mode=mybir.MatmulPerfMode.DoubleRow,
                        )
                else:
                    for kd in range(KD):
                        xsq = sq.tile([128, 512], BF16, tag="xsq")
                        emit_square(xsq, kd)
                        nc.tensor.matmul(
                            ssq, lhsT=on128, rhs=xsq,
                            start=(kd == 0), stop=(kd == KD - 1),
                        )
                # lnm = ln(mean + eps); s = exp(-0.5 lnm)
                nc.scalar.activation(lnm[:, js], ssq, AF.Ln, bias=epsb, scale=1.0 / D)
                nc.scalar.activation(s_row[:, js], lnm[:, js], AF.Exp, scale=-0.5)
                # round-trip for the [128, 4] t-tile layout slice (v scaling +
                # k-side s folded into the exp scale)
                rt_out = nc.sync.dma_start(
                    out=s_scr_ap[4 * j : 4 * (j + 1), :].rearrange("i p -> () (i p)"),
                    in_=s_row[:, js],
                )
                rt_in = nc.sync.dma_start(
                    out=sk_t[:, 4 * j : 4 * (j + 1)],
                    in_=s_scr_ap.rearrange("i p -> p i")[:, 4 * j : 4 * (j + 1)],
                )
                # DRAM deps are invisible to Tile: force read-after-write
                d1 = InstructionNameOrderedSet(); d1.add(rt_out.ins.name)
                rt_in.ins.add_sync_dependencies_from(d1)
                nc.vector.tensor_scalar_mul(
                    skx[:, 4 * j : 4 * (j + 1)], sk_t[:, 4 * j : 4 * (j + 1)],
                    INV_SQRT_DH,
                )

            def emit_cos_fold(j):
                mark(f"cosf{j}")
                js = slice(j * 512, (j + 1) * 512)
                sb = bcast.tile([128, 512], F32, tag="sb")
                nc.gpsimd.partition_broadcast(sb, s_row[:, js])
                nc.vector.tensor_mul(cos_s[:, js], cosr[:, js], sb)
                nc.vector.tensor_mul(sin_s[:, js], sinr[:, js], sb)

            # ---------------- projections --------------------------------------
            def emit_qk_strip(h, j, dst, w, ctab, stab):
                # q uses the s-folded tables; k uses raw tables (its s is
                # folded into the exp scale instead, so k never waits on s).
                mark(f"{'q' if dst is q_sb else 'k'}{j}h{h}")
                hs = slice(h * 128, (h + 1) * 128)
                js = slice(j * 512, (j + 1) * 512)
                ps = pmm.tile([128, 512], F32, tag="mm")
                for kd in range(KD):
                    nc.tensor.matmul(
                        ps, lhsT=w[:, kd, hs], rhs=xt[:, kd, js],
                        start=(kd == 0), stop=(kd == KD - 1),
                    )
                # m2's half-swap must read PSUM (cross-partition SBUF reads
                # are illegal); the aligned m1 path goes through an ACT copy so
                # the DVE muls get 2x mode and the psum frees quickly.
                qc = qtmp.tile([128, 512], BF16, tag="qc")
                nc.scalar.copy(qc, ps)
                m2 = rope.tile([128, 512], BF16, tag="m2")
                nc.vector.tensor_mul(m2[0:64, :], ps[64:128, :], stab[0:64, js])
                nc.vector.tensor_mul(m2[64:128, :], ps[0:64, :], stab[64:128, js])
                m1 = rope.tile([128, 512], BF16, tag="m1")
                nc.vector.tensor_mul(m1, qc, ctab[:, js])
                nc.gpsimd.tensor_add(dst[:, h, js], m1, m2)

            def emit_v_tile(tt):
                # v[t, dh] directly: lhsT = x^T tile, rhs = wv[d, nl]
                mark(f"v{tt}")
                ts = slice(tt * 128, (tt + 1) * 128)
                if tt % 2 == 0:
                    ps = pvm.tile([128, NL], F32, tag="vmm")
                else:
                    ps = pmm.tile([128, NL], F32, tag="mm", name="vps")
                for kd in range(KD):
                    nc.tensor.matmul(
                        ps, lhsT=xt[:, kd, ts], rhs=wv[:, kd, :],
                        start=(kd == 0), stop=(kd == KD - 1),
                    )
                nc.scalar.mul(v_sb[:, tt, :], ps, sk_t[:, tt : tt + 1])
                if FP8_ATT:
                    nc.scalar.mul(
                        v_dr[:, tt // 2, tt % 2, :], ps, sk_t[:, tt : tt + 1]
                    )

            # ---------------- attention ----------------------------------------
            def emit_attention_bf16(h, Q0, W, filler):
                mark(f"att{h}_q{Q0}")
                hs = slice(h * 128, (h + 1) * 128)
                ntk = (Q0 + W) // 128
                po = ppv.tile([128, 512], F32, tag="pv", name="po")[:, :W]
                su = psu.tile([1, 512], F32, tag="su", name="su")[:, :W]
                for i in range(ntk):
                    cb = 128 * i - Q0
                    c0 = max(cb, 0)
                    cs = slice(c0, W)
                    qs = slice(Q0 + c0, Q0 + W)
                    st = psc.tile([128, 512], F32, tag="sc")
                    nc.tensor.matmul(
                        st[:, cs], lhsT=k_sb[:, h, i * 128 : (i + 1) * 128],
                        rhs=q_sb[:, h, qs], start=True, stop=True,
                    )
                    e = ebf.tile([128, 512], BF16, tag="e")
                    nc.scalar.activation(e[:, cs], st[:, cs], AF.Exp,
                                         scale=skx[:, i : i + 1])
                    if cb >= 0:
                        nc.gpsimd.tensor_mul(
                            e[:, cb : cb + 128], e[:, cb : cb + 128], msk
                        )
                    if filler:
                        filler.pop(0)()
                    nc.tensor.matmul(
                        po[:, cs], lhsT=v_sb[:, i, hs], rhs=e[:, cs],
                        start=(i == 0), stop=(i == ntk - 1),
                    )
                    nc.tensor.matmul(
                        su[:, cs], lhsT=on128, rhs=e[:, cs],
                        start=(i == 0), stop=(i == ntk - 1),
                    )
                emit_epilogue(h, Q0, W, po, su)

            def emit_attention_fp8(h, Q0, W, filler, tail_hook=None):
                mark(f"att{h}_q{Q0}f8")
                hs = slice(h * 128, (h + 1) * 128)
                npair = (Q0 + W) // 256
                po = ppv.tile([128, 512], F32, tag="pv", name="po")[:, :W]
                su = psu.tile([1, 512], F32, tag="su", name="su")[:, :W]
                for p_ in range(npair):
                    i0 = 2 * p_
                    c0 = max(128 * i0 - Q0, 0)       # pair-wide col start
                    cs = slice(c0, W)
                    e = epool.tile([128, 2, 512], FP8, tag="edr")
                    for m in range(2):
                        i = i0 + m
                        cb = 128 * i - Q0
                        cm = max(cb, 0)              # member col start
                        st = psc.tile([128, 512], F32, tag="sc")
                        nc.tensor.matmul(
                            st[:, cm:W],
                            lhsT=k_sb[:, h, i * 128 : (i + 1) * 128],
                            rhs=q_sb[:, h, Q0 + cm : Q0 + W],
                            start=True, stop=True,
                        )
                        nc.scalar.activation(
                            e[:, m, cm:W], st[:, cm:W], AF.Exp,
                            bias=f8bias[:, 0:1], scale=skx[:, i : i + 1],
                        )
                        if cm > c0:
                            nc.gpsimd.memset(e[:, m, c0:cm], 0)
                        if cb >= 0 and cb < W:
                            nc.gpsimd.tensor_mul(
                                e[:, m, cb : cb + 128], e[:, m, cb : cb + 128], msk
                            )
                    if filler:
                        filler.pop(0)()
                    nc.tensor.matmul(
                        po[:, cs], lhsT=v_dr[:, p_, :, hs], rhs=e[:, :, cs],
                        start=(p_ == 0), stop=(p_ == npair - 1),
                        perf_mode=mybir.MatmulPerfMode.DoubleRow,
                    )
                    nc.tensor.matmul(
                        su[:, cs], lhsT=onedr, rhs=e[:, :, cs],
                        start=(p_ == 0), stop=(p_ == npair - 1),
                        perf_mode=mybir.MatmulPerfMode.DoubleRow,
                    )
                    if filler:
                        filler.pop(0)()
                    if tail_hook is not None and p_ == npair - 2:
                        emit_epilogue_piece(h, Q0, po, su, 0, W - 256)
                        tail_hook()
                if tail_hook is not None:
                    emit_epilogue_piece(h, Q0, po, su, W - 256, W)
                else:
                    emit_epilogue(h, Q0, W, po, su)

            def emit_epilogue_piece(h, Q0, po, su, c0, c1):
                mark(f"epp{h}_q{Q0}_{c0}")
                rec = small.tile([1, 512], F32, tag="rec", name="rec")[:, c0:c1]
                nc.vector.reciprocal_approx_fast(rec, su[:, c0:c1])
                rb = bcast.tile([128, 512], F32, tag="rb", name="rb")[:, c0:c1]
                nc.gpsimd.partition_broadcast(rb, rec)
                nc.vector.tensor_mul(outT[:, h, Q0 + c0 : Q0 + c1], po[:, c0:c1], rb)

            def emit_epilogue(h, Q0, W, po, su):
                mark(f"epi{h}_q{Q0}")
                rec = small.tile([1, 512], F32, tag="rec", name="rec")[:, :W]
                nc.vector.reciprocal_approx_fast(rec, su)
                rb = bcast.tile([128, 512], F32, tag="rb", name="rb")[:, :W]
                nc.gpsimd.partition_broadcast(rb, rec)
                nc.vector.tensor_mul(outT[:, h, Q0 : Q0 + W], po, rb)
                if DBG:
                    sud = small.tile([1, 512], F32, tag="sud", name="sud")[:, :W]
                    nc.vector.tensor_copy(sud, su)
                    nc.sync.dma_start(out=su_dbg.ap()[h : h + 1, Q0 : Q0 + W], in_=sud)
                    nc.sync.dma_start(out=rec_dbg.ap()[h : h + 1, Q0 : Q0 + W], in_=rec)

            def emit_attention(h, Q0, W, filler, tail_hook=None):
                if FP8_ATT and Q0 >= 512:
                    emit_attention_fp8(h, Q0, W, filler, tail_hook)
                else:
                    emit_attention_bf16(h, Q0, W, filler)

            # ---------------- output projection --------------------------------
            def make_wo_chunk(tt, n, stg, pool, tag, evac):
                ts = slice(tt * 128, (tt + 1) * 128)
                ns = slice(n * 512, (n + 1) * 512)

                def emit():
                    mark(f"wo_t{tt}n{n}")
                    ps = pool.tile([128, 512], F32, tag=tag)
                    for h in range(H_LOC):
                        nc.tensor.matmul(
                            ps, lhsT=outT[:, h, ts], rhs=wo[:, h, ns],
                            start=(h == 0), stop=(h == H_LOC - 1),
                        )
                    if evac is nc.scalar:
                        nc.scalar.copy(stg[:, ns], ps)
                    else:
                        evac.tensor_copy(stg[:, ns], ps)
                    if tt >= TT - 4:
                        if n % 2 == 1:
                            hs_ = slice((n - 1) * 512, (n + 1) * 512)
                            nc.sync.dma_start(out=out_ap[ts, hs_], in_=stg[:, hs_])
                    elif n == NS - 1:
                        nc.sync.dma_start(out=out_ap[ts, :], in_=stg)

                return emit

            def wo_chunks_range(tt0, tt1, rotate=False, evacs=None):
                out = []
                rot = [(pmm, "mm"), (ppv, "pv"), (psc, "sc")] if rotate else [(pmm, "mm")]
                evacs = evacs or [nc.vector, nc.scalar]
                k = 0
                for tt in range(tt0, tt1):
                    stg = stage.tile([128, T], BF16, tag="stg", name=f"stg{tt}_{rep}")
                    for n in range(NS):
                        pool, tag = rot[k % len(rot)]
                        out.append(make_wo_chunk(tt, n, stg, pool, tag,
                                                 evacs[k % len(evacs)]))
                        k += 1
                return out

            # ---------------- schedule -----------------------------------------
            # Per strip: attention j immediately after strip-j projections;
            # strip j+1's ssq/k/q/v work follows (matching x DMA arrival).
            # ACT order stays exps(j) before squares(j+1).
            def emit_kqv_slot(jn):
                for h in range(H_LOC):
                    emit_qk_strip(h, jn, k_sb, wk, cosr, sinr)
                emit_cos_fold(jn)
                for h in range(H_LOC):
                    emit_qk_strip(h, jn, q_sb, wq, cos_s, sin_s)
                for tt in range(4 * jn, 4 * (jn + 1)):
                    emit_v_tile(tt)

            emit_ssq_s_strip(0)
            load_part_b()
            emit_kqv_slot(0)
            for j in range(NS - 1):
                ev = [nc.vector] if j >= 2 else [nc.vector, nc.vector, nc.scalar]
                filler = wo_chunks_range(4 * (j - 1), 4 * j, evacs=ev) if j >= 1 else []
                half = len(filler) // 2
                fa, fb = filler[:half], filler[half:]
                emit_attention(0, 512 * j, 512, fa)
                emit_attention(1, 512 * j, 512, fb)
                for f in fa + fb:
                    f()
                if j == 0:
                    emit_ssq_s_strip(1)
                emit_kqv_slot(j + 1)
                if j + 2 < NS:
                    emit_ssq_s_strip(j + 2)
            filler = wo_chunks_range(8, 12, evacs=[nc.vector])
            fa, fb = filler[:6], filler[6:]
            emit_attention(0, 1536, 512, fa)

            TAIL_HOOK = os.environ.get("TAIL_HOOK", "1") == "1"

            def tail_hook():
                for f in wo_chunks_range(12, 14, rotate=True):
                    f()

            emit_attention(1, 1536, 512, fb,
                           tail_hook=tail_hook if TAIL_HOOK else None)
            for f in fa + fb:
                f()
            for f in wo_chunks_range(14, 16 if TAIL_HOOK else 12, rotate=True):
                f()
            if not TAIL_HOOK:
                for f in wo_chunks_range(12, 16, rotate=True):
                    f()

        for _rep in range(repeats):
            emit_body(_rep)

    # Force Exp and Ln onto the single combined table set so the table-load
    # pass emits one ACT_TABLE_LOAD for the whole kernel.
    from concourse.hw_specs import get_activation_tables
    tabs = get_activation_tables(nc.m.arch)
    for nm_, fs_ in tabs.items():
        if nm_ != "natural_log_exp_and_others":
            fs_.discard(AF.Exp)
            fs_.discard(AF.Ln)
    nc.compile()
    _CACHED[repeats] = nc
    return nc


def _host_prep(x, w_ln, wq, wk, wv, wo, cos, sin):
    bf = ml_dtypes.bfloat16
    f8 = mybir.dt.np(FP8)
    x = np.asarray(x, np.float32)
    w_ln = np.asarray(w_ln, np.float32)
    cosT = np.ascontiguousarray(np.asarray(cos, np.float32).T).astype(bf)
    sinTf = np.ascontiguousarray(np.asarray(sin, np.float32).T)
    sinTf[0:64] *= -1.0          # rotate_half sign folded into the table
    sinT = sinTf.astype(bf)
    xT = np.ascontiguousarray(x.T).astype(bf)

    # causal boundary mask for diagonal tiles: mask[p, f] = 1 if f >= p
    f = np.arange(128)[None, :]
    p = np.arange(128)[:, None]
    masks = (f >= p).astype(bf)

    ones128 = np.ones((128, 1), bf)
    ones_dr = np.ones((128, 2, 16), f8)

    wq_s = (np.asarray(wq, np.float32) * w_ln[None, :])
    wk_s = (np.asarray(wk, np.float32) * w_ln[None, :])
    wv_s = (np.asarray(wv, np.float32) * w_ln[None, :])
    wo32 = np.asarray(wo, np.float32)

    in_maps = []
    for c in range(N_CORES):
        sl = slice(c * NL, (c + 1) * NL)
        in_maps.append({
            "xT": xT,
            "wqT": np.ascontiguousarray(wq_s[sl].T).astype(bf),
            "wkT": np.ascontiguousarray(wk_s[sl].T).astype(bf),
            "wvT": np.ascontiguousarray(wv_s[sl].T).astype(bf),
            "woT": np.ascontiguousarray(wo32[:, sl].T).astype(bf),
            "cosT": cosT,
            "sinT": sinT,
            "masks": masks,
            "ones128": ones128,
            "ones_dr": ones_dr,
        })
    return in_maps


def kernel(x, w_ln, wq, wk, wv, wo, cos, sin):
    nc = _build_program()
    in_maps = _host_prep(x, w_ln, wq, wk, wv, wo, cos, sin)
    t0 = time.time()
    res = run_bass_kernel_spmd(nc, in_maps, core_ids=list(range(N_CORES)))
    t1 = time.time()
    print(f"run_bass_kernel_spmd wall: {(t1 - t0) * 1e3:.1f} ms", file=sys.stderr)
    acc = np.zeros((T, D), np.float32)
    for r in res.results:
        acc += np.asarray(r["out"], np.float32)
    return np.asarray(x, np.float32) + acc


# revision 4
# speedup vs baseline: 1.0149x; 1.0149x over previous
"""Trainium2 Bass kernel for nn_Attention (T=2048, D=2048, H=16, Dh=128).

Tensor-parallel over heads, 2 heads per core on 8 cores. v2 schedule:
  - DMA issue order = need order: x strip 0 (quartered) -> wq -> wk ->
    cos/sin strip 0 -> wv -> x s1 -> ... -> wo -> x s3; input x streamed
    strip-major so the RMSNorm scale s[j] unblocks per strip.
  - RMSNorm: per-strip squares (ACT/DVE) + ones-matmul; s = exp(-0.5
    ln(mean+eps)); broadcast via Pool partition_broadcast (no PE);
    per-strip DRAM round trip for the [128, TT] t-tile layout (v scaling).
  - q/k^T projections from resident x^T; RoPE on DVE with s-folded tables.
  - v projected directly in [t, dh] layout (lhsT = x^T tile), evacuated
    via ACT copy with per-partition scale = s (no DMA transpose).
  - causal attention in S^T[tk,tq] layout, per-diagonal-tile trimming;
    strip 0 in bf16; strips 1-3 run PV + softmax-sum matmuls in fp8e4
    DoubleRow (two key tiles per matmul) — exp emitted straight to packed
    fp8 pairs; scores stay bf16 everywhere.
  - softmax normalization deferred: rec = exp(-ln(sum)), Pool broadcast,
    DVE multiply into outT; per-head output projection accumulated in
    PSUM; partial outputs written bf16 (summed f32 on host with residual).
"""

import math
import os
import sys
import time

for _p in ("/opt/trn_rl_repo", "/root/.axon_site/_ro/trn_rl_repo"):
    if os.path.isdir(_p) and _p not in sys.path:
        sys.path.insert(0, _p)

import numpy as np
import ml_dtypes

import concourse.bass as bass
import concourse.tile as tile
from concourse.bass import InstructionNameOrderedSet
from concourse import bacc, mybir
from concourse.bass_utils import run_bass_kernel_spmd

BF16 = mybir.dt.bfloat16
F32 = mybir.dt.float32
FP8 = mybir.dt.float8e4
AF = mybir.ActivationFunctionType

T = 2048
D = 2048
N_H = 16
D_H = 128
N_CORES = 8
H_LOC = N_H // N_CORES          # heads per core = 2
NL = H_LOC * D_H                # local head width = 256
KD = D // 128                   # contraction tiles = 16
TT = T // 128                   # t tiles = 16
NS = T // 512                   # 512-wide strips = 4
EPS = 1e-5
INV_SQRT_DH = 1.0 / math.sqrt(D_H)
FP8_EXP_BIAS = -4.0             # keeps exp() under fp8e4m3 max (saw 8.6 sigma); cancels in norm

FP8_ATT = os.environ.get('FP8_ATT', '1') == '1'                  # fp8 DoubleRow PV+sum for strips >= 1
FP8_SSQ = os.environ.get('FP8_SSQ', '1') == '1'                  # fp8 DoubleRow for sum(x^2)

_CACHED = {}
PHASES = []  # (label, first_instruction_id) — emission-order markers for sim analysis


def _build_program(repeats=1):
    if repeats in _CACHED:
        return _CACHED[repeats]

    nc = bacc.Bacc("TRN2", target_bir_lowering=False, debug=False, num_devices=N_CORES)

    xT_d = nc.dram_tensor("xT", [D, T], BF16, kind="ExternalInput")
    wq_d = nc.dram_tensor("wqT", [D, NL], BF16, kind="ExternalInput")
    wk_d = nc.dram_tensor("wkT", [D, NL], BF16, kind="ExternalInput")
    wv_d = nc.dram_tensor("wvT", [D, NL], BF16, kind="ExternalInput")
    wo_d = nc.dram_tensor("woT", [NL, T], BF16, kind="ExternalInput")
    cos_d = nc.dram_tensor("cosT", [D_H, T], BF16, kind="ExternalInput")
    sin_d = nc.dram_tensor("sinT", [D_H, T], BF16, kind="ExternalInput")
    msk_d = nc.dram_tensor("masks", [128, 128], BF16, kind="ExternalInput")
    on128_d = nc.dram_tensor("ones128", [128, 1], BF16, kind="ExternalInput")
    onedr_d = nc.dram_tensor("ones_dr", [128, 2, 16], FP8, kind="ExternalInput")
    out_d = nc.dram_tensor("out", [T, D], BF16, kind="ExternalOutput")
    DBG = os.environ.get("DBG_OUTT", "0") == "1"
    if DBG:
        outT_dbg = nc.dram_tensor("outT_dbg", [128, H_LOC, T], BF16, kind="ExternalOutput")
        su_dbg = nc.dram_tensor("su_dbg", [H_LOC, T], F32, kind="ExternalOutput")
        sk_dbg = nc.dram_tensor("sk_dbg", [128, TT], F32, kind="ExternalOutput")
        v_dbg = nc.dram_tensor("v_dbg", [128, TT, NL], BF16, kind="ExternalOutput")
        rec_dbg = nc.dram_tensor("rec_dbg", [H_LOC, T], F32, kind="ExternalOutput")
    # DRAM scratch for the s row->tile-layout round trip
    s_scr = nc.dram_tensor("s_scr", [TT, 128], F32, kind="Internal")

    ap = lambda h: h.ap()
    xT, out_ap, s_scr_ap = ap(xT_d), ap(out_d), ap(s_scr)

    from contextlib import ExitStack

    with tile.TileContext(nc) as tc, ExitStack() as ctx:
        P = ctx.enter_context  # noqa

        singles = P(tc.tile_pool(name="singles", bufs=1))
        sq = P(tc.tile_pool(name="sq", bufs=2))            # square scratch
        rope = P(tc.tile_pool(name="rope", bufs=4))        # [128,512] bf16
        qtmp = P(tc.tile_pool(name="qtmp", bufs=2 if os.environ.get("DBG_OUTT","0")=="0" else 1))        # raw qk evac copies
        epool = P(tc.tile_pool(name="epool", bufs=4 if os.environ.get("DBG_OUTT","0")=="0" else 3))      # fp8 exp pairs
        ebf = P(tc.tile_pool(name="ebf", bufs=2 if os.environ.get("DBG_OUTT","0")=="0" else 1))          # bf16 exp tiles (strip 0)
        small = P(tc.tile_pool(name="small", bufs=2))      # [1,512] f32
        bcast = P(tc.tile_pool(name="bcast", bufs=2))      # [128,512] bcast rows
        stage = P(tc.tile_pool(name="stage", bufs=3))      # [128,T] out staging
        pmm = P(tc.tile_pool(name="pmm", bufs=2, space="PSUM"))
        pvm = P(tc.tile_pool(name="pvm", bufs=1, space="PSUM"))
        psc = P(tc.tile_pool(name="psc", bufs=2, space="PSUM"))
        ppv = P(tc.tile_pool(name="ppv", bufs=2, space="PSUM"))
        psu = P(tc.tile_pool(name="psu", bufs=1, space="PSUM"))

        def mark(label):
            PHASES.append((label, nc.next_id()))



        def emit_body(rep):
            # ---------------- DMA issue (need-ordered) -------------------------
            mark("dma_issue")
            xt = singles.tile([128, KD, T], BF16, tag="xt")
            xTv = xT.rearrange("(n p) t -> p n t", p=128)

            def load_x_chunk(j, k0, k1):
                js = slice(j * 512, (j + 1) * 512)
                nc.sync.dma_start(out=xt[:, k0:k1, js], in_=xTv[:, k0:k1, js])

            def load_w(dram, tag):
                t_ = singles.tile([128, KD, NL], BF16, tag=tag)
                nc.sync.dma_start(out=t_, in_=ap(dram).rearrange("(a p) m -> p a m", p=128))
                return t_

            cosr = singles.tile([128, T], BF16, tag="cosr")
            sinr = singles.tile([128, T], BF16, tag="sinr")

            def load_cs_strip(j):
                js = slice(j * 512, (j + 1) * 512)
                nc.sync.dma_start(out=cosr[:, js], in_=ap(cos_d)[:, js])
                nc.sync.dma_start(out=sinr[:, js], in_=ap(sin_d)[:, js])

            # part A: everything needed before/while s0 resolves.  Later loads
            # are issued after ssq0's round-trip DMAs so the round trip does
            # not queue behind them on the serialized DMA engines.
            load_x_chunk(0, 0, 2)
            on128 = singles.tile([128, 1], BF16, tag="on128")
            nc.sync.dma_start(out=on128, in_=ap(on128_d))
            onedr_f = singles.tile([128, 2, 16], FP8, tag="onedr")
            nc.sync.dma_start(out=onedr_f, in_=ap(onedr_d))
            # dual-fp8 ldweights needs the pair-dim step 16B-aligned
            onedr = onedr_f[:, :, 0:1]
            wk = load_w(wk_d, "wk")
            load_x_chunk(0, 2, 9)
            load_x_chunk(0, 9, 16)
            load_cs_strip(0)
            wq = load_w(wq_d, "wq")
            wv = load_w(wv_d, "wv")
            msk = singles.tile([128, 128], BF16, tag="msk")
            nc.sync.dma_start(out=msk, in_=ap(msk_d))
            load_cs_strip(1)
            load_x_chunk(1, 0, 8)
            load_x_chunk(1, 8, 16)
            wo = singles.tile([128, H_LOC, T], BF16, tag="wo")

            def load_part_b():
                load_x_chunk(2, 0, 8)
                load_x_chunk(2, 8, 16)
                load_cs_strip(2)
                nc.sync.dma_start(
                    out=wo, in_=ap(wo_d).rearrange("(h p) t -> p h t", p=128))
                load_x_chunk(3, 0, 8)
                load_x_chunk(3, 8, 16)
                load_cs_strip(3)

            # ---------------- persistent SBUF state ----------------------------
            epsb = singles.tile([1, 1], F32, tag="epsb")
            nc.vector.memset(epsb, EPS)
            f8bias = singles.tile([128, 1], F32, tag="f8bias")
            nc.vector.memset(f8bias, FP8_EXP_BIAS)
            s_row = singles.tile([1, T], F32, tag="srow")
            lnm = singles.tile([1, T], F32, tag="lnm")
            cos_s = singles.tile([128, T], BF16, tag="cos_s")
            sin_s = singles.tile([128, T], BF16, tag="sin_s")
            sk_t = singles.tile([128, TT], F32, tag="sk")
            skx = singles.tile([128, TT], F32, tag="skx")
            q_sb = singles.tile([128, H_LOC, T], BF16, tag="q_sb")
            k_sb = singles.tile([128, H_LOC, T], BF16, tag="k_sb")
            v_sb = singles.tile([128, TT, NL], BF16, tag="v_sb")
            if FP8_ATT:
                v_dr = singles.tile([128, TT // 2, 2, NL], FP8, tag="v_dr")
            outT = singles.tile([128, H_LOC, T], BF16, tag="outT")

            # ---------------- per-strip RMSNorm sums + s pipeline ---------------
            def emit_ssq_s_strip(j):
                mark(f"ssq_s{j}")
                js = slice(j * 512, (j + 1) * 512)
                ssq = psu.tile([1, 512], F32, tag="su", name=f"ssq{j}_{rep}")
                # squares striped across ACT/DVE/Pool so no engine serializes
                sq_rot = [1, 2, 1, 2, 1, 1, 2, 1, 1, 2, 1, 1, 2, 1, 2, 1]

                def emit_square(dst, kd):
                    eng = sq_rot[kd]
                    if eng == 0:
                        nc.scalar.activation(dst, xt[:, kd, js], AF.Square)
                    else:
                        (None, nc.vector, nc.gpsimd)[eng].tensor_mul(
                            dst, xt[:, kd, js], xt[:, kd, js]
                        )

                if FP8_SSQ:
                    for p_ in range(KD // 2):
                        xsq = sq.tile([128, 2, 512], FP8, tag="xsq")
                        for m in range(2):
                            emit_square(xsq[:, m, :], 2 * p_ + m)
                        nc.tensor.matmul(
                            ssq, lhsT=onedr, rhs=xsq,
                            start=(p_ == 0), stop=(p_ == KD // 2 - 1),
                            perf_mode=mybir.MatmulPerfMode.DoubleRow,
                        )
                else:
                    for kd in range(KD):
                        xsq = sq.tile([128, 512], BF16, tag="xsq")
                        emit_square(xsq, kd)
                        nc.tensor.matmul(
                            ssq, lhsT=on128, rhs=xsq,
                            start=(kd == 0), stop=(kd == KD - 1),
                        )
                # lnm = ln(mean + eps); s = exp(-0.5 lnm)
                nc.scalar.activation(lnm[:, js], ssq, AF.Ln, bias=epsb, scale=1.0 / D)
                nc.scalar.activation(s_row[:, js], lnm[:, js], AF.Exp, scale=-0.5)
                # round-trip for the [128, 4] t-tile layout slice (v scaling +
                # k-side s folded into the exp scale)
                rt_out = nc.sync.dma_start(
                    out=s_scr_ap[4 * j : 4 * (j + 1), :].rearrange("i p -> () (i p)"),
                    in_=s_row[:, js],
                )
                rt_in = nc.sync.dma_start(
                    out=sk_t[:, 4 * j : 4 * (j + 1)],
                    in_=s_scr_ap.rearrange("i p -> p i")[:, 4 * j : 4 * (j + 1)],
                )
                # DRAM deps are invisible to Tile: force read-after-write
                d1 = InstructionNameOrderedSet(); d1.add(rt_out.ins.name)
                rt_in.ins.add_sync_dependencies_from(d1)
                nc.vector.tensor_scalar_mul(
                    skx[:, 4 * j : 4 * (j + 1)], sk_t[:, 4 * j : 4 * (j + 1)],
                    INV_SQRT_DH,
                )

            def emit_cos_fold(j):
                mark(f"cosf{j}")
                js = slice(j * 512, (j + 1) * 512)
                sb = bcast.tile([128, 512], F32, tag="sb")
                nc.gpsimd.partition_broadcast(sb, s_row[:, js])
                nc.vector.tensor_mul(cos_s[:, js], cosr[:, js], sb)
                nc.vector.tensor_mul(sin_s[:, js], sinr[:, js], sb)

            # ---------------- projections --------------------------------------
            def emit_qk_strip(h, j, dst, w, ctab, stab):
                # q uses the s-folded tables; k uses raw tables (its s is
                # folded into the exp scale instead, so k never waits on s).
                mark(f"{'q' if dst is q_sb else 'k'}{j}h{h}")
                hs = slice(h * 128, (h + 1) * 128)
                js = slice(j * 512, (j + 1) * 512)
                ps = pmm.tile([128, 512], F32, tag="mm")
                for kd in range(KD):
                    nc.tensor.matmul(
                        ps, lhsT=w[:, kd, hs], rhs=xt[:, kd, js],
                        start=(kd == 0), stop=(kd == KD - 1),
                    )
                # m2's half-swap must read PSUM (cross-partition SBUF reads
                # are illegal); the aligned m1 path goes through an ACT copy so
                # the DVE muls get 2x mode and the psum frees quickly.
                qc = qtmp.tile([128, 512], BF16, tag="qc")
                nc.scalar.copy(qc, ps)
                m2 = rope.tile([128, 512], BF16, tag="m2")
                nc.vector.tensor_mul(m2[0:64, :], ps[64:128, :], stab[0:64, js])
                nc.vector.tensor_mul(m2[64:128, :], ps[0:64, :], stab[64:128, js])
                m1 = rope.tile([128, 512], BF16, tag="m1")
                nc.vector.tensor_mul(m1, qc, ctab[:, js])
                nc.gpsimd.tensor_add(dst[:, h, js], m1, m2)

            def emit_v_tile(tt):
                # v[t, dh] directly: lhsT = x^T tile, rhs = wv[d, nl]
                mark(f"v{tt}")
                ts = slice(tt * 128, (tt + 1) * 128)
                if tt % 2 == 0:
                    ps = pvm.tile([128, NL], F32, tag="vmm")
                else:
                    ps = pmm.tile([128, NL], F32, tag="mm", name="vps")
                for kd in range(KD):
                    nc.tensor.matmul(
                        ps, lhsT=xt[:, kd, ts], rhs=wv[:, kd, :],
                        start=(kd == 0), stop=(kd == KD - 1),
                    )
                nc.scalar.mul(v_sb[:, tt, :], ps, sk_t[:, tt : tt + 1])
                if FP8_ATT:
                    nc.scalar.mul(
                        v_dr[:, tt // 2, tt % 2, :], ps, sk_t[:, tt : tt + 1]
                    )

            # ---------------- attention ----------------------------------------
            def emit_attention_bf16(h, Q0, W, filler):
                mark(f"att{h}_q{Q0}")
                hs = slice(h * 128, (h + 1) * 128)
                ntk = (Q0 + W) // 128
                po = ppv.tile([128, 512], F32, tag="pv", name="po")[:, :W]
                su = psu.tile([1, 512], F32, tag="su", name="su")[:, :W]
                for i in range(ntk):
                    cb = 128 * i - Q0
                    c0 = max(cb, 0)
                    cs = slice(c0, W)
                    qs = slice(Q0 + c0, Q0 + W)
                    st = psc.tile([128, 512], F32, tag="sc")
                    nc.tensor.matmul(
                        st[:, cs], lhsT=k_sb[:, h, i * 128 : (i + 1) * 128],
                        rhs=q_sb[:, h, qs], start=True, stop=True,
                    )
                    e = ebf.tile([128, 512], BF16, tag="e")
                    nc.scalar.activation(e[:, cs], st[:, cs], AF.Exp,
                                         scale=skx[:, i : i + 1])
                    if cb >= 0:
                        nc.gpsimd.tensor_mul(
                            e[:, cb : cb + 128], e[:, cb : cb + 128], msk
                        )
                    if filler:
                        filler.pop(0)()
                    nc.tensor.matmul(
                        po[:, cs], lhsT=v_sb[:, i, hs], rhs=e[:, cs],
                        start=(i == 0), stop=(i == ntk - 1),
                    )
                    nc.tensor.matmul(
                        su[:, cs], lhsT=on128, rhs=e[:, cs],
                        start=(i == 0), stop=(i == ntk - 1),
                    )
                emit_epilogue(h, Q0, W, po, su)

            def emit_attention_fp8(h, Q0, W, filler, tail_hook=None):
                mark(f"att{h}_q{Q0}f8")
                hs = slice(h * 128, (h + 1) * 128)
                npair = (Q0 + W) // 256
                po = ppv.tile([128, 512], F32, tag="pv", name="po")[:, :W]
                su = psu.tile([1, 512], F32, tag="su", name="su")[:, :W]
                for p_ in range(npair):
                    i0 = 2 * p_
                    c0 = max(128 * i0 - Q0, 0)       # pair-wide col start
                    cs = slice(c0, W)
                    e = epool.tile([128, 2, 512], FP8, tag="edr")
                    for m in range(2):
                        i = i0 + m
                        cb = 128 * i - Q0
                        cm = max(cb, 0)              # member col start
                        st = psc.tile([128, 512], F32, tag="sc")
                        nc.tensor.matmul(
                            st[:, cm:W],
                            lhsT=k_sb[:, h, i * 128 : (i + 1) * 128],
                            rhs=q_sb[:, h, Q0 + cm : Q0 + W],
                            start=True, stop=True,
                        )
                        nc.scalar.activation(
                            e[:, m, cm:W], st[:, cm:W], AF.Exp,
                            bias=f8bias[:, 0:1], scale=skx[:, i : i + 1],
                        )
                        if cm > c0:
                            nc.gpsimd.memset(e[:, m, c0:cm], 0)
                        if cb >= 0 and cb < W:
                            nc.gpsimd.tensor_mul(
                                e[:, m, cb : cb + 128], e[:, m, cb : cb + 128], msk
                            )
                    if filler:
                        filler.pop(0)()
                    nc.tensor.matmul(
                        po[:, cs], lhsT=v_dr[:, p_, :, hs], rhs=e[:, :, cs],
                        start=(p_ == 0), stop=(p_ == npair - 1),
                        perf_mode=mybir.MatmulPerfMode.DoubleRow,
                    )
                    nc.tensor.matmul(
                        su[:, cs], lhsT=onedr, rhs=e[:, :, cs],
                        start=(p_ == 0), stop=(p_ == npair - 1),
                        perf_mode=mybir.MatmulPerfMode.DoubleRow,
                    )
                    if filler:
                        filler.pop(0)()
                    if tail_hook is not None and p_ == npair - 2:
                        emit_epilogue_piece(h, Q0, po, su, 0, W - 256)
                        tail_hook()
                if tail_hook is not None:
                    emit_epilogue_piece(h, Q0, po, su, W - 256, W)
                else:
                    emit_epilogue(h, Q0, W, po, su)

            def emit_epilogue_piece(h, Q0, po, su, c0, c1):
                mark(f"epp{h}_q{Q0}_{c0}")
                rec = small.tile([1, 512], F32, tag="rec", name="rec")[:, c0:c1]
                nc.vector.reciprocal_approx_fast(rec, su[:, c0:c1])
                rb = bcast.tile([128, 512], F32, tag="rb", name="rb")[:, c0:c1]
                nc.gpsimd.partition_broadcast(rb, rec)
                nc.vector.tensor_mul(outT[:, h, Q0 + c0 : Q0 + c1], po[:, c0:c1], rb)

            def emit_epilogue(h, Q0, W, po, su):
                mark(f"epi{h}_q{Q0}")
                rec = small.tile([1, 512], F32, tag="rec", name="rec")[:, :W]
                nc.vector.reciprocal_approx_fast(rec, su)
                rb = bcast.tile([128, 512], F32, tag="rb", name="rb")[:, :W]
                nc.gpsimd.partition_broadcast(rb, rec)
                nc.vector.tensor_mul(outT[:, h, Q0 : Q0 + W], po, rb)
                if DBG:
                    sud = small.tile([1, 512], F32, tag="sud", name="sud")[:, :W]
                    nc.vector.tensor_copy(sud, su)
                    nc.sync.dma_start(out=su_dbg.ap()[h : h + 1, Q0 : Q0 + W], in_=sud)
                    nc.sync.dma_start(out=rec_dbg.ap()[h : h + 1, Q0 : Q0 + W], in_=rec)

            def emit_attention(h, Q0, W, filler, tail_hook=None):
                if FP8_ATT and Q0 >= 512:
                    emit_attention_fp8(h, Q0, W, filler, tail_hook)
                else:
                    emit_attention_bf16(h, Q0, W, filler)

            # ---------------- output projection --------------------------------
            def make_wo_chunk(tt, n, stg, pool, tag, evac):
                ts = slice(tt * 128, (tt + 1) * 128)
                ns = slice(n * 512, (n + 1) * 512)

                def emit():
                    mark(f"wo_t{tt}n{n}")
                    ps = pool.tile([128, 512], F32, tag=tag)
                    for h in range(H_LOC):
                        nc.tensor.matmul(
                            ps, lhsT=outT[:, h, ts], rhs=wo[:, h, ns],
                            start=(h == 0), stop=(h == H_LOC - 1),
                        )
                    if evac is nc.scalar:
                        nc.scalar.copy(stg[:, ns], ps)
                    else:
                        evac.tensor_copy(stg[:, ns], ps)
                    if tt >= TT - 4:
                        if n % 2 == 1:
                            hs_ = slice((n - 1) * 512, (n + 1) * 512)
                            nc.sync.dma_start(out=out_ap[ts, hs_], in_=stg[:, hs_])
                    elif n == NS - 1:
                        nc.sync.dma_start(out=out_ap[ts, :], in_=stg)

                return emit

            def wo_chunks_range(tt0, tt1, rotate=False, evacs=None):
                out = []
                rot = [(pmm, "mm"), (ppv, "pv"), (psc, "sc")] if rotate else [(pmm, "mm")]
                evacs = evacs or [nc.vector, nc.scalar]
                k = 0
                for tt in range(tt0, tt1):
                    stg = stage.tile([128, T], BF16, tag="stg", name=f"stg{tt}_{rep}")
                    for n in range(NS):
                        pool, tag = rot[k % len(rot)]
                        out.append(make_wo_chunk(tt, n, stg, pool, tag,
                                                 evacs[k % len(evacs)]))
                        k += 1
                return out

            # ---------------- schedule -----------------------------------------
            # Per strip: attention j immediately after strip-j projections;
            # strip j+1's ssq/k/q/v work follows (matching x DMA arrival).
            # ACT order stays exps(j) before squares(j+1).
            def emit_kqv_slot(jn):
                for h in range(H_LOC):
                    emit_qk_strip(h, jn, k_sb, wk, cosr, sinr)
                emit_cos_fold(jn)
                for h in range(H_LOC):
                    emit_qk_strip(h, jn, q_sb, wq, cos_s, sin_s)
                for tt in range(4 * jn, 4 * (jn + 1)):
                    emit_v_tile(tt)

            emit_ssq_s_strip(0)
            load_part_b()
            emit_kqv_slot(0)
            for j in range(NS - 1):
                ev = [nc.vector] if j >= 2 else [nc.vector, nc.scalar]
                filler = wo_chunks_range(4 * (j - 1), 4 * j, evacs=ev) if j >= 1 else []
                half = len(filler) // 2
                fa, fb = filler[:half], filler[half:]
                emit_attention(0, 512 * j, 512, fa)
                emit_attention(1, 512 * j, 512, fb)
                for f in fa + fb:
                    f()
                if j == 0:
                    emit_ssq_s_strip(1)
                emit_kqv_slot(j + 1)
                if j + 2 < NS:
                    emit_ssq_s_strip(j + 2)
            filler = wo_chunks_range(8, 12, evacs=[nc.vector])
            fa, fb = filler[:6], filler[6:]
            emit_attention(0, 1536, 512, fa)

            TAIL_HOOK = os.environ.get("TAIL_HOOK", "1") == "1"

            def tail_hook():
                for f in wo_chunks_range(12, 14, rotate=True):
                    f()

            emit_attention(1, 1536, 512, fb,
                           tail_hook=tail_hook if TAIL_HOOK else None)
            for f in fa + fb:
                f()
            for f in wo_chunks_range(14, 16 if TAIL_HOOK else 12, rotate=True):
                f()
            if not TAIL_HOOK:
                for f in wo_chunks_range(12, 16, rotate=True):
                    f()

        for _rep in range(repeats):
            emit_body(_rep)

    # Force Exp and Ln onto the single combined table set so the table-load
    # pass emits one ACT_TABLE_LOAD for the whole kernel.
    from concourse.hw_specs import get_activation_tables
    tabs = get_activation_tables(nc.m.arch)
    for nm_, fs_ in tabs.items():
        if nm_ != "natural_log_exp_and_others":
            fs_.discard(AF.Exp)
            fs_.discard(AF.Ln)
    nc.compile()
    _CACHED[repeats] = nc
    return nc


def _host_prep(x, w_ln, wq, wk, wv, wo, cos, sin):
    bf = ml_dtypes.bfloat16
    f8 = mybir.dt.np(FP8)
    x = np.asarray(x, np.float32)
    w_ln = np.asarray(w_ln, np.float32)
    cosT = np.ascontiguousarray(np.asarray(cos, np.float32).T).astype(bf)
    sinTf = np.ascontiguousarray(np.asarray(sin, np.float32).T)
    sinTf[0:64] *= -1.0          # rotate_half sign folded into the table
    sinT = sinTf.astype(bf)
    xT = np.ascontiguousarray(x.T).astype(bf)

    # causal boundary mask for diagonal tiles: mask[p, f] = 1 if f >= p
    f = np.arange(128)[None, :]
    p = np.arange(128)[:, None]
    masks = (f >= p).astype(bf)

    ones128 = np.ones((128, 1), bf)
    ones_dr = np.ones((128, 2, 16), f8)

    wq_s = (np.asarray(wq, np.float32) * w_ln[None, :])
    wk_s = (np.asarray(wk, np.float32) * w_ln[None, :])
    wv_s = (np.asarray(wv, np.float32) * w_ln[None, :])
    wo32 = np.asarray(wo, np.float32)

    in_maps = []
    for c in range(N_CORES):
        sl = slice(c * NL, (c + 1) * NL)
        in_maps.append({
            "xT": xT,
            "wqT": np.ascontiguousarray(wq_s[sl].T).astype(bf),
            "wkT": np.ascontiguousarray(wk_s[sl].T).astype(bf),
            "wvT": np.ascontiguousarray(wv_s[sl].T).astype(bf),
            "woT": np.ascontiguousarray(wo32[:, sl].T).astype(bf),
            "cosT": cosT,
            "sinT": sinT,
            "masks": masks,
            "ones128": ones128,
            "ones_dr": ones_dr,
        })
    return in_maps


def kernel(x, w_ln, wq, wk, wv, wo, cos, sin):
    nc = _build_program()
    in_maps = _host_prep(x, w_ln, wq, wk, wv, wo, cos, sin)
    t0 = time.time()
    res = run_bass_kernel_spmd(nc, in_maps, core_ids=list(range(N_CORES)))
    t1 = time.time()
    print(f"run_bass_kernel_spmd wall: {(t1 - t0) * 1e3:.1f} ms", file=sys.stderr)
    acc = np.zeros((T, D), np.float32)
    for r in res.results:
        acc += np.asarray(r["out"], np.float32)
    return np.asarray(x, np.float32) + acc


# revision 5
# speedup vs baseline: 1.0155x; 1.0006x over previous
"""Trainium2 Bass kernel for nn_Attention (T=2048, D=2048, H=16, Dh=128).

Tensor-parallel over heads, 2 heads per core on 8 cores. v2 schedule:
  - DMA issue order = need order: x strip 0 (quartered) -> wq -> wk ->
    cos/sin strip 0 -> wv -> x s1 -> ... -> wo -> x s3; input x streamed
    strip-major so the RMSNorm scale s[j] unblocks per strip.
  - RMSNorm: per-strip squares (ACT/DVE) + ones-matmul; s = exp(-0.5
    ln(mean+eps)); broadcast via Pool partition_broadcast (no PE);
    per-strip DRAM round trip for the [128, TT] t-tile layout (v scaling).
  - q/k^T projections from resident x^T; RoPE on DVE with s-folded tables.
  - v projected directly in [t, dh] layout (lhsT = x^T tile), evacuated
    via ACT copy with per-partition scale = s (no DMA transpose).
  - causal attention in S^T[tk,tq] layout, per-diagonal-tile trimming;
    strip 0 in bf16; strips 1-3 run PV + softmax-sum matmuls in fp8e4
    DoubleRow (two key tiles per matmul) — exp emitted straight to packed
    fp8 pairs; scores stay bf16 everywhere.
  - softmax normalization deferred: rec = exp(-ln(sum)), Pool broadcast,
    DVE multiply into outT; per-head output projection accumulated in
    PSUM; partial outputs written bf16 (summed f32 on host with residual).
"""

import math
import os
import sys
import time

for _p in ("/opt/trn_rl_repo", "/root/.axon_site/_ro/trn_rl_repo"):
    if os.path.isdir(_p) and _p not in sys.path:
        sys.path.insert(0, _p)

import numpy as np
import ml_dtypes

import concourse.bass as bass
import concourse.tile as tile
from concourse.bass import InstructionNameOrderedSet
from concourse import bacc, mybir
from concourse.bass_utils import run_bass_kernel_spmd

BF16 = mybir.dt.bfloat16
F32 = mybir.dt.float32
FP8 = mybir.dt.float8e4
AF = mybir.ActivationFunctionType

T = 2048
D = 2048
N_H = 16
D_H = 128
N_CORES = 8
H_LOC = N_H // N_CORES          # heads per core = 2
NL = H_LOC * D_H                # local head width = 256
KD = D // 128                   # contraction tiles = 16
TT = T // 128                   # t tiles = 16
NS = T // 512                   # 512-wide strips = 4
EPS = 1e-5
INV_SQRT_DH = 1.0 / math.sqrt(D_H)
FP8_EXP_BIAS = -4.0             # keeps exp() under fp8e4m3 max (saw 8.6 sigma); cancels in norm

FP8_ATT = os.environ.get('FP8_ATT', '1') == '1'                  # fp8 DoubleRow PV+sum for strips >= 1
FP8_SSQ = os.environ.get('FP8_SSQ', '1') == '1'                  # fp8 DoubleRow for sum(x^2)

_CACHED = {}
PHASES = []  # (label, first_instruction_id) — emission-order markers for sim analysis


def _build_program(repeats=1):
    if repeats in _CACHED:
        return _CACHED[repeats]

    nc = bacc.Bacc("TRN2", target_bir_lowering=False, debug=False, num_devices=N_CORES)

    xT_d = nc.dram_tensor("xT", [D, T], BF16, kind="ExternalInput")
    wq_d = nc.dram_tensor("wqT", [D, NL], BF16, kind="ExternalInput")
    wk_d = nc.dram_tensor("wkT", [D, NL], BF16, kind="ExternalInput")
    wv_d = nc.dram_tensor("wvT", [D, NL], BF16, kind="ExternalInput")
    wo_d = nc.dram_tensor("woT", [NL, T], BF16, kind="ExternalInput")
    cos_d = nc.dram_tensor("cosT", [D_H, T], BF16, kind="ExternalInput")
    sin_d = nc.dram_tensor("sinT", [D_H, T], BF16, kind="ExternalInput")
    msk_d = nc.dram_tensor("masks", [128, 128], BF16, kind="ExternalInput")
    on128_d = nc.dram_tensor("ones128", [128, 1], BF16, kind="ExternalInput")
    onedr_d = nc.dram_tensor("ones_dr", [128, 2, 16], FP8, kind="ExternalInput")
    out_d = nc.dram_tensor("out", [T, D], BF16, kind="ExternalOutput")
    DBG = os.environ.get("DBG_OUTT", "0") == "1"
    if DBG:
        outT_dbg = nc.dram_tensor("outT_dbg", [128, H_LOC, T], BF16, kind="ExternalOutput")
        su_dbg = nc.dram_tensor("su_dbg", [H_LOC, T], F32, kind="ExternalOutput")
        sk_dbg = nc.dram_tensor("sk_dbg", [128, TT], F32, kind="ExternalOutput")
        v_dbg = nc.dram_tensor("v_dbg", [128, TT, NL], BF16, kind="ExternalOutput")
        rec_dbg = nc.dram_tensor("rec_dbg", [H_LOC, T], F32, kind="ExternalOutput")
    # DRAM scratch for the s row->tile-layout round trip
    s_scr = nc.dram_tensor("s_scr", [TT, 128], F32, kind="Internal")

    ap = lambda h: h.ap()
    xT, out_ap, s_scr_ap = ap(xT_d), ap(out_d), ap(s_scr)

    from contextlib import ExitStack

    with tile.TileContext(nc) as tc, ExitStack() as ctx:
        P = ctx.enter_context  # noqa

        singles = P(tc.tile_pool(name="singles", bufs=1))
        sq = P(tc.tile_pool(name="sq", bufs=2))            # square scratch
        rope = P(tc.tile_pool(name="rope", bufs=4))        # [128,512] bf16
        qtmp = P(tc.tile_pool(name="qtmp", bufs=2 if os.environ.get("DBG_OUTT","0")=="0" else 1))        # raw qk evac copies
        epool = P(tc.tile_pool(name="epool", bufs=4 if os.environ.get("DBG_OUTT","0")=="0" else 3))      # fp8 exp pairs
        ebf = P(tc.tile_pool(name="ebf", bufs=2 if os.environ.get("DBG_OUTT","0")=="0" else 1))          # bf16 exp tiles (strip 0)
        small = P(tc.tile_pool(name="small", bufs=2))      # [1,512] f32
        bcast = P(tc.tile_pool(name="bcast", bufs=2))      # [128,512] bcast rows
        stage = P(tc.tile_pool(name="stage", bufs=3))      # [128,T] out staging
        pmm = P(tc.tile_pool(name="pmm", bufs=2, space="PSUM"))
        pvm = P(tc.tile_pool(name="pvm", bufs=1, space="PSUM"))
        psc = P(tc.tile_pool(name="psc", bufs=2, space="PSUM"))
        ppv = P(tc.tile_pool(name="ppv", bufs=2, space="PSUM"))
        psu = P(tc.tile_pool(name="psu", bufs=1, space="PSUM"))

        def mark(label):
            PHASES.append((label, nc.next_id()))



        def emit_body(rep):
            # ---------------- DMA issue (need-ordered) -------------------------
            mark("dma_issue")
            xt = singles.tile([128, KD, T], BF16, tag="xt")
            xTv = xT.rearrange("(n p) t -> p n t", p=128)

            def load_x_chunk(j, k0, k1):
                js = slice(j * 512, (j + 1) * 512)
                nc.sync.dma_start(out=xt[:, k0:k1, js], in_=xTv[:, k0:k1, js])

            def load_w(dram, tag):
                t_ = singles.tile([128, KD, NL], BF16, tag=tag)
                nc.sync.dma_start(out=t_, in_=ap(dram).rearrange("(a p) m -> p a m", p=128))
                return t_

            cosr = singles.tile([128, T], BF16, tag="cosr")
            sinr = singles.tile([128, T], BF16, tag="sinr")

            def load_cs_strip(j):
                js = slice(j * 512, (j + 1) * 512)
                nc.sync.dma_start(out=cosr[:, js], in_=ap(cos_d)[:, js])
                nc.sync.dma_start(out=sinr[:, js], in_=ap(sin_d)[:, js])

            # part A: everything needed before/while s0 resolves.  Later loads
            # are issued after ssq0's round-trip DMAs so the round trip does
            # not queue behind them on the serialized DMA engines.
            load_x_chunk(0, 0, 2)
            on128 = singles.tile([128, 1], BF16, tag="on128")
            nc.sync.dma_start(out=on128, in_=ap(on128_d))
            onedr_f = singles.tile([128, 2, 16], FP8, tag="onedr")
            nc.sync.dma_start(out=onedr_f, in_=ap(onedr_d))
            # dual-fp8 ldweights needs the pair-dim step 16B-aligned
            onedr = onedr_f[:, :, 0:1]
            wk = load_w(wk_d, "wk")
            load_x_chunk(0, 2, 9)
            load_x_chunk(0, 9, 16)
            load_cs_strip(0)
            wq = load_w(wq_d, "wq")
            wv = load_w(wv_d, "wv")
            msk = singles.tile([128, 128], BF16, tag="msk")
            nc.sync.dma_start(out=msk, in_=ap(msk_d))
            load_cs_strip(1)
            load_x_chunk(1, 0, 4)
            load_x_chunk(1, 4, 8)
            load_x_chunk(1, 8, 12)
            load_x_chunk(1, 12, 16)
            wo = singles.tile([128, H_LOC, T], BF16, tag="wo")

            def load_part_b():
                load_x_chunk(2, 0, 8)
                load_x_chunk(2, 8, 16)
                load_cs_strip(2)
                nc.sync.dma_start(
                    out=wo, in_=ap(wo_d).rearrange("(h p) t -> p h t", p=128))
                load_x_chunk(3, 0, 8)
                load_x_chunk(3, 8, 16)
                load_cs_strip(3)

            # ---------------- persistent SBUF state ----------------------------
            epsb = singles.tile([1, 1], F32, tag="epsb")
            nc.vector.memset(epsb, EPS)
            f8bias = singles.tile([128, 1], F32, tag="f8bias")
            nc.vector.memset(f8bias, FP8_EXP_BIAS)
            s_row = singles.tile([1, T], F32, tag="srow")
            lnm = singles.tile([1, T], F32, tag="lnm")
            cos_s = singles.tile([128, T], BF16, tag="cos_s")
            sin_s = singles.tile([128, T], BF16, tag="sin_s")
            sk_t = singles.tile([128, TT], F32, tag="sk")
            skx = singles.tile([128, TT], F32, tag="skx")
            q_sb = singles.tile([128, H_LOC, T], BF16, tag="q_sb")
            k_sb = singles.tile([128, H_LOC, T], BF16, tag="k_sb")
            v_sb = singles.tile([128, TT, NL], BF16, tag="v_sb")
            if FP8_ATT:
                v_dr = singles.tile([128, TT // 2, 2, NL], FP8, tag="v_dr")
            outT = singles.tile([128, H_LOC, T], BF16, tag="outT")

            # ---------------- per-strip RMSNorm sums + s pipeline ---------------
            def emit_ssq_s_strip(j):
                mark(f"ssq_s{j}")
                js = slice(j * 512, (j + 1) * 512)
                ssq = psu.tile([1, 512], F32, tag="su", name=f"ssq{j}_{rep}")
                # squares striped across ACT/DVE/Pool so no engine serializes
                sq_rot = [1, 2, 1, 2, 1, 1, 2, 1, 1, 2, 1, 1, 2, 1, 2, 1]

                def emit_square(dst, kd):
                    eng = sq_rot[kd]
                    if eng == 0:
                        nc.scalar.activation(dst, xt[:, kd, js], AF.Square)
                    else:
                        (None, nc.vector, nc.gpsimd)[eng].tensor_mul(
                            dst, xt[:, kd, js], xt[:, kd, js]
                        )

                if FP8_SSQ:
                    for p_ in range(KD // 2):
                        xsq = sq.tile([128, 2, 512], FP8, tag="xsq")
                        for m in range(2):
                            emit_square(xsq[:, m, :], 2 * p_ + m)
                        nc.tensor.matmul(
                            ssq, lhsT=onedr, rhs=xsq,
                            start=(p_ == 0), stop=(p_ == KD // 2 - 1),
                            perf_mode=mybir.MatmulPerfMode.DoubleRow,
                        )
                else:
                    for kd in range(KD):
                        xsq = sq.tile([128, 512], BF16, tag="xsq")
                        emit_square(xsq, kd)
                        nc.tensor.matmul(
                            ssq, lhsT=on128, rhs=xsq,
                            start=(kd == 0), stop=(kd == KD - 1),
                        )
                # lnm = ln(mean + eps); s = exp(-0.5 lnm)
                nc.scalar.activation(lnm[:, js], ssq, AF.Ln, bias=epsb, scale=1.0 / D)
                nc.scalar.activation(s_row[:, js], lnm[:, js], AF.Exp, scale=-0.5)
                # round-trip for the [128, 4] t-tile layout slice (v scaling +
                # k-side s folded into the exp scale)
                rt_out = nc.sync.dma_start(
                    out=s_scr_ap[4 * j : 4 * (j + 1), :].rearrange("i p -> () (i p)"),
                    in_=s_row[:, js],
                )
                rt_in = nc.sync.dma_start(
                    out=sk_t[:, 4 * j : 4 * (j + 1)],
                    in_=s_scr_ap.rearrange("i p -> p i")[:, 4 * j : 4 * (j + 1)],
                )
                # DRAM deps are invisible to Tile: force read-after-write
                d1 = InstructionNameOrderedSet(); d1.add(rt_out.ins.name)
                rt_in.ins.add_sync_dependencies_from(d1)
                nc.vector.tensor_scalar_mul(
                    skx[:, 4 * j : 4 * (j + 1)], sk_t[:, 4 * j : 4 * (j + 1)],
                    INV_SQRT_DH,
                )

            def emit_cos_fold(j):
                mark(f"cosf{j}")
                js = slice(j * 512, (j + 1) * 512)
                sb = bcast.tile([128, 512], F32, tag="sb")
                nc.gpsimd.partition_broadcast(sb, s_row[:, js])
                nc.vector.tensor_mul(cos_s[:, js], cosr[:, js], sb)
                nc.vector.tensor_mul(sin_s[:, js], sinr[:, js], sb)

            # ---------------- projections --------------------------------------
            def emit_qk_strip(h, j, dst, w, ctab, stab):
                # q uses the s-folded tables; k uses raw tables (its s is
                # folded into the exp scale instead, so k never waits on s).
                mark(f"{'q' if dst is q_sb else 'k'}{j}h{h}")
                hs = slice(h * 128, (h + 1) * 128)
                js = slice(j * 512, (j + 1) * 512)
                ps = pmm.tile([128, 512], F32, tag="mm")
                for kd in range(KD):
                    nc.tensor.matmul(
                        ps, lhsT=w[:, kd, hs], rhs=xt[:, kd, js],
                        start=(kd == 0), stop=(kd == KD - 1),
                    )
                # m2's half-swap must read PSUM (cross-partition SBUF reads
                # are illegal); the aligned m1 path goes through an ACT copy so
                # the DVE muls get 2x mode and the psum frees quickly.
                qc = qtmp.tile([128, 512], BF16, tag="qc")
                nc.scalar.copy(qc, ps)
                m2 = rope.tile([128, 512], BF16, tag="m2")
                nc.vector.tensor_mul(m2[0:64, :], ps[64:128, :], stab[0:64, js])
                nc.vector.tensor_mul(m2[64:128, :], ps[0:64, :], stab[64:128, js])
                m1 = rope.tile([128, 512], BF16, tag="m1")
                nc.vector.tensor_mul(m1, qc, ctab[:, js])
                nc.gpsimd.tensor_add(dst[:, h, js], m1, m2)

            def emit_v_tile(tt):
                # v[t, dh] directly: lhsT = x^T tile, rhs = wv[d, nl]
                mark(f"v{tt}")
                ts = slice(tt * 128, (tt + 1) * 128)
                if tt % 2 == 0:
                    ps = pvm.tile([128, NL], F32, tag="vmm")
                else:
                    ps = pmm.tile([128, NL], F32, tag="mm", name="vps")
                for kd in range(KD):
                    nc.tensor.matmul(
                        ps, lhsT=xt[:, kd, ts], rhs=wv[:, kd, :],
                        start=(kd == 0), stop=(kd == KD - 1),
                    )
                nc.scalar.mul(v_sb[:, tt, :], ps, sk_t[:, tt : tt + 1])
                if FP8_ATT:
                    nc.scalar.mul(
                        v_dr[:, tt // 2, tt % 2, :], ps, sk_t[:, tt : tt + 1]
                    )

            # ---------------- attention ----------------------------------------
            def emit_attention_bf16(h, Q0, W, filler):
                mark(f"att{h}_q{Q0}")
                hs = slice(h * 128, (h + 1) * 128)
                ntk = (Q0 + W) // 128
                po = ppv.tile([128, 512], F32, tag="pv", name="po")[:, :W]
                su = psu.tile([1, 512], F32, tag="su", name="su")[:, :W]
                for i in range(ntk):
                    cb = 128 * i - Q0
                    c0 = max(cb, 0)
                    cs = slice(c0, W)
                    qs = slice(Q0 + c0, Q0 + W)
                    st = psc.tile([128, 512], F32, tag="sc")
                    nc.tensor.matmul(
                        st[:, cs], lhsT=k_sb[:, h, i * 128 : (i + 1) * 128],
                        rhs=q_sb[:, h, qs], start=True, stop=True,
                    )
                    e = ebf.tile([128, 512], BF16, tag="e")
                    nc.scalar.activation(e[:, cs], st[:, cs], AF.Exp,
                                         scale=skx[:, i : i + 1])
                    if cb >= 0:
                        nc.gpsimd.tensor_mul(
                            e[:, cb : cb + 128], e[:, cb : cb + 128], msk
                        )
                    if filler:
                        filler.pop(0)()
                    nc.tensor.matmul(
                        po[:, cs], lhsT=v_sb[:, i, hs], rhs=e[:, cs],
                        start=(i == 0), stop=(i == ntk - 1),
                    )
                    nc.tensor.matmul(
                        su[:, cs], lhsT=on128, rhs=e[:, cs],
                        start=(i == 0), stop=(i == ntk - 1),
                    )
                emit_epilogue(h, Q0, W, po, su)

            def emit_attention_fp8(h, Q0, W, filler, tail_hook=None):
                mark(f"att{h}_q{Q0}f8")
                hs = slice(h * 128, (h + 1) * 128)
                npair = (Q0 + W) // 256
                po = ppv.tile([128, 512], F32, tag="pv", name="po")[:, :W]
                su = psu.tile([1, 512], F32, tag="su", name="su")[:, :W]
                for p_ in range(npair):
                    i0 = 2 * p_
                    c0 = max(128 * i0 - Q0, 0)       # pair-wide col start
                    cs = slice(c0, W)
                    e = epool.tile([128, 2, 512], FP8, tag="edr")
                    for m in range(2):
                        i = i0 + m
                        cb = 128 * i - Q0
                        cm = max(cb, 0)              # member col start
                        st = psc.tile([128, 512], F32, tag="sc")
                        nc.tensor.matmul(
                            st[:, cm:W],
                            lhsT=k_sb[:, h, i * 128 : (i + 1) * 128],
                            rhs=q_sb[:, h, Q0 + cm : Q0 + W],
                            start=True, stop=True,
                        )
                        nc.scalar.activation(
                            e[:, m, cm:W], st[:, cm:W], AF.Exp,
                            bias=f8bias[:, 0:1], scale=skx[:, i : i + 1],
                        )
                        if cm > c0:
                            nc.gpsimd.memset(e[:, m, c0:cm], 0)
                        if cb >= 0 and cb < W:
                            nc.gpsimd.tensor_mul(
                                e[:, m, cb : cb + 128], e[:, m, cb : cb + 128], msk
                            )
                    if filler:
                        filler.pop(0)()
                    nc.tensor.matmul(
                        po[:, cs], lhsT=v_dr[:, p_, :, hs], rhs=e[:, :, cs],
                        start=(p_ == 0), stop=(p_ == npair - 1),
                        perf_mode=mybir.MatmulPerfMode.DoubleRow,
                    )
                    nc.tensor.matmul(
                        su[:, cs], lhsT=onedr, rhs=e[:, :, cs],
                        start=(p_ == 0), stop=(p_ == npair - 1),
                        perf_mode=mybir.MatmulPerfMode.DoubleRow,
                    )
                    if filler:
                        filler.pop(0)()
                    if tail_hook is not None and p_ == npair - 2:
                        emit_epilogue_piece(h, Q0, po, su, 0, W - 256)
                        tail_hook()
                if tail_hook is not None:
                    emit_epilogue_piece(h, Q0, po, su, W - 256, W)
                else:
                    emit_epilogue(h, Q0, W, po, su)

            def emit_epilogue_piece(h, Q0, po, su, c0, c1):
                mark(f"epp{h}_q{Q0}_{c0}")
                rec = small.tile([1, 512], F32, tag="rec", name="rec")[:, c0:c1]
                nc.vector.reciprocal_approx_fast(rec, su[:, c0:c1])
                rb = bcast.tile([128, 512], F32, tag="rb", name="rb")[:, c0:c1]
                nc.gpsimd.partition_broadcast(rb, rec)
                nc.vector.tensor_mul(outT[:, h, Q0 + c0 : Q0 + c1], po[:, c0:c1], rb)

            def emit_epilogue(h, Q0, W, po, su):
                mark(f"epi{h}_q{Q0}")
                rec = small.tile([1, 512], F32, tag="rec", name="rec")[:, :W]
                nc.vector.reciprocal_approx_fast(rec, su)
                rb = bcast.tile([128, 512], F32, tag="rb", name="rb")[:, :W]
                nc.gpsimd.partition_broadcast(rb, rec)
                nc.vector.tensor_mul(outT[:, h, Q0 : Q0 + W], po, rb)
                if DBG:
                    sud = small.tile([1, 512], F32, tag="sud", name="sud")[:, :W]
                    nc.vector.tensor_copy(sud, su)
                    nc.sync.dma_start(out=su_dbg.ap()[h : h + 1, Q0 : Q0 + W], in_=sud)
                    nc.sync.dma_start(out=rec_dbg.ap()[h : h + 1, Q0 : Q0 + W], in_=rec)

            def emit_attention(h, Q0, W, filler, tail_hook=None):
                if FP8_ATT and Q0 >= 512:
                    emit_attention_fp8(h, Q0, W, filler, tail_hook)
                else:
                    emit_attention_bf16(h, Q0, W, filler)

            # ---------------- output projection --------------------------------
            def make_wo_chunk(tt, n, stg, pool, tag, evac):
                ts = slice(tt * 128, (tt + 1) * 128)
                ns = slice(n * 512, (n + 1) * 512)

                def emit():
                    mark(f"wo_t{tt}n{n}")
                    ps = pool.tile([128, 512], F32, tag=tag)
                    for h in range(H_LOC):
                        nc.tensor.matmul(
                            ps, lhsT=outT[:, h, ts], rhs=wo[:, h, ns],
                            start=(h == 0), stop=(h == H_LOC - 1),
                        )
                    if evac is nc.scalar:
                        nc.scalar.copy(stg[:, ns], ps)
                    else:
                        evac.tensor_copy(stg[:, ns], ps)
                    if tt >= TT - 4:
                        if n % 2 == 1:
                            hs_ = slice((n - 1) * 512, (n + 1) * 512)
                            nc.sync.dma_start(out=out_ap[ts, hs_], in_=stg[:, hs_])
                    elif n == NS - 1:
                        nc.sync.dma_start(out=out_ap[ts, :], in_=stg)

                return emit

            def wo_chunks_range(tt0, tt1, rotate=False, evacs=None):
                out = []
                rot = [(pmm, "mm"), (ppv, "pv"), (psc, "sc")] if rotate else [(pmm, "mm")]
                evacs = evacs or [nc.vector, nc.scalar]
                k = 0
                for tt in range(tt0, tt1):
                    stg = stage.tile([128, T], BF16, tag="stg", name=f"stg{tt}_{rep}")
                    for n in range(NS):
                        pool, tag = rot[k % len(rot)]
                        out.append(make_wo_chunk(tt, n, stg, pool, tag,
                                                 evacs[k % len(evacs)]))
                        k += 1
                return out

            # ---------------- schedule -----------------------------------------
            # Per strip: attention j immediately after strip-j projections;
            # strip j+1's ssq/k/q/v work follows (matching x DMA arrival).
            # ACT order stays exps(j) before squares(j+1).
            def emit_kqv_slot(jn):
                for h in range(H_LOC):
                    emit_qk_strip(h, jn, k_sb, wk, cosr, sinr)
                emit_cos_fold(jn)
                for h in range(H_LOC):
                    emit_qk_strip(h, jn, q_sb, wq, cos_s, sin_s)
                for tt in range(4 * jn, 4 * (jn + 1)):
                    emit_v_tile(tt)

            emit_ssq_s_strip(0)
            load_part_b()
            emit_kqv_slot(0)
            for j in range(NS - 1):
                ev = [nc.vector] if j >= 2 else [nc.vector, nc.scalar]
                filler = wo_chunks_range(4 * (j - 1), 4 * j, evacs=ev) if j >= 1 else []
                half = len(filler) // 2
                fa, fb = filler[:half], filler[half:]
                emit_attention(0, 512 * j, 512, fa)
                emit_attention(1, 512 * j, 512, fb)
                for f in fa + fb:
                    f()
                if j == 0:
                    emit_ssq_s_strip(1)
                emit_kqv_slot(j + 1)
                if j + 2 < NS:
                    emit_ssq_s_strip(j + 2)
            filler = wo_chunks_range(8, 12, evacs=[nc.vector])
            fa, fb = filler[:6], filler[6:]
            emit_attention(0, 1536, 512, fa)

            TAIL_HOOK = os.environ.get("TAIL_HOOK", "1") == "1"

            def tail_hook():
                for f in wo_chunks_range(12, 14, rotate=True):
                    f()

            emit_attention(1, 1536, 512, fb,
                           tail_hook=tail_hook if TAIL_HOOK else None)
            for f in fa + fb:
                f()
            for f in wo_chunks_range(14, 16 if TAIL_HOOK else 12, rotate=True):
                f()
            if not TAIL_HOOK:
                for f in wo_chunks_range(12, 16, rotate=True):
                    f()

        for _rep in range(repeats):
            emit_body(_rep)

    # Force Exp and Ln onto the single combined table set so the table-load
    # pass emits one ACT_TABLE_LOAD for the whole kernel.
    from concourse.hw_specs import get_activation_tables
    tabs = get_activation_tables(nc.m.arch)
    for nm_, fs_ in tabs.items():
        if nm_ != "natural_log_exp_and_others":
            fs_.discard(AF.Exp)
            fs_.discard(AF.Ln)
    nc.compile()
    _CACHED[repeats] = nc
    return nc


def _host_prep(x, w_ln, wq, wk, wv, wo, cos, sin):
    bf = ml_dtypes.bfloat16
    f8 = mybir.dt.np(FP8)
    x = np.asarray(x, np.float32)
    w_ln = np.asarray(w_ln, np.float32)
    cosT = np.ascontiguousarray(np.asarray(cos, np.float32).T).astype(bf)
    sinTf = np.ascontiguousarray(np.asarray(sin, np.float32).T)
    sinTf[0:64] *= -1.0          # rotate_half sign folded into the table
    sinT = sinTf.astype(bf)
    xT = np.ascontiguousarray(x.T).astype(bf)

    # causal boundary mask for diagonal tiles: mask[p, f] = 1 if f >= p
    f = np.arange(128)[None, :]
    p = np.arange(128)[:, None]
    masks = (f >= p).astype(bf)

    ones128 = np.ones((128, 1), bf)
    ones_dr = np.ones((128, 2, 16), f8)

    wq_s = (np.asarray(wq, np.float32) * w_ln[None, :])
    wk_s = (np.asarray(wk, np.float32) * w_ln[None, :])
    wv_s = (np.asarray(wv, np.float32) * w_ln[None, :])
    wo32 = np.asarray(wo, np.float32)

    in_maps = []
    for c in range(N_CORES):
        sl = slice(c * NL, (c + 1) * NL)
        in_maps.append({
            "xT": xT,
            "wqT": np.ascontiguousarray(wq_s[sl].T).astype(bf),
            "wkT": np.ascontiguousarray(wk_s[sl].T).astype(bf),
            "wvT": np.ascontiguousarray(wv_s[sl].T).astype(bf),
            "woT": np.ascontiguousarray(wo32[:, sl].T).astype(bf),
            "cosT": cosT,
            "sinT": sinT,
            "masks": masks,
            "ones128": ones128,
            "ones_dr": ones_dr,
        })
    return in_maps


def kernel(x, w_ln, wq, wk, wv, wo, cos, sin):
    nc = _build_program()
    in_maps = _host_prep(x, w_ln, wq, wk, wv, wo, cos, sin)
    t0 = time.time()
    res = run_bass_kernel_spmd(nc, in_maps, core_ids=list(range(N_CORES)))
    t1 = time.time()
    print(f"run_bass_kernel_spmd wall: {(t1 - t0) * 1e3:.1f} ms", file=sys.stderr)
    acc = np.zeros((T, D), np.float32)
    for r in res.results:
        acc += np.asarray(r["out"], np.float32)
    return np.asarray(x, np.float32) + acc


# revision 6
# speedup vs baseline: 1.0348x; 1.0190x over previous
"""Trainium2 Bass kernel for nn_Attention (T=2048, D=2048, H=16, Dh=128).

Tensor-parallel over heads, 2 heads per core on 8 cores. v2 schedule:
  - DMA issue order = need order: x strip 0 (quartered) -> wq -> wk ->
    cos/sin strip 0 -> wv -> x s1 -> ... -> wo -> x s3; input x streamed
    strip-major so the RMSNorm scale s[j] unblocks per strip.
  - RMSNorm: per-strip squares (ACT/DVE) + ones-matmul; s = exp(-0.5
    ln(mean+eps)); broadcast via Pool partition_broadcast (no PE);
    per-strip DRAM round trip for the [128, TT] t-tile layout (v scaling).
  - q/k^T projections from resident x^T; RoPE on DVE with s-folded tables.
  - v projected directly in [t, dh] layout (lhsT = x^T tile), evacuated
    via ACT copy with per-partition scale = s (no DMA transpose).
  - causal attention in S^T[tk,tq] layout, per-diagonal-tile trimming;
    strip 0 in bf16; strips 1-3 run PV + softmax-sum matmuls in fp8e4
    DoubleRow (two key tiles per matmul) — exp emitted straight to packed
    fp8 pairs; scores stay bf16 everywhere.
  - softmax normalization deferred: rec = exp(-ln(sum)), Pool broadcast,
    DVE multiply into outT; per-head output projection accumulated in
    PSUM; partial outputs written bf16 (summed f32 on host with residual).
"""

import math
import os
import sys
import time

for _p in ("/opt/trn_rl_repo", "/root/.axon_site/_ro/trn_rl_repo"):
    if os.path.isdir(_p) and _p not in sys.path:
        sys.path.insert(0, _p)

import numpy as np
import ml_dtypes

import concourse.bass as bass
import concourse.tile as tile
from concourse.bass import InstructionNameOrderedSet
from concourse import bacc, mybir
from concourse.bass_utils import run_bass_kernel_spmd

BF16 = mybir.dt.bfloat16
F32 = mybir.dt.float32
FP8 = mybir.dt.float8e4
AF = mybir.ActivationFunctionType

T = 2048
D = 2048
N_H = 16
D_H = 128
N_CORES = 8
H_LOC = N_H // N_CORES          # heads per core = 2
NL = H_LOC * D_H                # local head width = 256
KD = D // 128                   # contraction tiles = 16
TT = T // 128                   # t tiles = 16
NS = T // 512                   # 512-wide strips = 4
EPS = 1e-5
INV_SQRT_DH = 1.0 / math.sqrt(D_H)
FP8_EXP_BIAS = -4.0             # keeps exp() under fp8e4m3 max (saw 8.6 sigma); cancels in norm

FP8_ATT = os.environ.get('FP8_ATT', '1') == '1'                  # fp8 DoubleRow PV+sum for strips >= 1
FP8_SSQ = os.environ.get('FP8_SSQ', '1') == '1'                  # fp8 DoubleRow for sum(x^2)

_CACHED = {}
PHASES = []  # (label, first_instruction_id) — emission-order markers for sim analysis


def _build_program(repeats=1):
    if repeats in _CACHED:
        return _CACHED[repeats]

    nc = bacc.Bacc("TRN2", target_bir_lowering=False, debug=False, num_devices=N_CORES)

    xT_d = nc.dram_tensor("xT", [D, T], BF16, kind="ExternalInput")
    wq_d = nc.dram_tensor("wqT", [D, NL], BF16, kind="ExternalInput")
    wk_d = nc.dram_tensor("wkT", [D, NL], BF16, kind="ExternalInput")
    wv_d = nc.dram_tensor("wvT", [D, NL], BF16, kind="ExternalInput")
    wo_d = nc.dram_tensor("woT", [NL, T], BF16, kind="ExternalInput")
    cos_d = nc.dram_tensor("cosT", [D_H, T], BF16, kind="ExternalInput")
    sin_d = nc.dram_tensor("sinT", [D_H, T], BF16, kind="ExternalInput")
    msk_d = nc.dram_tensor("masks", [128, 128], BF16, kind="ExternalInput")
    on128_d = nc.dram_tensor("ones128", [128, 1], BF16, kind="ExternalInput")
    onedr_d = nc.dram_tensor("ones_dr", [128, 2, 16], FP8, kind="ExternalInput")
    out_d = nc.dram_tensor("out", [T, D], BF16, kind="ExternalOutput")
    DBG = os.environ.get("DBG_OUTT", "0") == "1"
    if DBG:
        outT_dbg = nc.dram_tensor("outT_dbg", [128, H_LOC, T], BF16, kind="ExternalOutput")
        su_dbg = nc.dram_tensor("su_dbg", [H_LOC, T], F32, kind="ExternalOutput")
        sk_dbg = nc.dram_tensor("sk_dbg", [128, TT], F32, kind="ExternalOutput")
        v_dbg = nc.dram_tensor("v_dbg", [128, TT, NL], BF16, kind="ExternalOutput")
        rec_dbg = nc.dram_tensor("rec_dbg", [H_LOC, T], F32, kind="ExternalOutput")
    # DRAM scratch for the s row->tile-layout round trip
    s_scr = nc.dram_tensor("s_scr", [TT, 128], F32, kind="Internal")

    ap = lambda h: h.ap()
    xT, out_ap, s_scr_ap = ap(xT_d), ap(out_d), ap(s_scr)

    from contextlib import ExitStack

    with tile.TileContext(nc) as tc, ExitStack() as ctx:
        P = ctx.enter_context  # noqa

        singles = P(tc.tile_pool(name="singles", bufs=1))
        sq = P(tc.tile_pool(name="sq", bufs=2))            # square scratch
        rope = P(tc.tile_pool(name="rope", bufs=4))        # [128,512] bf16
        qtmp = P(tc.tile_pool(name="qtmp", bufs=2 if os.environ.get("DBG_OUTT","0")=="0" else 1))        # raw qk evac copies
        epool = P(tc.tile_pool(name="epool", bufs=4 if os.environ.get("DBG_OUTT","0")=="0" else 3))      # fp8 exp pairs
        ebf = P(tc.tile_pool(name="ebf", bufs=2 if os.environ.get("DBG_OUTT","0")=="0" else 1))          # bf16 exp tiles (strip 0)
        small = P(tc.tile_pool(name="small", bufs=2))      # [1,512] f32
        bcast = P(tc.tile_pool(name="bcast", bufs=2))      # [128,512] bcast rows
        stage = P(tc.tile_pool(name="stage", bufs=3))      # [128,T] out staging
        pmm = P(tc.tile_pool(name="pmm", bufs=2, space="PSUM"))
        pvm = P(tc.tile_pool(name="pvm", bufs=1, space="PSUM"))
        psc = P(tc.tile_pool(name="psc", bufs=2, space="PSUM"))
        ppv = P(tc.tile_pool(name="ppv", bufs=2, space="PSUM"))
        psu = P(tc.tile_pool(name="psu", bufs=1, space="PSUM"))

        def mark(label):
            PHASES.append((label, nc.next_id()))



        def emit_body(rep):
            # ---------------- DMA issue (need-ordered) -------------------------
            mark("dma_issue")
            xt = singles.tile([128, KD, T], BF16, tag="xt")
            xTv = xT.rearrange("(n p) t -> p n t", p=128)

            def load_x_chunk(j, k0, k1):
                js = slice(j * 512, (j + 1) * 512)
                nc.sync.dma_start(out=xt[:, k0:k1, js], in_=xTv[:, k0:k1, js])

            def load_w(dram, tag):
                t_ = singles.tile([128, KD, NL], BF16, tag=tag)
                nc.sync.dma_start(out=t_, in_=ap(dram).rearrange("(a p) m -> p a m", p=128))
                return t_

            cosr = singles.tile([128, T], BF16, tag="cosr")
            sinr = singles.tile([128, T], BF16, tag="sinr")

            def load_cs_strip(j):
                js = slice(j * 512, (j + 1) * 512)
                nc.sync.dma_start(out=cosr[:, js], in_=ap(cos_d)[:, js])
                nc.sync.dma_start(out=sinr[:, js], in_=ap(sin_d)[:, js])

            # part A: everything needed before/while s0 resolves.  Later loads
            # are issued after ssq0's round-trip DMAs so the round trip does
            # not queue behind them on the serialized DMA engines.
            load_x_chunk(0, 0, 2)
            on128 = singles.tile([128, 1], BF16, tag="on128")
            nc.sync.dma_start(out=on128, in_=ap(on128_d))
            onedr_f = singles.tile([128, 2, 16], FP8, tag="onedr")
            nc.sync.dma_start(out=onedr_f, in_=ap(onedr_d))
            # dual-fp8 ldweights needs the pair-dim step 16B-aligned
            onedr = onedr_f[:, :, 0:1]
            wk = load_w(wk_d, "wk")
            load_x_chunk(0, 2, 9)
            load_x_chunk(0, 9, 16)
            load_cs_strip(0)
            wq = load_w(wq_d, "wq")
            wv = load_w(wv_d, "wv")
            msk = singles.tile([128, 128], BF16, tag="msk")
            nc.sync.dma_start(out=msk, in_=ap(msk_d))
            load_cs_strip(1)
            load_x_chunk(1, 0, 4)
            load_x_chunk(1, 4, 8)
            load_x_chunk(1, 8, 12)
            load_x_chunk(1, 12, 16)
            wo = singles.tile([128, H_LOC, T], BF16, tag="wo")

            def load_part_b():
                load_x_chunk(2, 0, 8)
                load_x_chunk(2, 8, 16)
                load_cs_strip(2)
                nc.sync.dma_start(
                    out=wo, in_=ap(wo_d).rearrange("(h p) t -> p h t", p=128))
                load_x_chunk(3, 0, 8)
                load_x_chunk(3, 8, 16)
                load_cs_strip(3)

            # ---------------- persistent SBUF state ----------------------------
            epsb = singles.tile([1, 1], F32, tag="epsb")
            nc.vector.memset(epsb, EPS)
            f8bias = singles.tile([128, 1], F32, tag="f8bias")
            nc.vector.memset(f8bias, FP8_EXP_BIAS)
            s_row = singles.tile([1, T], F32, tag="srow")
            lnm = singles.tile([1, T], F32, tag="lnm")
            cos_s = singles.tile([128, T], BF16, tag="cos_s")
            sin_s = singles.tile([128, T], BF16, tag="sin_s")
            sk_t = singles.tile([128, TT], F32, tag="sk")
            skx = singles.tile([128, TT], F32, tag="skx")
            q_sb = singles.tile([128, H_LOC, T], BF16, tag="q_sb")
            k_sb = singles.tile([128, H_LOC, T], BF16, tag="k_sb")
            v_sb = singles.tile([128, TT, NL], BF16, tag="v_sb")
            if FP8_ATT:
                v_dr = singles.tile([128, TT // 2, 2, NL], FP8, tag="v_dr")
            outT = singles.tile([128, H_LOC, T], BF16, tag="outT")

            # ---------------- per-strip RMSNorm sums + s pipeline ---------------
            def emit_ssq_s_strip(j):
                mark(f"ssq_s{j}")
                js = slice(j * 512, (j + 1) * 512)
                ssq = psu.tile([1, 512], F32, tag="su", name=f"ssq{j}_{rep}")
                # squares striped across ACT/DVE/Pool so no engine serializes
                sq_rot = [1, 2, 1, 2, 1, 1, 2, 1, 1, 2, 1, 1, 2, 1, 2, 1]

                def emit_square(dst, kd):
                    eng = sq_rot[kd]
                    if eng == 0:
                        nc.scalar.activation(dst, xt[:, kd, js], AF.Square)
                    else:
                        (None, nc.vector, nc.gpsimd)[eng].tensor_mul(
                            dst, xt[:, kd, js], xt[:, kd, js]
                        )

                if FP8_SSQ:
                    for p_ in range(KD // 2):
                        xsq = sq.tile([128, 2, 512], FP8, tag="xsq")
                        for m in range(2):
                            emit_square(xsq[:, m, :], 2 * p_ + m)
                        nc.tensor.matmul(
                            ssq, lhsT=onedr, rhs=xsq,
                            start=(p_ == 0), stop=(p_ == KD // 2 - 1),
                            perf_mode=mybir.MatmulPerfMode.DoubleRow,
                        )
                else:
                    for kd in range(KD):
                        xsq = sq.tile([128, 512], BF16, tag="xsq")
                        emit_square(xsq, kd)
                        nc.tensor.matmul(
                            ssq, lhsT=on128, rhs=xsq,
                            start=(kd == 0), stop=(kd == KD - 1),
                        )
                # lnm = ln(mean + eps); s = exp(-0.5 lnm)
                nc.scalar.activation(lnm[:, js], ssq, AF.Ln, bias=epsb, scale=1.0 / D)
                nc.scalar.activation(s_row[:, js], lnm[:, js], AF.Exp, scale=-0.5)
                # round-trip for the [128, 4] t-tile layout slice (v scaling +
                # k-side s folded into the exp scale)
                rt_out = nc.sync.dma_start(
                    out=s_scr_ap[4 * j : 4 * (j + 1), :].rearrange("i p -> () (i p)"),
                    in_=s_row[:, js],
                )
                rt_in = nc.sync.dma_start(
                    out=sk_t[:, 4 * j : 4 * (j + 1)],
                    in_=s_scr_ap.rearrange("i p -> p i")[:, 4 * j : 4 * (j + 1)],
                )
                # DRAM deps are invisible to Tile: force read-after-write
                d1 = InstructionNameOrderedSet(); d1.add(rt_out.ins.name)
                rt_in.ins.add_sync_dependencies_from(d1)
                nc.vector.tensor_scalar_mul(
                    skx[:, 4 * j : 4 * (j + 1)], sk_t[:, 4 * j : 4 * (j + 1)],
                    INV_SQRT_DH,
                )

            def emit_cos_fold(j):
                mark(f"cosf{j}")
                js = slice(j * 512, (j + 1) * 512)
                sb = bcast.tile([128, 512], F32, tag="sb")
                nc.gpsimd.partition_broadcast(sb, s_row[:, js])
                nc.vector.tensor_mul(cos_s[:, js], cosr[:, js], sb)
                nc.vector.tensor_mul(sin_s[:, js], sinr[:, js], sb)

            # ---------------- projections --------------------------------------
            def emit_qk_strip(h, j, dst, w, ctab, stab):
                # q uses the s-folded tables; k uses raw tables (its s is
                # folded into the exp scale instead, so k never waits on s).
                mark(f"{'q' if dst is q_sb else 'k'}{j}h{h}")
                hs = slice(h * 128, (h + 1) * 128)
                js = slice(j * 512, (j + 1) * 512)
                ps = pmm.tile([128, 512], F32, tag="mm")
                for kd in range(KD):
                    nc.tensor.matmul(
                        ps, lhsT=w[:, kd, hs], rhs=xt[:, kd, js],
                        start=(kd == 0), stop=(kd == KD - 1),
                    )
                # m2's half-swap must read PSUM (cross-partition SBUF reads
                # are illegal); the aligned m1 path goes through an ACT copy so
                # the DVE muls get 2x mode and the psum frees quickly.
                qc = qtmp.tile([128, 512], BF16, tag="qc")
                nc.scalar.copy(qc, ps)
                m2 = rope.tile([128, 512], BF16, tag="m2")
                nc.vector.tensor_mul(m2[0:64, :], ps[64:128, :], stab[0:64, js])
                nc.vector.tensor_mul(m2[64:128, :], ps[0:64, :], stab[64:128, js])
                m1 = rope.tile([128, 512], BF16, tag="m1")
                nc.vector.tensor_mul(m1, qc, ctab[:, js])
                nc.gpsimd.tensor_add(dst[:, h, js], m1, m2)

            def emit_v_tile(tt):
                # v[t, dh] directly: lhsT = x^T tile, rhs = wv[d, nl]
                mark(f"v{tt}")
                ts = slice(tt * 128, (tt + 1) * 128)
                if tt % 2 == 0:
                    ps = pvm.tile([128, NL], F32, tag="vmm")
                else:
                    ps = pmm.tile([128, NL], F32, tag="mm", name="vps")
                for kd in range(KD):
                    nc.tensor.matmul(
                        ps, lhsT=xt[:, kd, ts], rhs=wv[:, kd, :],
                        start=(kd == 0), stop=(kd == KD - 1),
                    )
                nc.scalar.mul(v_sb[:, tt, :], ps, sk_t[:, tt : tt + 1])
                if FP8_ATT:
                    nc.scalar.mul(
                        v_dr[:, tt // 2, tt % 2, :], ps, sk_t[:, tt : tt + 1]
                    )

            # ---------------- attention ----------------------------------------
            def emit_attention_bf16(h, Q0, W, filler):
                mark(f"att{h}_q{Q0}")
                hs = slice(h * 128, (h + 1) * 128)
                ntk = (Q0 + W) // 128
                po = ppv.tile([128, 512], F32, tag="pv", name="po")[:, :W]
                su = psu.tile([1, 512], F32, tag="su", name="su")[:, :W]
                for i in range(ntk):
                    cb = 128 * i - Q0
                    c0 = max(cb, 0)
                    cs = slice(c0, W)
                    qs = slice(Q0 + c0, Q0 + W)
                    st = psc.tile([128, 512], F32, tag="sc")
                    nc.tensor.matmul(
                        st[:, cs], lhsT=k_sb[:, h, i * 128 : (i + 1) * 128],
                        rhs=q_sb[:, h, qs], start=True, stop=True,
                    )
                    e = ebf.tile([128, 512], BF16, tag="e")
                    nc.scalar.activation(e[:, cs], st[:, cs], AF.Exp,
                                         scale=skx[:, i : i + 1])
                    if cb >= 0:
                        nc.vector.tensor_mul(
                            e[:, cb : cb + 128], e[:, cb : cb + 128], msk
                        )
                    if filler:
                        filler.pop(0)()
                    nc.tensor.matmul(
                        po[:, cs], lhsT=v_sb[:, i, hs], rhs=e[:, cs],
                        start=(i == 0), stop=(i == ntk - 1),
                    )
                    nc.tensor.matmul(
                        su[:, cs], lhsT=on128, rhs=e[:, cs],
                        start=(i == 0), stop=(i == ntk - 1),
                    )
                emit_epilogue(h, Q0, W, po, su)

            def emit_attention_fp8(h, Q0, W, filler, tail_hook=None):
                mark(f"att{h}_q{Q0}f8")
                hs = slice(h * 128, (h + 1) * 128)
                npair = (Q0 + W) // 256
                po = ppv.tile([128, 512], F32, tag="pv", name="po")[:, :W]
                su = psu.tile([1, 512], F32, tag="su", name="su")[:, :W]
                for p_ in range(npair):
                    i0 = 2 * p_
                    c0 = max(128 * i0 - Q0, 0)       # pair-wide col start
                    cs = slice(c0, W)
                    e = epool.tile([128, 2, 512], FP8, tag="edr")
                    for m in range(2):
                        i = i0 + m
                        cb = 128 * i - Q0
                        cm = max(cb, 0)              # member col start
                        st = psc.tile([128, 512], F32, tag="sc")
                        nc.tensor.matmul(
                            st[:, cm:W],
                            lhsT=k_sb[:, h, i * 128 : (i + 1) * 128],
                            rhs=q_sb[:, h, Q0 + cm : Q0 + W],
                            start=True, stop=True,
                        )
                        nc.scalar.activation(
                            e[:, m, cm:W], st[:, cm:W], AF.Exp,
                            bias=f8bias[:, 0:1], scale=skx[:, i : i + 1],
                        )
                        if cm > c0:
                            nc.gpsimd.memset(e[:, m, c0:cm], 0)
                        if cb >= 0 and cb < W:
                            nc.vector.tensor_mul(
                                e[:, m, cb : cb + 128], e[:, m, cb : cb + 128], msk
                            )
                    if filler:
                        filler.pop(0)()
                    nc.tensor.matmul(
                        po[:, cs], lhsT=v_dr[:, p_, :, hs], rhs=e[:, :, cs],
                        start=(p_ == 0), stop=(p_ == npair - 1),
                        perf_mode=mybir.MatmulPerfMode.DoubleRow,
                    )
                    nc.tensor.matmul(
                        su[:, cs], lhsT=onedr, rhs=e[:, :, cs],
                        start=(p_ == 0), stop=(p_ == npair - 1),
                        perf_mode=mybir.MatmulPerfMode.DoubleRow,
                    )
                    if filler:
                        filler.pop(0)()
                    if tail_hook is not None and p_ == npair - 2:
                        emit_epilogue_piece(h, Q0, po, su, 0, W - 384)
                        emit_epilogue_piece(h, Q0, po, su, W - 384, W - 256)
                        tail_hook()
                if tail_hook is not None:
                    emit_epilogue_piece(h, Q0, po, su, W - 256, W - 128)
                    emit_epilogue_piece(h, Q0, po, su, W - 128, W)
                else:
                    emit_epilogue(h, Q0, W, po, su)

            def emit_epilogue_piece(h, Q0, po, su, c0, c1):
                mark(f"epp{h}_q{Q0}_{c0}")
                rec = small.tile([1, 512], F32, tag="rec", name="rec")[:, c0:c1]
                nc.vector.reciprocal_approx_fast(rec, su[:, c0:c1])
                rb = bcast.tile([128, 512], F32, tag="rb", name="rb")[:, c0:c1]
                nc.gpsimd.partition_broadcast(rb, rec)
                nc.vector.tensor_mul(outT[:, h, Q0 + c0 : Q0 + c1], po[:, c0:c1], rb)

            def emit_epilogue(h, Q0, W, po, su):
                mark(f"epi{h}_q{Q0}")
                rec = small.tile([1, 512], F32, tag="rec", name="rec")[:, :W]
                nc.vector.reciprocal_approx_fast(rec, su)
                rb = bcast.tile([128, 512], F32, tag="rb", name="rb")[:, :W]
                nc.gpsimd.partition_broadcast(rb, rec)
                nc.vector.tensor_mul(outT[:, h, Q0 : Q0 + W], po, rb)
                if DBG:
                    sud = small.tile([1, 512], F32, tag="sud", name="sud")[:, :W]
                    nc.vector.tensor_copy(sud, su)
                    nc.sync.dma_start(out=su_dbg.ap()[h : h + 1, Q0 : Q0 + W], in_=sud)
                    nc.sync.dma_start(out=rec_dbg.ap()[h : h + 1, Q0 : Q0 + W], in_=rec)

            def emit_attention(h, Q0, W, filler, tail_hook=None):
                if FP8_ATT and Q0 >= 512:
                    emit_attention_fp8(h, Q0, W, filler, tail_hook)
                else:
                    emit_attention_bf16(h, Q0, W, filler)

            # ---------------- output projection --------------------------------
            def make_wo_chunk(tt, n, stg, pool, tag, evac):
                ts = slice(tt * 128, (tt + 1) * 128)
                ns = slice(n * 512, (n + 1) * 512)

                def emit():
                    mark(f"wo_t{tt}n{n}")
                    ps = pool.tile([128, 512], F32, tag=tag)
                    for h in range(H_LOC):
                        nc.tensor.matmul(
                            ps, lhsT=outT[:, h, ts], rhs=wo[:, h, ns],
                            start=(h == 0), stop=(h == H_LOC - 1),
                        )
                    if evac is nc.scalar:
                        nc.scalar.copy(stg[:, ns], ps)
                    else:
                        evac.tensor_copy(stg[:, ns], ps)
                    if tt >= TT - 4:
                        if n % 2 == 1:
                            hs_ = slice((n - 1) * 512, (n + 1) * 512)
                            nc.sync.dma_start(out=out_ap[ts, hs_], in_=stg[:, hs_])
                    elif n == NS - 1:
                        nc.sync.dma_start(out=out_ap[ts, :], in_=stg)

                return emit

            def wo_chunks_range(tt0, tt1, rotate=False, evacs=None):
                out = []
                rot = [(pmm, "mm"), (ppv, "pv"), (psc, "sc")] if rotate else [(pmm, "mm")]
                evacs = evacs or [nc.vector, nc.scalar]
                k = 0
                for tt in range(tt0, tt1):
                    stg = stage.tile([128, T], BF16, tag="stg", name=f"stg{tt}_{rep}")
                    for n in range(NS):
                        pool, tag = rot[k % len(rot)]
                        out.append(make_wo_chunk(tt, n, stg, pool, tag,
                                                 evacs[k % len(evacs)]))
                        k += 1
                return out

            # ---------------- schedule -----------------------------------------
            # Per strip: attention j immediately after strip-j projections;
            # strip j+1's ssq/k/q/v work follows (matching x DMA arrival).
            # ACT order stays exps(j) before squares(j+1).
            def emit_kqv_slot(jn):
                for h in range(H_LOC):
                    emit_qk_strip(h, jn, k_sb, wk, cosr, sinr)
                emit_cos_fold(jn)
                for h in range(H_LOC):
                    emit_qk_strip(h, jn, q_sb, wq, cos_s, sin_s)
                for tt in range(4 * jn, 4 * (jn + 1)):
                    emit_v_tile(tt)

            emit_ssq_s_strip(0)
            load_part_b()
            emit_kqv_slot(0)
            for j in range(NS - 1):
                ev = [nc.vector] if j >= 2 else [nc.vector, nc.scalar]
                filler = wo_chunks_range(4 * (j - 1), 4 * j, evacs=ev) if j >= 1 else []
                half = len(filler) // 2
                fa, fb = filler[:half], filler[half:]
                emit_attention(0, 512 * j, 512, fa)
                emit_attention(1, 512 * j, 512, fb)
                for f in fa + fb:
                    f()
                if j == 0:
                    emit_ssq_s_strip(1)
                emit_kqv_slot(j + 1)
                if j + 2 < NS:
                    emit_ssq_s_strip(j + 2)
            filler = wo_chunks_range(8, 12, evacs=[nc.vector])
            fa, fb = filler[:6], filler[6:]
            emit_attention(0, 1536, 512, fa)

            TAIL_HOOK = os.environ.get("TAIL_HOOK", "1") == "1"

            def tail_hook():
                for f in wo_chunks_range(12, 14, rotate=True):
                    f()

            emit_attention(1, 1536, 512, fb,
                           tail_hook=tail_hook if TAIL_HOOK else None)
            for f in fa + fb:
                f()
            for f in wo_chunks_range(14, 16 if TAIL_HOOK else 12, rotate=True):
                f()
            if not TAIL_HOOK:
                for f in wo_chunks_range(12, 16, rotate=True):
                    f()

        for _rep in range(repeats):
            emit_body(_rep)

    # Force Exp and Ln onto the single combined table set so the table-load
    # pass emits one ACT_TABLE_LOAD for the whole kernel.
    from concourse.hw_specs import get_activation_tables
    tabs = get_activation_tables(nc.m.arch)
    for nm_, fs_ in tabs.items():
        if nm_ != "natural_log_exp_and_others":
            fs_.discard(AF.Exp)
            fs_.discard(AF.Ln)
    nc.compile()
    _CACHED[repeats] = nc
    return nc


def _host_prep(x, w_ln, wq, wk, wv, wo, cos, sin):
    bf = ml_dtypes.bfloat16
    f8 = mybir.dt.np(FP8)
    x = np.asarray(x, np.float32)
    w_ln = np.asarray(w_ln, np.float32)
    cosT = np.ascontiguousarray(np.asarray(cos, np.float32).T).astype(bf)
    sinTf = np.ascontiguousarray(np.asarray(sin, np.float32).T)
    sinTf[0:64] *= -1.0          # rotate_half sign folded into the table
    sinT = sinTf.astype(bf)
    xT = np.ascontiguousarray(x.T).astype(bf)

    # causal boundary mask for diagonal tiles: mask[p, f] = 1 if f >= p
    f = np.arange(128)[None, :]
    p = np.arange(128)[:, None]
    masks = (f >= p).astype(bf)

    ones128 = np.ones((128, 1), bf)
    ones_dr = np.ones((128, 2, 16), f8)

    wq_s = (np.asarray(wq, np.float32) * w_ln[None, :])
    wk_s = (np.asarray(wk, np.float32) * w_ln[None, :])
    wv_s = (np.asarray(wv, np.float32) * w_ln[None, :])
    wo32 = np.asarray(wo, np.float32)

    in_maps = []
    for c in range(N_CORES):
        sl = slice(c * NL, (c + 1) * NL)
        in_maps.append({
            "xT": xT,
            "wqT": np.ascontiguousarray(wq_s[sl].T).astype(bf),
            "wkT": np.ascontiguousarray(wk_s[sl].T).astype(bf),
            "wvT": np.ascontiguousarray(wv_s[sl].T).astype(bf),
            "woT": np.ascontiguousarray(wo32[:, sl].T).astype(bf),
            "cosT": cosT,
            "sinT": sinT,
            "masks": masks,
            "ones128": ones128,
            "ones_dr": ones_dr,
        })
    return in_maps


def kernel(x, w_ln, wq, wk, wv, wo, cos, sin):
    nc = _build_program()
    in_maps = _host_prep(x, w_ln, wq, wk, wv, wo, cos, sin)
    t0 = time.time()
    res = run_bass_kernel_spmd(nc, in_maps, core_ids=list(range(N_CORES)))
    t1 = time.time()
    print(f"run_bass_kernel_spmd wall: {(t1 - t0) * 1e3:.1f} ms", file=sys.stderr)
    acc = np.zeros((T, D), np.float32)
    for r in res.results:
        acc += np.asarray(r["out"], np.float32)
    return np.asarray(x, np.float32) + acc


# revision 7
# speedup vs baseline: 1.0351x; 1.0003x over previous
"""Trainium2 Bass kernel for nn_Attention (T=2048, D=2048, H=16, Dh=128).

Tensor-parallel over heads, 2 heads per core on 8 cores. v2 schedule:
  - DMA issue order = need order: x strip 0 (quartered) -> wq -> wk ->
    cos/sin strip 0 -> wv -> x s1 -> ... -> wo -> x s3; input x streamed
    strip-major so the RMSNorm scale s[j] unblocks per strip.
  - RMSNorm: per-strip squares (ACT/DVE) + ones-matmul; s = exp(-0.5
    ln(mean+eps)); broadcast via Pool partition_broadcast (no PE);
    per-strip DRAM round trip for the [128, TT] t-tile layout (v scaling).
  - q/k^T projections from resident x^T; RoPE on DVE with s-folded tables.
  - v projected directly in [t, dh] layout (lhsT = x^T tile), evacuated
    via ACT copy with per-partition scale = s (no DMA transpose).
  - causal attention in S^T[tk,tq] layout, per-diagonal-tile trimming;
    strip 0 in bf16; strips 1-3 run PV + softmax-sum matmuls in fp8e4
    DoubleRow (two key tiles per matmul) — exp emitted straight to packed
    fp8 pairs; scores stay bf16 everywhere.
  - softmax normalization deferred: rec = exp(-ln(sum)), Pool broadcast,
    DVE multiply into outT; per-head output projection accumulated in
    PSUM; partial outputs written bf16 (summed f32 on host with residual).
"""

import math
import os
import sys
import time

for _p in ("/opt/trn_rl_repo", "/root/.axon_site/_ro/trn_rl_repo"):
    if os.path.isdir(_p) and _p not in sys.path:
        sys.path.insert(0, _p)

import numpy as np
import ml_dtypes

import concourse.bass as bass
import concourse.tile as tile
from concourse.bass import InstructionNameOrderedSet
from concourse import bacc, mybir
from concourse.bass_utils import run_bass_kernel_spmd

BF16 = mybir.dt.bfloat16
F32 = mybir.dt.float32
FP8 = mybir.dt.float8e4
AF = mybir.ActivationFunctionType

T = 2048
D = 2048
N_H = 16
D_H = 128
N_CORES = 8
H_LOC = N_H // N_CORES          # heads per core = 2
NL = H_LOC * D_H                # local head width = 256
KD = D // 128                   # contraction tiles = 16
TT = T // 128                   # t tiles = 16
NS = T // 512                   # 512-wide strips = 4
EPS = 1e-5
INV_SQRT_DH = 1.0 / math.sqrt(D_H)
FP8_EXP_BIAS = -4.0             # keeps exp() under fp8e4m3 max (saw 8.6 sigma); cancels in norm

FP8_ATT = os.environ.get('FP8_ATT', '1') == '1'                  # fp8 DoubleRow PV+sum for strips >= 1
FP8_SSQ = os.environ.get('FP8_SSQ', '1') == '1'                  # fp8 DoubleRow for sum(x^2)

_CACHED = {}
PHASES = []  # (label, first_instruction_id) — emission-order markers for sim analysis


def _build_program(repeats=1):
    if repeats in _CACHED:
        return _CACHED[repeats]

    nc = bacc.Bacc("TRN2", target_bir_lowering=False, debug=False, num_devices=N_CORES)

    xT_d = nc.dram_tensor("xT", [D, T], BF16, kind="ExternalInput")
    wq_d = nc.dram_tensor("wqT", [D, NL], BF16, kind="ExternalInput")
    wk_d = nc.dram_tensor("wkT", [D, NL], BF16, kind="ExternalInput")
    wv_d = nc.dram_tensor("wvT", [D, NL], BF16, kind="ExternalInput")
    wo_d = nc.dram_tensor("woT", [NL, T], BF16, kind="ExternalInput")
    cos_d = nc.dram_tensor("cosT", [D_H, T], BF16, kind="ExternalInput")
    sin_d = nc.dram_tensor("sinT", [D_H, T], BF16, kind="ExternalInput")
    msk_d = nc.dram_tensor("masks", [128, 128], BF16, kind="ExternalInput")
    on128_d = nc.dram_tensor("ones128", [128, 1], BF16, kind="ExternalInput")
    onedr_d = nc.dram_tensor("ones_dr", [128, 2, 16], FP8, kind="ExternalInput")
    out_d = nc.dram_tensor("out", [T, D], BF16, kind="ExternalOutput")
    DBG = os.environ.get("DBG_OUTT", "0") == "1"
    if DBG:
        outT_dbg = nc.dram_tensor("outT_dbg", [128, H_LOC, T], BF16, kind="ExternalOutput")
        su_dbg = nc.dram_tensor("su_dbg", [H_LOC, T], F32, kind="ExternalOutput")
        sk_dbg = nc.dram_tensor("sk_dbg", [128, TT], F32, kind="ExternalOutput")
        v_dbg = nc.dram_tensor("v_dbg", [128, TT, NL], BF16, kind="ExternalOutput")
        rec_dbg = nc.dram_tensor("rec_dbg", [H_LOC, T], F32, kind="ExternalOutput")
    # DRAM scratch for the s row->tile-layout round trip
    s_scr = nc.dram_tensor("s_scr", [TT, 128], F32, kind="Internal")

    ap = lambda h: h.ap()
    xT, out_ap, s_scr_ap = ap(xT_d), ap(out_d), ap(s_scr)

    from contextlib import ExitStack

    with tile.TileContext(nc) as tc, ExitStack() as ctx:
        P = ctx.enter_context  # noqa

        singles = P(tc.tile_pool(name="singles", bufs=1))
        sq = P(tc.tile_pool(name="sq", bufs=2))            # square scratch
        rope = P(tc.tile_pool(name="rope", bufs=4))        # [128,512] bf16
        qtmp = P(tc.tile_pool(name="qtmp", bufs=2 if os.environ.get("DBG_OUTT","0")=="0" else 1))        # raw qk evac copies
        epool = P(tc.tile_pool(name="epool", bufs=4 if os.environ.get("DBG_OUTT","0")=="0" else 3))      # fp8 exp pairs
        ebf = P(tc.tile_pool(name="ebf", bufs=2 if os.environ.get("DBG_OUTT","0")=="0" else 1))          # bf16 exp tiles (strip 0)
        small = P(tc.tile_pool(name="small", bufs=2))      # [1,512] f32
        bcast = P(tc.tile_pool(name="bcast", bufs=2))      # [128,512] bcast rows
        stage = P(tc.tile_pool(name="stage", bufs=3))      # [128,T] out staging
        pmm = P(tc.tile_pool(name="pmm", bufs=2, space="PSUM"))
        pvm = P(tc.tile_pool(name="pvm", bufs=1, space="PSUM"))
        psc = P(tc.tile_pool(name="psc", bufs=2, space="PSUM"))
        ppv = P(tc.tile_pool(name="ppv", bufs=2, space="PSUM"))
        psu = P(tc.tile_pool(name="psu", bufs=1, space="PSUM"))

        def mark(label):
            PHASES.append((label, nc.next_id()))



        def emit_body(rep):
            # ---------------- DMA issue (need-ordered) -------------------------
            mark("dma_issue")
            xt = singles.tile([128, KD, T], BF16, tag="xt")
            xTv = xT.rearrange("(n p) t -> p n t", p=128)

            def load_x_chunk(j, k0, k1):
                js = slice(j * 512, (j + 1) * 512)
                nc.sync.dma_start(out=xt[:, k0:k1, js], in_=xTv[:, k0:k1, js])

            def load_w(dram, tag, split=False):
                t_ = singles.tile([128, KD, NL], BF16, tag=tag)
                v = ap(dram).rearrange("(a p) m -> p a m", p=128)
                if split:
                    nc.sync.dma_start(out=t_[:, 0:8, :], in_=v[:, 0:8, :])
                    nc.sync.dma_start(out=t_[:, 8:16, :], in_=v[:, 8:16, :])
                else:
                    nc.sync.dma_start(out=t_, in_=v)
                return t_

            cosr = singles.tile([128, T], BF16, tag="cosr")
            sinr = singles.tile([128, T], BF16, tag="sinr")

            def load_cs_strip(j):
                js = slice(j * 512, (j + 1) * 512)
                nc.sync.dma_start(out=cosr[:, js], in_=ap(cos_d)[:, js])
                nc.sync.dma_start(out=sinr[:, js], in_=ap(sin_d)[:, js])

            # part A: everything needed before/while s0 resolves.  Later loads
            # are issued after ssq0's round-trip DMAs so the round trip does
            # not queue behind them on the serialized DMA engines.
            load_x_chunk(0, 0, 2)
            on128 = singles.tile([128, 1], BF16, tag="on128")
            nc.sync.dma_start(out=on128, in_=ap(on128_d))
            onedr_f = singles.tile([128, 2, 16], FP8, tag="onedr")
            nc.sync.dma_start(out=onedr_f, in_=ap(onedr_d))
            # dual-fp8 ldweights needs the pair-dim step 16B-aligned
            onedr = onedr_f[:, :, 0:1]
            wk = load_w(wk_d, "wk", split=True)
            load_x_chunk(0, 2, 9)
            load_x_chunk(0, 9, 16)
            load_cs_strip(0)
            wq = load_w(wq_d, "wq")
            wv = load_w(wv_d, "wv")
            msk = singles.tile([128, 128], BF16, tag="msk")
            nc.sync.dma_start(out=msk, in_=ap(msk_d))
            load_cs_strip(1)
            load_x_chunk(1, 0, 4)
            load_x_chunk(1, 4, 8)
            load_x_chunk(1, 8, 12)
            load_x_chunk(1, 12, 16)
            wo = singles.tile([128, H_LOC, T], BF16, tag="wo")

            def load_part_b():
                load_x_chunk(2, 0, 8)
                load_x_chunk(2, 8, 16)
                load_cs_strip(2)
                nc.sync.dma_start(
                    out=wo, in_=ap(wo_d).rearrange("(h p) t -> p h t", p=128))
                load_x_chunk(3, 0, 8)
                load_x_chunk(3, 8, 16)
                load_cs_strip(3)

            # ---------------- persistent SBUF state ----------------------------
            epsb = singles.tile([1, 1], F32, tag="epsb")
            nc.vector.memset(epsb, EPS)
            f8bias = singles.tile([128, 1], F32, tag="f8bias")
            nc.vector.memset(f8bias, FP8_EXP_BIAS)
            s_row = singles.tile([1, T], F32, tag="srow")
            lnm = singles.tile([1, T], F32, tag="lnm")
            cos_s = singles.tile([128, T], BF16, tag="cos_s")
            sin_s = singles.tile([128, T], BF16, tag="sin_s")
            sk_t = singles.tile([128, TT], F32, tag="sk")
            skx = singles.tile([128, TT], F32, tag="skx")
            q_sb = singles.tile([128, H_LOC, T], BF16, tag="q_sb")
            k_sb = singles.tile([128, H_LOC, T], BF16, tag="k_sb")
            v_sb = singles.tile([128, TT, NL], BF16, tag="v_sb")
            if FP8_ATT:
                v_dr = singles.tile([128, TT // 2, 2, NL], FP8, tag="v_dr")
            outT = singles.tile([128, H_LOC, T], BF16, tag="outT")

            # ---------------- per-strip RMSNorm sums + s pipeline ---------------
            def emit_ssq_s_strip(j):
                mark(f"ssq_s{j}")
                js = slice(j * 512, (j + 1) * 512)
                ssq = psu.tile([1, 512], F32, tag="su", name=f"ssq{j}_{rep}")
                # squares striped across ACT/DVE/Pool so no engine serializes
                sq_rot = [1, 2, 1, 2, 1, 1, 2, 1, 1, 2, 1, 1, 2, 1, 2, 1]

                def emit_square(dst, kd):
                    eng = sq_rot[kd]
                    if eng == 0:
                        nc.scalar.activation(dst, xt[:, kd, js], AF.Square)
                    else:
                        (None, nc.vector, nc.gpsimd)[eng].tensor_mul(
                            dst, xt[:, kd, js], xt[:, kd, js]
                        )

                if FP8_SSQ:
                    for p_ in range(KD // 2):
                        xsq = sq.tile([128, 2, 512], FP8, tag="xsq")
                        for m in range(2):
                            emit_square(xsq[:, m, :], 2 * p_ + m)
                        nc.tensor.matmul(
                            ssq, lhsT=onedr, rhs=xsq,
                            start=(p_ == 0), stop=(p_ == KD // 2 - 1),
                            perf_mode=mybir.MatmulPerfMode.DoubleRow,
                        )
                else:
                    for kd in range(KD):
                        xsq = sq.tile([128, 512], BF16, tag="xsq")
                        emit_square(xsq, kd)
                        nc.tensor.matmul(
                            ssq, lhsT=on128, rhs=xsq,
                            start=(kd == 0), stop=(kd == KD - 1),
                        )
                # lnm = ln(mean + eps); s = exp(-0.5 lnm)
                nc.scalar.activation(lnm[:, js], ssq, AF.Ln, bias=epsb, scale=1.0 / D)
                nc.scalar.activation(s_row[:, js], lnm[:, js], AF.Exp, scale=-0.5)
                # round-trip for the [128, 4] t-tile layout slice (v scaling +
                # k-side s folded into the exp scale)
                rt_out = nc.sync.dma_start(
                    out=s_scr_ap[4 * j : 4 * (j + 1), :].rearrange("i p -> () (i p)"),
                    in_=s_row[:, js],
                )
                rt_in = nc.sync.dma_start(
                    out=sk_t[:, 4 * j : 4 * (j + 1)],
                    in_=s_scr_ap.rearrange("i p -> p i")[:, 4 * j : 4 * (j + 1)],
                )
                # DRAM deps are invisible to Tile: force read-after-write
                d1 = InstructionNameOrderedSet(); d1.add(rt_out.ins.name)
                rt_in.ins.add_sync_dependencies_from(d1)
                nc.vector.tensor_scalar_mul(
                    skx[:, 4 * j : 4 * (j + 1)], sk_t[:, 4 * j : 4 * (j + 1)],
                    INV_SQRT_DH,
                )

            def emit_cos_fold(j):
                mark(f"cosf{j}")
                js = slice(j * 512, (j + 1) * 512)
                sb = bcast.tile([128, 512], F32, tag="sb")
                nc.gpsimd.partition_broadcast(sb, s_row[:, js])
                nc.vector.tensor_mul(cos_s[:, js], cosr[:, js], sb)
                nc.vector.tensor_mul(sin_s[:, js], sinr[:, js], sb)

            # ---------------- projections --------------------------------------
            def emit_qk_strip(h, j, dst, w, ctab, stab):
                # q uses the s-folded tables; k uses raw tables (its s is
                # folded into the exp scale instead, so k never waits on s).
                mark(f"{'q' if dst is q_sb else 'k'}{j}h{h}")
                hs = slice(h * 128, (h + 1) * 128)
                js = slice(j * 512, (j + 1) * 512)
                ps = pmm.tile([128, 512], F32, tag="mm")
                for kd in range(KD):
                    nc.tensor.matmul(
                        ps, lhsT=w[:, kd, hs], rhs=xt[:, kd, js],
                        start=(kd == 0), stop=(kd == KD - 1),
                    )
                # m2's half-swap must read PSUM (cross-partition SBUF reads
                # are illegal); the aligned m1 path goes through an ACT copy so
                # the DVE muls get 2x mode and the psum frees quickly.
                qc = qtmp.tile([128, 512], BF16, tag="qc")
                nc.scalar.copy(qc, ps)
                m2 = rope.tile([128, 512], BF16, tag="m2")
                nc.vector.tensor_mul(m2[0:64, :], ps[64:128, :], stab[0:64, js])
                nc.vector.tensor_mul(m2[64:128, :], ps[0:64, :], stab[64:128, js])
                m1 = rope.tile([128, 512], BF16, tag="m1")
                nc.vector.tensor_mul(m1, qc, ctab[:, js])
                nc.gpsimd.tensor_add(dst[:, h, js], m1, m2)

            def emit_v_tile(tt):
                # v[t, dh] directly: lhsT = x^T tile, rhs = wv[d, nl]
                mark(f"v{tt}")
                ts = slice(tt * 128, (tt + 1) * 128)
                if tt % 2 == 0:
                    ps = pvm.tile([128, NL], F32, tag="vmm")
                else:
                    ps = pmm.tile([128, NL], F32, tag="mm", name="vps")
                for kd in range(KD):
                    nc.tensor.matmul(
                        ps, lhsT=xt[:, kd, ts], rhs=wv[:, kd, :],
                        start=(kd == 0), stop=(kd == KD - 1),
                    )
                nc.scalar.mul(v_sb[:, tt, :], ps, sk_t[:, tt : tt + 1])
                if FP8_ATT:
                    nc.scalar.mul(
                        v_dr[:, tt // 2, tt % 2, :], ps, sk_t[:, tt : tt + 1]
                    )

            # ---------------- attention ----------------------------------------
            def emit_attention_bf16(h, Q0, W, filler):
                mark(f"att{h}_q{Q0}")
                hs = slice(h * 128, (h + 1) * 128)
                ntk = (Q0 + W) // 128
                po = ppv.tile([128, 512], F32, tag="pv", name="po")[:, :W]
                su = psu.tile([1, 512], F32, tag="su", name="su")[:, :W]
                for i in range(ntk):
                    cb = 128 * i - Q0
                    c0 = max(cb, 0)
                    cs = slice(c0, W)
                    qs = slice(Q0 + c0, Q0 + W)
                    st = psc.tile([128, 512], F32, tag="sc")
                    nc.tensor.matmul(
                        st[:, cs], lhsT=k_sb[:, h, i * 128 : (i + 1) * 128],
                        rhs=q_sb[:, h, qs], start=True, stop=True,
                    )
                    e = ebf.tile([128, 512], BF16, tag="e")
                    nc.scalar.activation(e[:, cs], st[:, cs], AF.Exp,
                                         scale=skx[:, i : i + 1])
                    if cb >= 0:
                        nc.vector.tensor_mul(
                            e[:, cb : cb + 128], e[:, cb : cb + 128], msk
                        )
                    if filler:
                        filler.pop(0)()
                    nc.tensor.matmul(
                        po[:, cs], lhsT=v_sb[:, i, hs], rhs=e[:, cs],
                        start=(i == 0), stop=(i == ntk - 1),
                    )
                    nc.tensor.matmul(
                        su[:, cs], lhsT=on128, rhs=e[:, cs],
                        start=(i == 0), stop=(i == ntk - 1),
                    )
                emit_epilogue(h, Q0, W, po, su)

            def emit_attention_fp8(h, Q0, W, filler, tail_hook=None):
                mark(f"att{h}_q{Q0}f8")
                hs = slice(h * 128, (h + 1) * 128)
                npair = (Q0 + W) // 256
                po = ppv.tile([128, 512], F32, tag="pv", name="po")[:, :W]
                su = psu.tile([1, 512], F32, tag="su", name="su")[:, :W]
                for p_ in range(npair):
                    i0 = 2 * p_
                    c0 = max(128 * i0 - Q0, 0)       # pair-wide col start
                    cs = slice(c0, W)
                    e = epool.tile([128, 2, 512], FP8, tag="edr")
                    for m in range(2):
                        i = i0 + m
                        cb = 128 * i - Q0
                        cm = max(cb, 0)              # member col start
                        st = psc.tile([128, 512], F32, tag="sc")
                        nc.tensor.matmul(
                            st[:, cm:W],
                            lhsT=k_sb[:, h, i * 128 : (i + 1) * 128],
                            rhs=q_sb[:, h, Q0 + cm : Q0 + W],
                            start=True, stop=True,
                        )
                        nc.scalar.activation(
                            e[:, m, cm:W], st[:, cm:W], AF.Exp,
                            bias=f8bias[:, 0:1], scale=skx[:, i : i + 1],
                        )
                        if cm > c0:
                            nc.gpsimd.memset(e[:, m, c0:cm], 0)
                        if cb >= 0 and cb < W:
                            nc.vector.tensor_mul(
                                e[:, m, cb : cb + 128], e[:, m, cb : cb + 128], msk
                            )
                    if filler:
                        filler.pop(0)()
                    nc.tensor.matmul(
                        po[:, cs], lhsT=v_dr[:, p_, :, hs], rhs=e[:, :, cs],
                        start=(p_ == 0), stop=(p_ == npair - 1),
                        perf_mode=mybir.MatmulPerfMode.DoubleRow,
                    )
                    nc.tensor.matmul(
                        su[:, cs], lhsT=onedr, rhs=e[:, :, cs],
                        start=(p_ == 0), stop=(p_ == npair - 1),
                        perf_mode=mybir.MatmulPerfMode.DoubleRow,
                    )
                    if filler:
                        filler.pop(0)()
                    if tail_hook is not None and p_ == npair - 2:
                        emit_epilogue_piece(h, Q0, po, su, 0, W - 384)
                        emit_epilogue_piece(h, Q0, po, su, W - 384, W - 256)
                        tail_hook()
                if tail_hook is not None:
                    emit_epilogue_piece(h, Q0, po, su, W - 256, W - 128)
                    emit_epilogue_piece(h, Q0, po, su, W - 128, W)
                else:
                    emit_epilogue(h, Q0, W, po, su)

            def emit_epilogue_piece(h, Q0, po, su, c0, c1):
                mark(f"epp{h}_q{Q0}_{c0}")
                rec = small.tile([1, 512], F32, tag="rec", name="rec")[:, c0:c1]
                nc.vector.reciprocal_approx_fast(rec, su[:, c0:c1])
                rb = bcast.tile([128, 512], F32, tag="rb", name="rb")[:, c0:c1]
                nc.gpsimd.partition_broadcast(rb, rec)
                nc.vector.tensor_mul(outT[:, h, Q0 + c0 : Q0 + c1], po[:, c0:c1], rb)

            def emit_epilogue(h, Q0, W, po, su):
                mark(f"epi{h}_q{Q0}")
                rec = small.tile([1, 512], F32, tag="rec", name="rec")[:, :W]
                nc.vector.reciprocal_approx_fast(rec, su)
                rb = bcast.tile([128, 512], F32, tag="rb", name="rb")[:, :W]
                nc.gpsimd.partition_broadcast(rb, rec)
                nc.vector.tensor_mul(outT[:, h, Q0 : Q0 + W], po, rb)
                if DBG:
                    sud = small.tile([1, 512], F32, tag="sud", name="sud")[:, :W]
                    nc.vector.tensor_copy(sud, su)
                    nc.sync.dma_start(out=su_dbg.ap()[h : h + 1, Q0 : Q0 + W], in_=sud)
                    nc.sync.dma_start(out=rec_dbg.ap()[h : h + 1, Q0 : Q0 + W], in_=rec)

            def emit_attention(h, Q0, W, filler, tail_hook=None):
                if FP8_ATT and Q0 >= 512:
                    emit_attention_fp8(h, Q0, W, filler, tail_hook)
                else:
                    emit_attention_bf16(h, Q0, W, filler)

            # ---------------- output projection --------------------------------
            def make_wo_chunk(tt, n, stg, pool, tag, evac):
                ts = slice(tt * 128, (tt + 1) * 128)
                ns = slice(n * 512, (n + 1) * 512)

                def emit():
                    mark(f"wo_t{tt}n{n}")
                    ps = pool.tile([128, 512], F32, tag=tag)
                    for h in range(H_LOC):
                        nc.tensor.matmul(
                            ps, lhsT=outT[:, h, ts], rhs=wo[:, h, ns],
                            start=(h == 0), stop=(h == H_LOC - 1),
                        )
                    if evac is nc.scalar:
                        nc.scalar.copy(stg[:, ns], ps)
                    else:
                        evac.tensor_copy(stg[:, ns], ps)
                    if tt >= TT - 4:
                        if n % 2 == 1:
                            hs_ = slice((n - 1) * 512, (n + 1) * 512)
                            nc.sync.dma_start(out=out_ap[ts, hs_], in_=stg[:, hs_])
                    elif n == NS - 1:
                        nc.sync.dma_start(out=out_ap[ts, :], in_=stg)

                return emit

            def wo_chunks_range(tt0, tt1, rotate=False, evacs=None):
                out = []
                rot = [(pmm, "mm"), (ppv, "pv"), (psc, "sc")] if rotate else [(pmm, "mm")]
                evacs = evacs or [nc.vector, nc.scalar]
                k = 0
                for tt in range(tt0, tt1):
                    stg = stage.tile([128, T], BF16, tag="stg", name=f"stg{tt}_{rep}")
                    for n in range(NS):
                        pool, tag = rot[k % len(rot)]
                        out.append(make_wo_chunk(tt, n, stg, pool, tag,
                                                 evacs[k % len(evacs)]))
                        k += 1
                return out

            # ---------------- schedule -----------------------------------------
            # Per strip: attention j immediately after strip-j projections;
            # strip j+1's ssq/k/q/v work follows (matching x DMA arrival).
            # ACT order stays exps(j) before squares(j+1).
            def emit_kqv_slot(jn):
                for h in range(H_LOC):
                    emit_qk_strip(h, jn, k_sb, wk, cosr, sinr)
                emit_cos_fold(jn)
                for h in range(H_LOC):
                    emit_qk_strip(h, jn, q_sb, wq, cos_s, sin_s)
                for tt in range(4 * jn, 4 * (jn + 1)):
                    emit_v_tile(tt)

            emit_ssq_s_strip(0)
            load_part_b()
            emit_kqv_slot(0)
            for j in range(NS - 1):
                ev = [nc.vector] if j >= 2 else [nc.vector, nc.scalar]
                filler = wo_chunks_range(4 * (j - 1), 4 * j, evacs=ev) if j >= 1 else []
                half = len(filler) // 2
                fa, fb = filler[:half], filler[half:]
                emit_attention(0, 512 * j, 512, fa)
                emit_attention(1, 512 * j, 512, fb)
                for f in fa + fb:
                    f()
                if j == 0:
                    emit_ssq_s_strip(1)
                emit_kqv_slot(j + 1)
                if j + 2 < NS:
                    emit_ssq_s_strip(j + 2)
            filler = wo_chunks_range(8, 12, evacs=[nc.vector])
            fa, fb = filler[:6], filler[6:]
            emit_attention(0, 1536, 512, fa)

            TAIL_HOOK = os.environ.get("TAIL_HOOK", "1") == "1"

            def tail_hook():
                for f in wo_chunks_range(12, 14, rotate=True):
                    f()

            emit_attention(1, 1536, 512, fb,
                           tail_hook=tail_hook if TAIL_HOOK else None)
            for f in fa + fb:
                f()
            for f in wo_chunks_range(14, 16 if TAIL_HOOK else 12, rotate=True):
                f()
            if not TAIL_HOOK:
                for f in wo_chunks_range(12, 16, rotate=True):
                    f()

        for _rep in range(repeats):
            emit_body(_rep)

    # Force Exp and Ln onto the single combined table set so the table-load
    # pass emits one ACT_TABLE_LOAD for the whole kernel.
    from concourse.hw_specs import get_activation_tables
    tabs = get_activation_tables(nc.m.arch)
    for nm_, fs_ in tabs.items():
        if nm_ != "natural_log_exp_and_others":
            fs_.discard(AF.Exp)
            fs_.discard(AF.Ln)
    nc.compile()
    _CACHED[repeats] = nc
    return nc


def _host_prep(x, w_ln, wq, wk, wv, wo, cos, sin):
    bf = ml_dtypes.bfloat16
    f8 = mybir.dt.np(FP8)
    x = np.asarray(x, np.float32)
    w_ln = np.asarray(w_ln, np.float32)
    cosT = np.ascontiguousarray(np.asarray(cos, np.float32).T).astype(bf)
    sinTf = np.ascontiguousarray(np.asarray(sin, np.float32).T)
    sinTf[0:64] *= -1.0          # rotate_half sign folded into the table
    sinT = sinTf.astype(bf)
    xT = np.ascontiguousarray(x.T).astype(bf)

    # causal boundary mask for diagonal tiles: mask[p, f] = 1 if f >= p
    f = np.arange(128)[None, :]
    p = np.arange(128)[:, None]
    masks = (f >= p).astype(bf)

    ones128 = np.ones((128, 1), bf)
    ones_dr = np.ones((128, 2, 16), f8)

    wq_s = (np.asarray(wq, np.float32) * w_ln[None, :])
    wk_s = (np.asarray(wk, np.float32) * w_ln[None, :])
    wv_s = (np.asarray(wv, np.float32) * w_ln[None, :])
    wo32 = np.asarray(wo, np.float32)

    in_maps = []
    for c in range(N_CORES):
        sl = slice(c * NL, (c + 1) * NL)
        in_maps.append({
            "xT": xT,
            "wqT": np.ascontiguousarray(wq_s[sl].T).astype(bf),
            "wkT": np.ascontiguousarray(wk_s[sl].T).astype(bf),
            "wvT": np.ascontiguousarray(wv_s[sl].T).astype(bf),
            "woT": np.ascontiguousarray(wo32[:, sl].T).astype(bf),
            "cosT": cosT,
            "sinT": sinT,
            "masks": masks,
            "ones128": ones128,
            "ones_dr": ones_dr,
        })
    return in_maps


def kernel(x, w_ln, wq, wk, wv, wo, cos, sin):
    nc = _build_program()
    in_maps = _host_prep(x, w_ln, wq, wk, wv, wo, cos, sin)
    t0 = time.time()
    res = run_bass_kernel_spmd(nc, in_maps, core_ids=list(range(N_CORES)))
    t1 = time.time()
    print(f"run_bass_kernel_spmd wall: {(t1 - t0) * 1e3:.1f} ms", file=sys.stderr)
    acc = np.zeros((T, D), np.float32)
    for r in res.results:
        acc += np.asarray(r["out"], np.float32)
    return np.asarray(x, np.float32) + acc


# revision 8
# speedup vs baseline: 1.0360x; 1.0009x over previous
"""Trainium2 Bass kernel for nn_Attention (T=2048, D=2048, H=16, Dh=128).

Tensor-parallel over heads, 2 heads per core on 8 cores. v2 schedule:
  - DMA issue order = need order: x strip 0 (quartered) -> wq -> wk ->
    cos/sin strip 0 -> wv -> x s1 -> ... -> wo -> x s3; input x streamed
    strip-major so the RMSNorm scale s[j] unblocks per strip.
  - RMSNorm: per-strip squares (ACT/DVE) + ones-matmul; s = exp(-0.5
    ln(mean+eps)); broadcast via Pool partition_broadcast (no PE);
    per-strip DRAM round trip for the [128, TT] t-tile layout (v scaling).
  - q/k^T projections from resident x^T; RoPE on DVE with s-folded tables.
  - v projected directly in [t, dh] layout (lhsT = x^T tile), evacuated
    via ACT copy with per-partition scale = s (no DMA transpose).
  - causal attention in S^T[tk,tq] layout, per-diagonal-tile trimming;
    strip 0 in bf16; strips 1-3 run PV + softmax-sum matmuls in fp8e4
    DoubleRow (two key tiles per matmul) — exp emitted straight to packed
    fp8 pairs; scores stay bf16 everywhere.
  - softmax normalization deferred: rec = exp(-ln(sum)), Pool broadcast,
    DVE multiply into outT; per-head output projection accumulated in
    PSUM; partial outputs written bf16 (summed f32 on host with residual).
"""

import math
import os
import sys
import time

for _p in ("/opt/trn_rl_repo", "/root/.axon_site/_ro/trn_rl_repo"):
    if os.path.isdir(_p) and _p not in sys.path:
        sys.path.insert(0, _p)

import numpy as np
import ml_dtypes

import concourse.bass as bass
import concourse.tile as tile
from concourse.bass import InstructionNameOrderedSet
from concourse import bacc, mybir
from concourse.bass_utils import run_bass_kernel_spmd

BF16 = mybir.dt.bfloat16
F32 = mybir.dt.float32
FP8 = mybir.dt.float8e4
AF = mybir.ActivationFunctionType

T = 2048
D = 2048
N_H = 16
D_H = 128
N_CORES = 8
H_LOC = N_H // N_CORES          # heads per core = 2
NL = H_LOC * D_H                # local head width = 256
KD = D // 128                   # contraction tiles = 16
TT = T // 128                   # t tiles = 16
NS = T // 512                   # 512-wide strips = 4
EPS = 1e-5
INV_SQRT_DH = 1.0 / math.sqrt(D_H)
FP8_EXP_BIAS = -4.0             # keeps exp() under fp8e4m3 max (saw 8.6 sigma); cancels in norm

FP8_ATT = os.environ.get('FP8_ATT', '1') == '1'                  # fp8 DoubleRow PV+sum for strips >= 1
FP8_SSQ = os.environ.get('FP8_SSQ', '1') == '1'                  # fp8 DoubleRow for sum(x^2)

_CACHED = {}
PHASES = []  # (label, first_instruction_id) — emission-order markers for sim analysis


def _build_program(repeats=1):
    if repeats in _CACHED:
        return _CACHED[repeats]

    nc = bacc.Bacc("TRN2", target_bir_lowering=False, debug=False, num_devices=N_CORES)

    xT_d = nc.dram_tensor("xT", [D, T], BF16, kind="ExternalInput")
    wq_d = nc.dram_tensor("wqT", [D, NL], BF16, kind="ExternalInput")
    wk_d = nc.dram_tensor("wkT", [D, NL], BF16, kind="ExternalInput")
    wv_d = nc.dram_tensor("wvT", [D, NL], BF16, kind="ExternalInput")
    wo_d = nc.dram_tensor("woT", [NL, T], BF16, kind="ExternalInput")
    cos_d = nc.dram_tensor("cosT", [D_H, T], BF16, kind="ExternalInput")
    sin_d = nc.dram_tensor("sinT", [D_H, T], BF16, kind="ExternalInput")
    msk_d = nc.dram_tensor("masks", [128, 128], BF16, kind="ExternalInput")
    on128_d = nc.dram_tensor("ones128", [128, 1], BF16, kind="ExternalInput")
    onedr_d = nc.dram_tensor("ones_dr", [128, 2, 16], FP8, kind="ExternalInput")
    out_d = nc.dram_tensor("out", [T, D], BF16, kind="ExternalOutput")
    DBG = os.environ.get("DBG_OUTT", "0") == "1"
    if DBG:
        outT_dbg = nc.dram_tensor("outT_dbg", [128, H_LOC, T], BF16, kind="ExternalOutput")
        su_dbg = nc.dram_tensor("su_dbg", [H_LOC, T], F32, kind="ExternalOutput")
        sk_dbg = nc.dram_tensor("sk_dbg", [128, TT], F32, kind="ExternalOutput")
        v_dbg = nc.dram_tensor("v_dbg", [128, TT, NL], BF16, kind="ExternalOutput")
        rec_dbg = nc.dram_tensor("rec_dbg", [H_LOC, T], F32, kind="ExternalOutput")
    # DRAM scratch for the s row->tile-layout round trip
    s_scr = nc.dram_tensor("s_scr", [TT, 128], F32, kind="Internal")

    ap = lambda h: h.ap()
    xT, out_ap, s_scr_ap = ap(xT_d), ap(out_d), ap(s_scr)

    from contextlib import ExitStack

    with tile.TileContext(nc) as tc, ExitStack() as ctx:
        P = ctx.enter_context  # noqa

        singles = P(tc.tile_pool(name="singles", bufs=1))
        sq = P(tc.tile_pool(name="sq", bufs=2))            # square scratch
        rope = P(tc.tile_pool(name="rope", bufs=4))        # [128,512] bf16
        qtmp = P(tc.tile_pool(name="qtmp", bufs=2 if os.environ.get("DBG_OUTT","0")=="0" else 1))        # raw qk evac copies
        epool = P(tc.tile_pool(name="epool", bufs=4 if os.environ.get("DBG_OUTT","0")=="0" else 3))      # fp8 exp pairs
        ebf = P(tc.tile_pool(name="ebf", bufs=2 if os.environ.get("DBG_OUTT","0")=="0" else 1))          # bf16 exp tiles (strip 0)
        small = P(tc.tile_pool(name="small", bufs=2))      # [1,512] f32
        bcast = P(tc.tile_pool(name="bcast", bufs=2))      # [128,512] bcast rows
        stage = P(tc.tile_pool(name="stage", bufs=3))      # [128,T] out staging
        pmm = P(tc.tile_pool(name="pmm", bufs=3, space="PSUM"))
        psc = P(tc.tile_pool(name="psc", bufs=2, space="PSUM"))
        ppv = P(tc.tile_pool(name="ppv", bufs=2, space="PSUM"))
        psu = P(tc.tile_pool(name="psu", bufs=1, space="PSUM"))

        def mark(label):
            PHASES.append((label, nc.next_id()))



        def emit_body(rep):
            # ---------------- DMA issue (need-ordered) -------------------------
            mark("dma_issue")
            xt = singles.tile([128, KD, T], BF16, tag="xt")
            xTv = xT.rearrange("(n p) t -> p n t", p=128)

            def load_x_chunk(j, k0, k1):
                js = slice(j * 512, (j + 1) * 512)
                nc.sync.dma_start(out=xt[:, k0:k1, js], in_=xTv[:, k0:k1, js])

            def load_w(dram, tag, split=False):
                t_ = singles.tile([128, KD, NL], BF16, tag=tag)
                v = ap(dram).rearrange("(a p) m -> p a m", p=128)
                if split:
                    nc.sync.dma_start(out=t_[:, 0:8, :], in_=v[:, 0:8, :])
                    nc.sync.dma_start(out=t_[:, 8:16, :], in_=v[:, 8:16, :])
                else:
                    nc.sync.dma_start(out=t_, in_=v)
                return t_

            cosr = singles.tile([128, T], BF16, tag="cosr")
            sinr = singles.tile([128, T], BF16, tag="sinr")

            def load_cs_strip(j):
                js = slice(j * 512, (j + 1) * 512)
                nc.sync.dma_start(out=cosr[:, js], in_=ap(cos_d)[:, js])
                nc.sync.dma_start(out=sinr[:, js], in_=ap(sin_d)[:, js])

            # part A: everything needed before/while s0 resolves.  Later loads
            # are issued after ssq0's round-trip DMAs so the round trip does
            # not queue behind them on the serialized DMA engines.
            load_x_chunk(0, 0, 2)
            on128 = singles.tile([128, 1], BF16, tag="on128")
            nc.sync.dma_start(out=on128, in_=ap(on128_d))
            onedr_f = singles.tile([128, 2, 16], FP8, tag="onedr")
            nc.sync.dma_start(out=onedr_f, in_=ap(onedr_d))
            # dual-fp8 ldweights needs the pair-dim step 16B-aligned
            onedr = onedr_f[:, :, 0:1]
            wk = load_w(wk_d, "wk", split=True)
            load_x_chunk(0, 2, 9)
            load_x_chunk(0, 9, 16)
            load_cs_strip(0)
            wq = load_w(wq_d, "wq")
            wv = load_w(wv_d, "wv")
            msk = singles.tile([128, 128], BF16, tag="msk")
            nc.sync.dma_start(out=msk, in_=ap(msk_d))
            load_cs_strip(1)
            load_x_chunk(1, 0, 4)
            load_x_chunk(1, 4, 8)
            load_x_chunk(1, 8, 12)
            load_x_chunk(1, 12, 16)
            wo = singles.tile([128, H_LOC, T], BF16, tag="wo")

            def load_part_b():
                load_x_chunk(2, 0, 8)
                load_x_chunk(2, 8, 16)
                load_cs_strip(2)
                nc.sync.dma_start(
                    out=wo, in_=ap(wo_d).rearrange("(h p) t -> p h t", p=128))
                load_x_chunk(3, 0, 8)
                load_x_chunk(3, 8, 16)
                load_cs_strip(3)

            # ---------------- persistent SBUF state ----------------------------
            epsb = singles.tile([1, 1], F32, tag="epsb")
            nc.vector.memset(epsb, EPS)
            f8bias = singles.tile([128, 1], F32, tag="f8bias")
            nc.vector.memset(f8bias, FP8_EXP_BIAS)
            s_row = singles.tile([1, T], F32, tag="srow")
            lnm = singles.tile([1, T], F32, tag="lnm")
            cos_s = singles.tile([128, T], BF16, tag="cos_s")
            sin_s = singles.tile([128, T], BF16, tag="sin_s")
            sk_t = singles.tile([128, TT], F32, tag="sk")
            skx = singles.tile([128, TT], F32, tag="skx")
            q_sb = singles.tile([128, H_LOC, T], BF16, tag="q_sb")
            k_sb = singles.tile([128, H_LOC, T], BF16, tag="k_sb")
            v_sb = singles.tile([128, TT, NL], BF16, tag="v_sb")
            if FP8_ATT:
                v_dr = singles.tile([128, TT // 2, 2, NL], FP8, tag="v_dr")
            outT = singles.tile([128, H_LOC, T], BF16, tag="outT")

            # ---------------- per-strip RMSNorm sums + s pipeline ---------------
            def emit_ssq_s_strip(j):
                mark(f"ssq_s{j}")
                js = slice(j * 512, (j + 1) * 512)
                ssq = psu.tile([1, 512], F32, tag="su", name=f"ssq{j}_{rep}")
                # squares striped across ACT/DVE/Pool so no engine serializes
                sq_rot = [1, 2, 1, 2, 1, 1, 2, 1, 1, 2, 1, 1, 2, 1, 2, 1]

                def emit_square(dst, kd):
                    eng = sq_rot[kd]
                    if eng == 0:
                        nc.scalar.activation(dst, xt[:, kd, js], AF.Square)
                    else:
                        (None, nc.vector, nc.gpsimd)[eng].tensor_mul(
                            dst, xt[:, kd, js], xt[:, kd, js]
                        )

                if FP8_SSQ:
                    for p_ in range(KD // 2):
                        xsq = sq.tile([128, 2, 512], FP8, tag="xsq")
                        for m in range(2):
                            emit_square(xsq[:, m, :], 2 * p_ + m)
                        nc.tensor.matmul(
                            ssq, lhsT=onedr, rhs=xsq,
                            start=(p_ == 0), stop=(p_ == KD // 2 - 1),
                            perf_mode=mybir.MatmulPerfMode.DoubleRow,
                        )
                else:
                    for kd in range(KD):
                        xsq = sq.tile([128, 512], BF16, tag="xsq")
                        emit_square(xsq, kd)
                        nc.tensor.matmul(
                            ssq, lhsT=on128, rhs=xsq,
                            start=(kd == 0), stop=(kd == KD - 1),
                        )
                # lnm = ln(mean + eps); s = exp(-0.5 lnm)
                nc.scalar.activation(lnm[:, js], ssq, AF.Ln, bias=epsb, scale=1.0 / D)
                nc.scalar.activation(s_row[:, js], lnm[:, js], AF.Exp, scale=-0.5)
                # round-trip for the [128, 4] t-tile layout slice (v scaling +
                # k-side s folded into the exp scale)
                rt_out = nc.sync.dma_start(
                    out=s_scr_ap[4 * j : 4 * (j + 1), :].rearrange("i p -> () (i p)"),
                    in_=s_row[:, js],
                )
                rt_in = nc.sync.dma_start(
                    out=sk_t[:, 4 * j : 4 * (j + 1)],
                    in_=s_scr_ap.rearrange("i p -> p i")[:, 4 * j : 4 * (j + 1)],
                )
                # DRAM deps are invisible to Tile: force read-after-write
                d1 = InstructionNameOrderedSet(); d1.add(rt_out.ins.name)
                rt_in.ins.add_sync_dependencies_from(d1)
                nc.vector.tensor_scalar_mul(
                    skx[:, 4 * j : 4 * (j + 1)], sk_t[:, 4 * j : 4 * (j + 1)],
                    INV_SQRT_DH,
                )

            def emit_cos_fold(j):
                mark(f"cosf{j}")
                js = slice(j * 512, (j + 1) * 512)
                sb = bcast.tile([128, 512], F32, tag="sb")
                nc.gpsimd.partition_broadcast(sb, s_row[:, js])
                nc.vector.tensor_mul(cos_s[:, js], cosr[:, js], sb)
                nc.vector.tensor_mul(sin_s[:, js], sinr[:, js], sb)

            # ---------------- projections --------------------------------------
            def emit_qk_strip(h, j, dst, w, ctab, stab):
                # q uses the s-folded tables; k uses raw tables (its s is
                # folded into the exp scale instead, so k never waits on s).
                mark(f"{'q' if dst is q_sb else 'k'}{j}h{h}")
                hs = slice(h * 128, (h + 1) * 128)
                js = slice(j * 512, (j + 1) * 512)
                ps = pmm.tile([128, 512], F32, tag="mm")
                for kd in range(KD):
                    nc.tensor.matmul(
                        ps, lhsT=w[:, kd, hs], rhs=xt[:, kd, js],
                        start=(kd == 0), stop=(kd == KD - 1),
                    )
                # m2's half-swap must read PSUM (cross-partition SBUF reads
                # are illegal); the aligned m1 path goes through an ACT copy so
                # the DVE muls get 2x mode and the psum frees quickly.
                qc = qtmp.tile([128, 512], BF16, tag="qc")
                nc.scalar.copy(qc, ps)
                m2 = rope.tile([128, 512], BF16, tag="m2")
                nc.vector.tensor_mul(m2[0:64, :], ps[64:128, :], stab[0:64, js])
                nc.vector.tensor_mul(m2[64:128, :], ps[0:64, :], stab[64:128, js])
                m1 = rope.tile([128, 512], BF16, tag="m1")
                nc.vector.tensor_mul(m1, qc, ctab[:, js])
                nc.gpsimd.tensor_add(dst[:, h, js], m1, m2)

            def emit_v_tile(tt):
                # v[t, dh] directly: lhsT = x^T tile, rhs = wv[d, nl]
                mark(f"v{tt}")
                ts = slice(tt * 128, (tt + 1) * 128)
                ps = pmm.tile([128, NL], F32, tag="mm", name="vps")
                for kd in range(KD):
                    nc.tensor.matmul(
                        ps, lhsT=xt[:, kd, ts], rhs=wv[:, kd, :],
                        start=(kd == 0), stop=(kd == KD - 1),
                    )
                nc.scalar.mul(v_sb[:, tt, :], ps, sk_t[:, tt : tt + 1])
                if FP8_ATT:
                    nc.scalar.mul(
                        v_dr[:, tt // 2, tt % 2, :], ps, sk_t[:, tt : tt + 1]
                    )

            # ---------------- attention ----------------------------------------
            def emit_attention_bf16(h, Q0, W, filler):
                mark(f"att{h}_q{Q0}")
                hs = slice(h * 128, (h + 1) * 128)
                ntk = (Q0 + W) // 128
                po = ppv.tile([128, 512], F32, tag="pv", name="po")[:, :W]
                su = psu.tile([1, 512], F32, tag="su", name="su")[:, :W]
                for i in range(ntk):
                    cb = 128 * i - Q0
                    c0 = max(cb, 0)
                    cs = slice(c0, W)
                    qs = slice(Q0 + c0, Q0 + W)
                    st = psc.tile([128, 512], F32, tag="sc")
                    nc.tensor.matmul(
                        st[:, cs], lhsT=k_sb[:, h, i * 128 : (i + 1) * 128],
                        rhs=q_sb[:, h, qs], start=True, stop=True,
                    )
                    e = ebf.tile([128, 512], BF16, tag="e")
                    nc.scalar.activation(e[:, cs], st[:, cs], AF.Exp,
                                         scale=skx[:, i : i + 1])
                    if cb >= 0:
                        nc.vector.tensor_mul(
                            e[:, cb : cb + 128], e[:, cb : cb + 128], msk
                        )
                    if filler:
                        filler.pop(0)()
                    nc.tensor.matmul(
                        po[:, cs], lhsT=v_sb[:, i, hs], rhs=e[:, cs],
                        start=(i == 0), stop=(i == ntk - 1),
                    )
                    nc.tensor.matmul(
                        su[:, cs], lhsT=on128, rhs=e[:, cs],
                        start=(i == 0), stop=(i == ntk - 1),
                    )
                emit_epilogue(h, Q0, W, po, su)

            def emit_attention_fp8(h, Q0, W, filler, tail_hook=None):
                mark(f"att{h}_q{Q0}f8")
                hs = slice(h * 128, (h + 1) * 128)
                npair = (Q0 + W) // 256
                po = ppv.tile([128, 512], F32, tag="pv", name="po")[:, :W]
                su = psu.tile([1, 512], F32, tag="su", name="su")[:, :W]
                for p_ in range(npair):
                    i0 = 2 * p_
                    c0 = max(128 * i0 - Q0, 0)       # pair-wide col start
                    cs = slice(c0, W)
                    e = epool.tile([128, 2, 512], FP8, tag="edr")
                    for m in range(2):
                        i = i0 + m
                        cb = 128 * i - Q0
                        cm = max(cb, 0)              # member col start
                        st = psc.tile([128, 512], F32, tag="sc")
                        nc.tensor.matmul(
                            st[:, cm:W],
                            lhsT=k_sb[:, h, i * 128 : (i + 1) * 128],
                            rhs=q_sb[:, h, Q0 + cm : Q0 + W],
                            start=True, stop=True,
                        )
                        nc.scalar.activation(
                            e[:, m, cm:W], st[:, cm:W], AF.Exp,
                            bias=f8bias[:, 0:1], scale=skx[:, i : i + 1],
                        )
                        if cm > c0:
                            nc.gpsimd.memset(e[:, m, c0:cm], 0)
                        if cb >= 0 and cb < W:
                            nc.vector.tensor_mul(
                                e[:, m, cb : cb + 128], e[:, m, cb : cb + 128], msk
                            )
                    if filler:
                        filler.pop(0)()
                    nc.tensor.matmul(
                        po[:, cs], lhsT=v_dr[:, p_, :, hs], rhs=e[:, :, cs],
                        start=(p_ == 0), stop=(p_ == npair - 1),
                        perf_mode=mybir.MatmulPerfMode.DoubleRow,
                    )
                    nc.tensor.matmul(
                        su[:, cs], lhsT=onedr, rhs=e[:, :, cs],
                        start=(p_ == 0), stop=(p_ == npair - 1),
                        perf_mode=mybir.MatmulPerfMode.DoubleRow,
                    )
                    if filler:
                        filler.pop(0)()
                    if tail_hook is not None and p_ == npair - 2:
                        emit_epilogue_piece(h, Q0, po, su, 0, W - 384)
                        emit_epilogue_piece(h, Q0, po, su, W - 384, W - 256)
                        tail_hook()
                if tail_hook is not None:
                    emit_epilogue_piece(h, Q0, po, su, W - 256, W - 128)
                    emit_epilogue_piece(h, Q0, po, su, W - 128, W)
                else:
                    emit_epilogue(h, Q0, W, po, su)

            def emit_epilogue_piece(h, Q0, po, su, c0, c1):
                mark(f"epp{h}_q{Q0}_{c0}")
                rec = small.tile([1, 512], F32, tag="rec", name="rec")[:, c0:c1]
                nc.vector.reciprocal_approx_fast(rec, su[:, c0:c1])
                rb = bcast.tile([128, 512], F32, tag="rb", name="rb")[:, c0:c1]
                nc.gpsimd.partition_broadcast(rb, rec)
                nc.vector.tensor_mul(outT[:, h, Q0 + c0 : Q0 + c1], po[:, c0:c1], rb)

            def emit_epilogue(h, Q0, W, po, su):
                mark(f"epi{h}_q{Q0}")
                rec = small.tile([1, 512], F32, tag="rec", name="rec")[:, :W]
                nc.vector.reciprocal_approx_fast(rec, su)
                rb = bcast.tile([128, 512], F32, tag="rb", name="rb")[:, :W]
                nc.gpsimd.partition_broadcast(rb, rec)
                nc.vector.tensor_mul(outT[:, h, Q0 : Q0 + W], po, rb)
                if DBG:
                    sud = small.tile([1, 512], F32, tag="sud", name="sud")[:, :W]
                    nc.vector.tensor_copy(sud, su)
                    nc.sync.dma_start(out=su_dbg.ap()[h : h + 1, Q0 : Q0 + W], in_=sud)
                    nc.sync.dma_start(out=rec_dbg.ap()[h : h + 1, Q0 : Q0 + W], in_=rec)

            def emit_attention(h, Q0, W, filler, tail_hook=None):
                if FP8_ATT and Q0 >= 512:
                    emit_attention_fp8(h, Q0, W, filler, tail_hook)
                else:
                    emit_attention_bf16(h, Q0, W, filler)

            # ---------------- output projection --------------------------------
            def make_wo_chunk(tt, n, stg, pool, tag, evac):
                ts = slice(tt * 128, (tt + 1) * 128)
                ns = slice(n * 512, (n + 1) * 512)

                def emit():
                    mark(f"wo_t{tt}n{n}")
                    ps = pool.tile([128, 512], F32, tag=tag)
                    for h in range(H_LOC):
                        nc.tensor.matmul(
                            ps, lhsT=outT[:, h, ts], rhs=wo[:, h, ns],
                            start=(h == 0), stop=(h == H_LOC - 1),
                        )
                    if evac is nc.scalar:
                        nc.scalar.copy(stg[:, ns], ps)
                    else:
                        evac.tensor_copy(stg[:, ns], ps)
                    if tt >= TT - 4:
                        if n % 2 == 1:
                            hs_ = slice((n - 1) * 512, (n + 1) * 512)
                            nc.sync.dma_start(out=out_ap[ts, hs_], in_=stg[:, hs_])
                    elif n == NS - 1:
                        nc.sync.dma_start(out=out_ap[ts, :], in_=stg)

                return emit

            def wo_chunks_range(tt0, tt1, rotate=False, evacs=None):
                out = []
                rot = [(pmm, "mm"), (ppv, "pv"), (psc, "sc")] if rotate else [(pmm, "mm")]
                evacs = evacs or [nc.vector, nc.scalar]
                k = 0
                for tt in range(tt0, tt1):
                    stg = stage.tile([128, T], BF16, tag="stg", name=f"stg{tt}_{rep}")
                    for n in range(NS):
                        pool, tag = rot[k % len(rot)]
                        out.append(make_wo_chunk(tt, n, stg, pool, tag,
                                                 evacs[k % len(evacs)]))
                        k += 1
                return out

            # ---------------- schedule -----------------------------------------
            # Per strip: attention j immediately after strip-j projections;
            # strip j+1's ssq/k/q/v work follows (matching x DMA arrival).
            # ACT order stays exps(j) before squares(j+1).
            def emit_kqv_slot(jn):
                for h in range(H_LOC):
                    emit_qk_strip(h, jn, k_sb, wk, cosr, sinr)
                emit_cos_fold(jn)
                for h in range(H_LOC):
                    emit_qk_strip(h, jn, q_sb, wq, cos_s, sin_s)
                for tt in range(4 * jn, 4 * (jn + 1)):
                    emit_v_tile(tt)

            emit_ssq_s_strip(0)
            load_part_b()
            emit_kqv_slot(0)
            for j in range(NS - 1):
                ev = [nc.vector] if j >= 2 else [nc.vector, nc.scalar]
                filler = wo_chunks_range(4 * (j - 1), 4 * j, evacs=ev) if j >= 1 else []
                half = len(filler) // 2
                fa, fb = filler[:half], filler[half:]
                emit_attention(0, 512 * j, 512, fa)
                emit_attention(1, 512 * j, 512, fb)
                for f in fa + fb:
                    f()
                if j == 0:
                    emit_ssq_s_strip(1)
                emit_kqv_slot(j + 1)
                if j + 2 < NS:
                    emit_ssq_s_strip(j + 2)
            filler = wo_chunks_range(8, 12, evacs=[nc.vector])
            fa, fb = filler[:6], filler[6:]
            emit_attention(0, 1536, 512, fa)

            TAIL_HOOK = os.environ.get("TAIL_HOOK", "1") == "1"

            def tail_hook():
                for f in wo_chunks_range(12, 14, rotate=True):
                    f()

            emit_attention(1, 1536, 512, fb,
                           tail_hook=tail_hook if TAIL_HOOK else None)
            for f in fa + fb:
                f()
            for f in wo_chunks_range(14, 16 if TAIL_HOOK else 12, rotate=True):
                f()
            if not TAIL_HOOK:
                for f in wo_chunks_range(12, 16, rotate=True):
                    f()

        for _rep in range(repeats):
            emit_body(_rep)

    # Force Exp and Ln onto the single combined table set so the table-load
    # pass emits one ACT_TABLE_LOAD for the whole kernel.
    from concourse.hw_specs import get_activation_tables
    tabs = get_activation_tables(nc.m.arch)
    for nm_, fs_ in tabs.items():
        if nm_ != "natural_log_exp_and_others":
            fs_.discard(AF.Exp)
            fs_.discard(AF.Ln)
    nc.compile()
    _CACHED[repeats] = nc
    return nc


def _host_prep(x, w_ln, wq, wk, wv, wo, cos, sin):
    bf = ml_dtypes.bfloat16
    f8 = mybir.dt.np(FP8)
    x = np.asarray(x, np.float32)
    w_ln = np.asarray(w_ln, np.float32)
    cosT = np.ascontiguousarray(np.asarray(cos, np.float32).T).astype(bf)
    sinTf = np.ascontiguousarray(np.asarray(sin, np.float32).T)
    sinTf[0:64] *= -1.0          # rotate_half sign folded into the table
    sinT = sinTf.astype(bf)
    xT = np.ascontiguousarray(x.T).astype(bf)

    # causal boundary mask for diagonal tiles: mask[p, f] = 1 if f >= p
    f = np.arange(128)[None, :]
    p = np.arange(128)[:, None]
    masks = (f >= p).astype(bf)

    ones128 = np.ones((128, 1), bf)
    ones_dr = np.ones((128, 2, 16), f8)

    wq_s = (np.asarray(wq, np.float32) * w_ln[None, :])
    wk_s = (np.asarray(wk, np.float32) * w_ln[None, :])
    wv_s = (np.asarray(wv, np.float32) * w_ln[None, :])
    wo32 = np.asarray(wo, np.float32)

    in_maps = []
    for c in range(N_CORES):
        sl = slice(c * NL, (c + 1) * NL)
        in_maps.append({
            "xT": xT,
            "wqT": np.ascontiguousarray(wq_s[sl].T).astype(bf),
            "wkT": np.ascontiguousarray(wk_s[sl].T).astype(bf),
            "wvT": np.ascontiguousarray(wv_s[sl].T).astype(bf),
            "woT": np.ascontiguousarray(wo32[:, sl].T).astype(bf),
            "cosT": cosT,
            "sinT": sinT,
            "masks": masks,
            "ones128": ones128,
            "ones_dr": ones_dr,
        })
    return in_maps


def kernel(x, w_ln, wq, wk, wv, wo, cos, sin):
    nc = _build_program()
    in_maps = _host_prep(x, w_ln, wq, wk, wv, wo, cos, sin)
    t0 = time.time()
    res = run_bass_kernel_spmd(nc, in_maps, core_ids=list(range(N_CORES)))
    t1 = time.time()
    print(f"run_bass_kernel_spmd wall: {(t1 - t0) * 1e3:.1f} ms", file=sys.stderr)
    acc = np.zeros((T, D), np.float32)
    for r in res.results:
        acc += np.asarray(r["out"], np.float32)
    return np.asarray(x, np.float32) + acc


# revision 9
# speedup vs baseline: 1.0464x; 1.0100x over previous
"""Trainium2 Bass kernel for nn_Attention (T=2048, D=2048, H=16, Dh=128).

Tensor-parallel over heads, 2 heads per core on 8 cores. v2 schedule:
  - DMA issue order = need order: x strip 0 (quartered) -> wq -> wk ->
    cos/sin strip 0 -> wv -> x s1 -> ... -> wo -> x s3; input x streamed
    strip-major so the RMSNorm scale s[j] unblocks per strip.
  - RMSNorm: per-strip squares (ACT/DVE) + ones-matmul; s = exp(-0.5
    ln(mean+eps)); broadcast via Pool partition_broadcast (no PE);
    per-strip DRAM round trip for the [128, TT] t-tile layout (v scaling).
  - q/k^T projections from resident x^T; RoPE on DVE with s-folded tables.
  - v projected directly in [t, dh] layout (lhsT = x^T tile), evacuated
    via ACT copy with per-partition scale = s (no DMA transpose).
  - causal attention in S^T[tk,tq] layout, per-diagonal-tile trimming;
    strip 0 in bf16; strips 1-3 run PV + softmax-sum matmuls in fp8e4
    DoubleRow (two key tiles per matmul) — exp emitted straight to packed
    fp8 pairs; scores stay bf16 everywhere.
  - softmax normalization deferred: rec = exp(-ln(sum)), Pool broadcast,
    DVE multiply into outT; per-head output projection accumulated in
    PSUM; partial outputs written bf16 (summed f32 on host with residual).
"""

import math
import os
import sys
import time

for _p in ("/opt/trn_rl_repo", "/root/.axon_site/_ro/trn_rl_repo"):
    if os.path.isdir(_p) and _p not in sys.path:
        sys.path.insert(0, _p)

import numpy as np
import ml_dtypes

import concourse.bass as bass
import concourse.tile as tile
from concourse.bass import InstructionNameOrderedSet
from concourse import bacc, mybir
from concourse.bass_utils import run_bass_kernel_spmd

BF16 = mybir.dt.bfloat16
F32 = mybir.dt.float32
FP8 = mybir.dt.float8e4
AF = mybir.ActivationFunctionType

T = 2048
D = 2048
N_H = 16
D_H = 128
N_CORES = 8
H_LOC = N_H // N_CORES          # heads per core = 2
NL = H_LOC * D_H                # local head width = 256
KD = D // 128                   # contraction tiles = 16
TT = T // 128                   # t tiles = 16
NS = T // 512                   # 512-wide strips = 4
EPS = 1e-5
INV_SQRT_DH = 1.0 / math.sqrt(D_H)
FP8_EXP_BIAS = -4.0             # keeps exp() under fp8e4m3 max (saw 8.6 sigma); cancels in norm

FP8_ATT = os.environ.get('FP8_ATT', '1') == '1'                  # fp8 DoubleRow PV+sum for strips >= 1
FP8_SSQ = os.environ.get('FP8_SSQ', '1') == '1'                  # fp8 DoubleRow for sum(x^2)

_CACHED = {}
PHASES = []  # (label, first_instruction_id) — emission-order markers for sim analysis


def _build_program(repeats=1):
    if repeats in _CACHED:
        return _CACHED[repeats]

    nc = bacc.Bacc("TRN2", target_bir_lowering=False, debug=False, num_devices=N_CORES)

    xT_d = nc.dram_tensor("xT", [D, T], BF16, kind="ExternalInput")
    wq_d = nc.dram_tensor("wqT", [D, NL], BF16, kind="ExternalInput")
    wk_d = nc.dram_tensor("wkT", [D, NL], BF16, kind="ExternalInput")
    wv_d = nc.dram_tensor("wvT", [D, NL], BF16, kind="ExternalInput")
    wo_d = nc.dram_tensor("woT", [NL, T], BF16, kind="ExternalInput")
    cos_d = nc.dram_tensor("cosT", [D_H, T], BF16, kind="ExternalInput")
    sin_d = nc.dram_tensor("sinT", [D_H, T], BF16, kind="ExternalInput")
    msk_d = nc.dram_tensor("masks", [128, 128], BF16, kind="ExternalInput")
    on128_d = nc.dram_tensor("ones128", [128, 1], BF16, kind="ExternalInput")
    onedr_d = nc.dram_tensor("ones_dr", [128, 2, 16], FP8, kind="ExternalInput")
    out_d = nc.dram_tensor("out", [T, D], BF16, kind="ExternalOutput")
    DBG = os.environ.get("DBG_OUTT", "0") == "1"
    if DBG:
        outT_dbg = nc.dram_tensor("outT_dbg", [128, H_LOC, T], BF16, kind="ExternalOutput")
        su_dbg = nc.dram_tensor("su_dbg", [H_LOC, T], F32, kind="ExternalOutput")
        sk_dbg = nc.dram_tensor("sk_dbg", [128, TT], F32, kind="ExternalOutput")
        v_dbg = nc.dram_tensor("v_dbg", [128, TT, NL], BF16, kind="ExternalOutput")
        rec_dbg = nc.dram_tensor("rec_dbg", [H_LOC, T], F32, kind="ExternalOutput")
    # DRAM scratch for the s row->tile-layout round trip
    s_scr = nc.dram_tensor("s_scr", [TT, 128], F32, kind="Internal")

    ap = lambda h: h.ap()
    xT, out_ap, s_scr_ap = ap(xT_d), ap(out_d), ap(s_scr)

    from contextlib import ExitStack

    with tile.TileContext(nc) as tc, ExitStack() as ctx:
        P = ctx.enter_context  # noqa

        singles = P(tc.tile_pool(name="singles", bufs=1))
        sq = P(tc.tile_pool(name="sq", bufs=2))            # square scratch
        rope = P(tc.tile_pool(name="rope", bufs=4))        # [128,512] bf16
        qtmp = P(tc.tile_pool(name="qtmp", bufs=2 if os.environ.get("DBG_OUTT","0")=="0" else 1))        # raw qk evac copies
        epool = P(tc.tile_pool(name="epool", bufs=4 if os.environ.get("DBG_OUTT","0")=="0" else 3))      # fp8 exp pairs
        ebf = P(tc.tile_pool(name="ebf", bufs=2 if os.environ.get("DBG_OUTT","0")=="0" else 1))          # bf16 exp tiles (strip 0)
        small = P(tc.tile_pool(name="small", bufs=2))      # [1,512] f32
        bcast = P(tc.tile_pool(name="bcast", bufs=2))      # [128,512] bcast rows
        stage = P(tc.tile_pool(name="stage", bufs=3))      # [128,T] out staging
        pmm = P(tc.tile_pool(name="pmm", bufs=3, space="PSUM"))
        psc = P(tc.tile_pool(name="psc", bufs=2, space="PSUM"))
        ppv = P(tc.tile_pool(name="ppv", bufs=2, space="PSUM"))
        psu = P(tc.tile_pool(name="psu", bufs=1, space="PSUM"))

        def mark(label):
            PHASES.append((label, nc.next_id()))



        def emit_body(rep):
            # ---------------- DMA issue (need-ordered) -------------------------
            mark("dma_issue")
            xt = singles.tile([128, KD, T], BF16, tag="xt")
            xTv = xT.rearrange("(n p) t -> p n t", p=128)

            def load_x_chunk(j, k0, k1):
                js = slice(j * 512, (j + 1) * 512)
                nc.sync.dma_start(out=xt[:, k0:k1, js], in_=xTv[:, k0:k1, js])

            def load_w(dram, tag, split=False):
                t_ = singles.tile([128, KD, NL], BF16, tag=tag)
                v = ap(dram).rearrange("(a p) m -> p a m", p=128)
                if split:
                    nc.sync.dma_start(out=t_[:, 0:8, :], in_=v[:, 0:8, :])
                    nc.sync.dma_start(out=t_[:, 8:16, :], in_=v[:, 8:16, :])
                else:
                    nc.sync.dma_start(out=t_, in_=v)
                return t_

            cosr = singles.tile([128, T], BF16, tag="cosr")
            sinr = singles.tile([128, T], BF16, tag="sinr")

            def load_cs_strip(j):
                js = slice(j * 512, (j + 1) * 512)
                nc.sync.dma_start(out=cosr[:, js], in_=ap(cos_d)[:, js])
                nc.sync.dma_start(out=sinr[:, js], in_=ap(sin_d)[:, js])

            # part A: everything needed before/while s0 resolves.  Later loads
            # are issued after ssq0's round-trip DMAs so the round trip does
            # not queue behind them on the serialized DMA engines.
            load_x_chunk(0, 0, 2)
            on128 = singles.tile([128, 1], BF16, tag="on128")
            nc.sync.dma_start(out=on128, in_=ap(on128_d))
            onedr_f = singles.tile([128, 2, 16], FP8, tag="onedr")
            nc.sync.dma_start(out=onedr_f, in_=ap(onedr_d))
            # dual-fp8 ldweights needs the pair-dim step 16B-aligned
            onedr = onedr_f[:, :, 0:1]
            wk = load_w(wk_d, "wk", split=True)
            load_x_chunk(0, 2, 9)
            load_x_chunk(0, 9, 16)
            load_cs_strip(0)
            wq = load_w(wq_d, "wq")
            wv = load_w(wv_d, "wv")
            msk = singles.tile([128, 128], BF16, tag="msk")
            nc.sync.dma_start(out=msk, in_=ap(msk_d))
            load_cs_strip(1)
            load_x_chunk(1, 0, 4)
            load_x_chunk(1, 4, 8)
            load_x_chunk(1, 8, 12)
            load_x_chunk(1, 12, 16)
            wo = singles.tile([128, H_LOC, T], BF16, tag="wo")

            def load_part_b():
                load_x_chunk(2, 0, 8)
                load_x_chunk(2, 8, 16)
                load_cs_strip(2)
                nc.sync.dma_start(
                    out=wo, in_=ap(wo_d).rearrange("(h p) t -> p h t", p=128))
                load_x_chunk(3, 0, 8)
                load_x_chunk(3, 8, 16)
                load_cs_strip(3)

            # ---------------- persistent SBUF state ----------------------------
            epsb = singles.tile([1, 1], F32, tag="epsb")
            nc.vector.memset(epsb, EPS)
            f8bias = singles.tile([128, 1], F32, tag="f8bias")
            nc.vector.memset(f8bias, FP8_EXP_BIAS)
            s_row = singles.tile([1, T], F32, tag="srow")
            lnm = singles.tile([1, T], F32, tag="lnm")
            cos_s = singles.tile([128, T], BF16, tag="cos_s")
            sin_s = singles.tile([128, T], BF16, tag="sin_s")
            sk_t = singles.tile([128, TT], F32, tag="sk")
            skx = singles.tile([128, TT], F32, tag="skx")
            q_sb = singles.tile([128, H_LOC, T], BF16, tag="q_sb")
            k_sb = singles.tile([128, H_LOC, T], BF16, tag="k_sb")
            v_sb = singles.tile([128, TT, NL], BF16, tag="v_sb")
            if FP8_ATT:
                v_dr = singles.tile([128, TT // 2, 2, NL], FP8, tag="v_dr")
            outT = singles.tile([128, H_LOC, T], BF16, tag="outT")

            # ---------------- per-strip RMSNorm sums + s pipeline ---------------
            def emit_ssq_s_strip(j):
                mark(f"ssq_s{j}")
                js = slice(j * 512, (j + 1) * 512)
                ssq = psu.tile([1, 512], F32, tag="su", name=f"ssq{j}_{rep}")
                # squares striped across ACT/DVE/Pool so no engine serializes
                sq_rot = [1, 2, 1, 2, 1, 1, 2, 1, 1, 2, 1, 1, 2, 1, 2, 1]

                def emit_square(dst, kd):
                    eng = sq_rot[kd]
                    if eng == 0:
                        nc.scalar.activation(dst, xt[:, kd, js], AF.Square)
                    else:
                        (None, nc.vector, nc.gpsimd)[eng].tensor_mul(
                            dst, xt[:, kd, js], xt[:, kd, js]
                        )

                if FP8_SSQ:
                    for p_ in range(KD // 2):
                        xsq = sq.tile([128, 2, 512], FP8, tag="xsq")
                        for m in range(2):
                            emit_square(xsq[:, m, :], 2 * p_ + m)
                        nc.tensor.matmul(
                            ssq, lhsT=onedr, rhs=xsq,
                            start=(p_ == 0), stop=(p_ == KD // 2 - 1),
                            perf_mode=mybir.MatmulPerfMode.DoubleRow,
                        )
                else:
                    for kd in range(KD):
                        xsq = sq.tile([128, 512], BF16, tag="xsq")
                        emit_square(xsq, kd)
                        nc.tensor.matmul(
                            ssq, lhsT=on128, rhs=xsq,
                            start=(kd == 0), stop=(kd == KD - 1),
                        )
                # lnm = ln(mean + eps); s = exp(-0.5 lnm)
                nc.scalar.activation(lnm[:, js], ssq, AF.Ln, bias=epsb, scale=1.0 / D)
                nc.scalar.activation(s_row[:, js], lnm[:, js], AF.Exp, scale=-0.5)
                # round-trip for the [128, 4] t-tile layout slice (v scaling +
                # k-side s folded into the exp scale)
                rt_out = nc.sync.dma_start(
                    out=s_scr_ap[4 * j : 4 * (j + 1), :].rearrange("i p -> () (i p)"),
                    in_=s_row[:, js],
                )
                rt_in = nc.sync.dma_start(
                    out=sk_t[:, 4 * j : 4 * (j + 1)],
                    in_=s_scr_ap.rearrange("i p -> p i")[:, 4 * j : 4 * (j + 1)],
                )
                # DRAM deps are invisible to Tile: force read-after-write
                d1 = InstructionNameOrderedSet(); d1.add(rt_out.ins.name)
                rt_in.ins.add_sync_dependencies_from(d1)
                nc.vector.tensor_scalar_mul(
                    skx[:, 4 * j : 4 * (j + 1)], sk_t[:, 4 * j : 4 * (j + 1)],
                    INV_SQRT_DH,
                )

            def emit_cos_fold(j):
                mark(f"cosf{j}")
                js = slice(j * 512, (j + 1) * 512)
                sb = bcast.tile([128, 512], F32, tag="sb")
                nc.gpsimd.partition_broadcast(sb, s_row[:, js])
                nc.vector.tensor_mul(cos_s[:, js], cosr[:, js], sb)
                nc.vector.tensor_mul(sin_s[:, js], sinr[:, js], sb)

            # ---------------- projections --------------------------------------
            def emit_qk_strip(h, j, dst, w, ctab, stab):
                # q uses the s-folded tables; k uses raw tables (its s is
                # folded into the exp scale instead, so k never waits on s).
                mark(f"{'q' if dst is q_sb else 'k'}{j}h{h}")
                hs = slice(h * 128, (h + 1) * 128)
                js = slice(j * 512, (j + 1) * 512)
                ps = pmm.tile([128, 512], F32, tag="mm")
                for kd in range(KD):
                    nc.tensor.matmul(
                        ps, lhsT=w[:, kd, hs], rhs=xt[:, kd, js],
                        start=(kd == 0), stop=(kd == KD - 1),
                    )
                # m2's half-swap must read PSUM (cross-partition SBUF reads
                # are illegal); the aligned m1 path goes through an ACT copy so
                # the DVE muls get 2x mode and the psum frees quickly.
                qc = qtmp.tile([128, 512], BF16, tag="qc")
                nc.scalar.copy(qc, ps)
                m2 = rope.tile([128, 512], BF16, tag="m2")
                nc.vector.tensor_mul(m2[0:64, :], ps[64:128, :], stab[0:64, js])
                nc.vector.tensor_mul(m2[64:128, :], ps[0:64, :], stab[64:128, js])
                m1 = rope.tile([128, 512], BF16, tag="m1")
                nc.vector.tensor_mul(m1, qc, ctab[:, js])
                nc.gpsimd.tensor_add(dst[:, h, js], m1, m2)

            def emit_v_tile(tt):
                # v[t, dh] directly: lhsT = x^T tile, rhs = wv[d, nl]
                mark(f"v{tt}")
                ts = slice(tt * 128, (tt + 1) * 128)
                ps = pmm.tile([128, NL], F32, tag="mm", name="vps")
                for kd in range(KD):
                    nc.tensor.matmul(
                        ps, lhsT=xt[:, kd, ts], rhs=wv[:, kd, :],
                        start=(kd == 0), stop=(kd == KD - 1),
                    )
                nc.scalar.mul(v_sb[:, tt, :], ps, sk_t[:, tt : tt + 1])
                if FP8_ATT:
                    nc.scalar.mul(
                        v_dr[:, tt // 2, tt % 2, :], ps, sk_t[:, tt : tt + 1]
                    )

            # ---------------- attention ----------------------------------------
            def emit_attention_bf16(h, Q0, W, filler):
                mark(f"att{h}_q{Q0}")
                hs = slice(h * 128, (h + 1) * 128)
                ntk = (Q0 + W) // 128
                po = ppv.tile([128, 512], F32, tag="pv", name="po")[:, :W]
                su = psu.tile([1, 512], F32, tag="su", name="su")[:, :W]
                for i in range(ntk):
                    cb = 128 * i - Q0
                    c0 = max(cb, 0)
                    cs = slice(c0, W)
                    qs = slice(Q0 + c0, Q0 + W)
                    st = psc.tile([128, 512], F32, tag="sc")
                    nc.tensor.matmul(
                        st[:, cs], lhsT=k_sb[:, h, i * 128 : (i + 1) * 128],
                        rhs=q_sb[:, h, qs], start=True, stop=True,
                    )
                    e = ebf.tile([128, 512], BF16, tag="e")
                    nc.scalar.activation(e[:, cs], st[:, cs], AF.Exp,
                                         scale=skx[:, i : i + 1])
                    if cb >= 0:
                        nc.vector.tensor_mul(
                            e[:, cb : cb + 128], e[:, cb : cb + 128], msk
                        )
                    if filler:
                        filler.pop(0)()
                    nc.tensor.matmul(
                        po[:, cs], lhsT=v_sb[:, i, hs], rhs=e[:, cs],
                        start=(i == 0), stop=(i == ntk - 1),
                    )
                    nc.tensor.matmul(
                        su[:, cs], lhsT=on128, rhs=e[:, cs],
                        start=(i == 0), stop=(i == ntk - 1),
                    )
                emit_epilogue(h, Q0, W, po, su)

            def emit_attention_fp8(h, Q0, W, filler, tail_hook=None):
                mark(f"att{h}_q{Q0}f8")
                hs = slice(h * 128, (h + 1) * 128)
                npair = (Q0 + W) // 256
                po = ppv.tile([128, 512], F32, tag="pv", name="po")[:, :W]
                su = psu.tile([1, 512], F32, tag="su", name="su")[:, :W]
                for p_ in range(npair):
                    i0 = 2 * p_
                    c0 = max(128 * i0 - Q0, 0)       # pair-wide col start
                    cs = slice(c0, W)
                    e = epool.tile([128, 2, 512], FP8, tag="edr")
                    for m in range(2):
                        i = i0 + m
                        cb = 128 * i - Q0
                        cm = max(cb, 0)              # member col start
                        st = psc.tile([128, 512], F32, tag="sc")
                        nc.tensor.matmul(
                            st[:, cm:W],
                            lhsT=k_sb[:, h, i * 128 : (i + 1) * 128],
                            rhs=q_sb[:, h, Q0 + cm : Q0 + W],
                            start=True, stop=True,
                        )
                        nc.scalar.activation(
                            e[:, m, cm:W], st[:, cm:W], AF.Exp,
                            bias=f8bias[:, 0:1], scale=skx[:, i : i + 1],
                        )
                        if cm > c0:
                            nc.gpsimd.memset(e[:, m, c0:cm], 0)
                        if cb >= 0 and cb < W:
                            nc.vector.tensor_mul(
                                e[:, m, cb : cb + 128], e[:, m, cb : cb + 128], msk
                            )
                    if filler:
                        filler.pop(0)()
                    nc.tensor.matmul(
                        po[:, cs], lhsT=v_dr[:, p_, :, hs], rhs=e[:, :, cs],
                        start=(p_ == 0), stop=(p_ == npair - 1),
                        perf_mode=mybir.MatmulPerfMode.DoubleRow,
                    )
                    nc.tensor.matmul(
                        su[:, cs], lhsT=onedr, rhs=e[:, :, cs],
                        start=(p_ == 0), stop=(p_ == npair - 1),
                        perf_mode=mybir.MatmulPerfMode.DoubleRow,
                    )
                    if filler:
                        filler.pop(0)()
                    if tail_hook is not None and p_ == npair - 2:
                        emit_epilogue_piece(h, Q0, po, su, 0, W - 384)
                        emit_epilogue_piece(h, Q0, po, su, W - 384, W - 256)
                        tail_hook()
                if tail_hook is not None:
                    emit_epilogue_piece(h, Q0, po, su, W - 256, W - 128)
                    emit_epilogue_piece(h, Q0, po, su, W - 128, W)
                else:
                    emit_epilogue(h, Q0, W, po, su)

            def emit_epilogue_piece(h, Q0, po, su, c0, c1):
                mark(f"epp{h}_q{Q0}_{c0}")
                rec = small.tile([1, 512], F32, tag="rec", name="rec")[:, c0:c1]
                nc.vector.reciprocal_approx_fast(rec, su[:, c0:c1])
                rb = bcast.tile([128, 512], F32, tag="rb", name="rb")[:, c0:c1]
                nc.gpsimd.partition_broadcast(rb, rec)
                nc.vector.tensor_mul(outT[:, h, Q0 + c0 : Q0 + c1], po[:, c0:c1], rb)

            def emit_epilogue(h, Q0, W, po, su):
                mark(f"epi{h}_q{Q0}")
                rec = small.tile([1, 512], F32, tag="rec", name="rec")[:, :W]
                nc.vector.reciprocal_approx_fast(rec, su)
                rb = bcast.tile([128, 512], F32, tag="rb", name="rb")[:, :W]
                nc.gpsimd.partition_broadcast(rb, rec)
                nc.vector.tensor_mul(outT[:, h, Q0 : Q0 + W], po, rb)
                if DBG:
                    sud = small.tile([1, 512], F32, tag="sud", name="sud")[:, :W]
                    nc.vector.tensor_copy(sud, su)
                    nc.sync.dma_start(out=su_dbg.ap()[h : h + 1, Q0 : Q0 + W], in_=sud)
                    nc.sync.dma_start(out=rec_dbg.ap()[h : h + 1, Q0 : Q0 + W], in_=rec)

            def emit_attention(h, Q0, W, filler, tail_hook=None):
                if FP8_ATT and Q0 >= 512:
                    emit_attention_fp8(h, Q0, W, filler, tail_hook)
                else:
                    emit_attention_bf16(h, Q0, W, filler)

            # ---------------- output projection --------------------------------
            def make_wo_chunk(tt, n, stg, pool, tag, evac):
                ts = slice(tt * 128, (tt + 1) * 128)
                ns = slice(n * 512, (n + 1) * 512)

                def emit():
                    mark(f"wo_t{tt}n{n}")
                    ps = pool.tile([128, 512], F32, tag=tag)
                    for h in range(H_LOC):
                        nc.tensor.matmul(
                            ps, lhsT=outT[:, h, ts], rhs=wo[:, h, ns],
                            start=(h == 0), stop=(h == H_LOC - 1),
                        )
                    if evac is nc.scalar:
                        nc.scalar.copy(stg[:, ns], ps)
                    else:
                        evac.tensor_copy(stg[:, ns], ps)
                    if tt >= TT - 4:
                        if n % 2 == 1:
                            hs_ = slice((n - 1) * 512, (n + 1) * 512)
                            nc.sync.dma_start(out=out_ap[ts, hs_], in_=stg[:, hs_])
                    elif n == NS - 1:
                        nc.sync.dma_start(out=out_ap[ts, :], in_=stg)

                return emit

            def wo_chunks_range(tt0, tt1, rotate=False, evacs=None):
                out = []
                rot = [(pmm, "mm"), (ppv, "pv"), (psc, "sc")] if rotate else [(pmm, "mm")]
                evacs = evacs or [nc.vector, nc.scalar]
                k = 0
                for tt in range(tt0, tt1):
                    stg = stage.tile([128, T], BF16, tag="stg", name=f"stg{tt}_{rep}")
                    for n in range(NS):
                        pool, tag = rot[k % len(rot)]
                        out.append(make_wo_chunk(tt, n, stg, pool, tag,
                                                 evacs[k % len(evacs)]))
                        k += 1
                return out

            # ---------------- schedule -----------------------------------------
            # Per strip: attention j immediately after strip-j projections;
            # strip j+1's ssq/k/q/v work follows (matching x DMA arrival).
            # ACT order stays exps(j) before squares(j+1).
            def emit_kqv_slot(jn):
                for h in range(H_LOC):
                    emit_qk_strip(h, jn, k_sb, wk, cosr, sinr)
                emit_cos_fold(jn)
                for h in range(H_LOC):
                    emit_qk_strip(h, jn, q_sb, wq, cos_s, sin_s)
                for tt in range(4 * jn, 4 * (jn + 1)):
                    emit_v_tile(tt)

            emit_ssq_s_strip(0)
            load_part_b()
            emit_kqv_slot(0)
            for j in range(NS - 1):
                ev = [nc.vector] if j >= 2 else [nc.vector, nc.scalar]
                filler = wo_chunks_range(4 * (j - 1), 4 * j, evacs=ev) if j >= 1 else []
                half = len(filler) // 2
                fa, fb = filler[:half], filler[half:]
                emit_attention(0, 512 * j, 512, fa)
                emit_attention(1, 512 * j, 512, fb)
                for f in fa + fb:
                    f()
                if j == 0:
                    emit_ssq_s_strip(1)
                emit_kqv_slot(j + 1)
                if j + 2 < NS:
                    emit_ssq_s_strip(j + 2)
            filler = wo_chunks_range(8, 12, evacs=[nc.vector])
            fa, fb = filler[:6], filler[6:]
            emit_attention(0, 1536, 512, fa)

            TAIL_HOOK = os.environ.get("TAIL_HOOK", "1") == "1"

            def tail_hook():
                for f in wo_chunks_range(12, 13, rotate=True):
                    f()

            emit_attention(1, 1536, 512, fb,
                           tail_hook=tail_hook if TAIL_HOOK else None)
            for f in fa + fb:
                f()
            for f in wo_chunks_range(13, 16 if TAIL_HOOK else 12, rotate=True):
                f()
            if not TAIL_HOOK:
                for f in wo_chunks_range(12, 16, rotate=True):
                    f()

        for _rep in range(repeats):
            emit_body(_rep)

    # Force Exp and Ln onto the single combined table set so the table-load
    # pass emits one ACT_TABLE_LOAD for the whole kernel.
    from concourse.hw_specs import get_activation_tables
    tabs = get_activation_tables(nc.m.arch)
    for nm_, fs_ in tabs.items():
        if nm_ != "natural_log_exp_and_others":
            fs_.discard(AF.Exp)
            fs_.discard(AF.Ln)
    nc.compile()
    _CACHED[repeats] = nc
    return nc


def _host_prep(x, w_ln, wq, wk, wv, wo, cos, sin):
    bf = ml_dtypes.bfloat16
    f8 = mybir.dt.np(FP8)
    x = np.asarray(x, np.float32)
    w_ln = np.asarray(w_ln, np.float32)
    cosT = np.ascontiguousarray(np.asarray(cos, np.float32).T).astype(bf)
    sinTf = np.ascontiguousarray(np.asarray(sin, np.float32).T)
    sinTf[0:64] *= -1.0          # rotate_half sign folded into the table
    sinT = sinTf.astype(bf)
    xT = np.ascontiguousarray(x.T).astype(bf)

    # causal boundary mask for diagonal tiles: mask[p, f] = 1 if f >= p
    f = np.arange(128)[None, :]
    p = np.arange(128)[:, None]
    masks = (f >= p).astype(bf)

    ones128 = np.ones((128, 1), bf)
    ones_dr = np.ones((128, 2, 16), f8)

    wq_s = (np.asarray(wq, np.float32) * w_ln[None, :])
    wk_s = (np.asarray(wk, np.float32) * w_ln[None, :])
    wv_s = (np.asarray(wv, np.float32) * w_ln[None, :])
    wo32 = np.asarray(wo, np.float32)

    in_maps = []
    for c in range(N_CORES):
        sl = slice(c * NL, (c + 1) * NL)
        in_maps.append({
            "xT": xT,
            "wqT": np.ascontiguousarray(wq_s[sl].T).astype(bf),
            "wkT": np.ascontiguousarray(wk_s[sl].T).astype(bf),
            "wvT": np.ascontiguousarray(wv_s[sl].T).astype(bf),
            "woT": np.ascontiguousarray(wo32[:, sl].T).astype(bf),
            "cosT": cosT,
            "sinT": sinT,
            "masks": masks,
            "ones128": ones128,
            "ones_dr": ones_dr,
        })
    return in_maps


def kernel(x, w_ln, wq, wk, wv, wo, cos, sin):
    nc = _build_program()
    in_maps = _host_prep(x, w_ln, wq, wk, wv, wo, cos, sin)
    t0 = time.time()
    res = run_bass_kernel_spmd(nc, in_maps, core_ids=list(range(N_CORES)))
    t1 = time.time()
    print(f"run_bass_kernel_spmd wall: {(t1 - t0) * 1e3:.1f} ms", file=sys.stderr)
    acc = np.zeros((T, D), np.float32)
    for r in res.results:
        acc += np.asarray(r["out"], np.float32)
    return np.asarray(x, np.float32) + acc
